# revision 1
# baseline (speedup 1.0000x reference)
"""DogeDynamicMaskAttention Trainium2 kernel.

Sharding: 8 cores = 2 batches x 4 head-groups. Core c: batch b=c//4,
head-group g=c%4 -> heads [4g..4g+4), kv heads {2g, 2g+1}.

Device program (SPMD; identical program on all cores, different data):
  - q/k/v projections from xT as fp32r matmuls, outputs in transposed
    [out_dim, S] layout; SCALING folded into Wq on host.
  - dt = v_flat @ Wdt.T (all kv heads), dyn = exp(A * softplus(dt)).
  - exact per-head kthvalue threshold via 31-step bisection on float bits
    (dyn > 0 so float bits are monotonic; one fused DVE op per step).
  - RoPE via permutation-matmul rotate-half + DVE combine.
  - full SxS attention per head: scores psum = qk (fp32r) + rank-1 dyn-mask
    row + rank-1 causal-const row, DVE add for the non-constant (diagonal)
    mask blocks; exp with no max-subtraction (masked entries <= -1.7e38 so
    exp == 0); P * (1/l); PE transpose; attn@v; per-head output projection
    partials summed on host.
  - fully-masked (degenerate) rows give l == 0; host detects via the l
    output (and any non-finite rows) and recomputes those rows faithfully
    in numpy; expected count is ~1 row per (batch, head).
"""
import sys
import numpy as np

sys.path.insert(0, "/root/.axon_site/_ro/trn_rl_repo")

import concourse.bass as bass  # noqa: E402,F401
from concourse import bacc  # noqa: E402
import concourse.tile as tile  # noqa: E402
import concourse.mybir as mybir  # noqa: E402
from concourse.bass_utils import run_bass_kernel_spmd  # noqa: E402
from concourse.alu_op_type import AluOpType  # noqa: E402

F32 = mybir.dt.float32
F32R = mybir.dt.float32r
BF16 = mybir.dt.bfloat16
I32 = mybir.dt.int32
AF = mybir.ActivationFunctionType
AX = mybir.AxisListType.X

B, S, HID = 2, 2048, 2048
H, KV, D = 16, 8, 128
HPC, KVPC = 4, 2
GROUPS = H // KV
NUM_DYN = S // 2
SCALING = D ** -0.5
MIN = float(np.finfo(np.float32).min)
BIG = 1.7e38
P = 128
NT = S // P          # 16
NQ = 4
QW = S // NQ         # 512
NCORES = 8

_cache = {}


def _build_program(blkstate):
    key = ("nc", blkstate)
    if key in _cache:
        return _cache[key]
    nc = bacc.Bacc("TRN2", target_bir_lowering=False, debug=False,
                   num_devices=NCORES)
    dram = {}
    for name, shape in [
            ("xT", [HID, S]), ("wqT", [HID, HPC * D]), ("wkT", [HID, KVPC * D]),
            ("wvT", [HID, KVPC * D]), ("wdtvT", [HID, HPC]),
            ("woT", [HPC * D, HID]), ("acol", [HPC, 1]),
            ("cosT", [D, S]), ("sinT", [D, S]),
            ("varblk", [P, NT * P]),
            ("eye", [P, P]), ("perm", [P, P]), ("ones1", [1, P])]:
        dram[name] = nc.dram_tensor(name, shape, F32, kind="ExternalInput").ap()
    outT_d = nc.dram_tensor("outT", [HID, S], F32, kind="ExternalOutput").ap()
    dram["dyn_dr"] = nc.dram_tensor("dyn_dr", [HPC, S], F32R).ap()
    dram["vnat_dr"] = nc.dram_tensor("vnat_dr", [KVPC * NT * P, P], F32R).ap()
    l_d = nc.dram_tensor("l_out", [HPC, S], F32, kind="ExternalOutput").ap()

    with tile.TileContext(nc) as tc:
        _emit(nc, tc, dram, outT_d, l_d, blkstate)
    nc.compile()
    _cache[key] = nc
    return nc


def _emit(nc, tc, dram, outT_d, l_d, blkstate):
    from contextlib import ExitStack
    ctx = ExitStack()
    consts = ctx.enter_context(tc.tile_pool(name="consts", bufs=1))

    def cst(name, shape, src=None, as_f32r=False):
        t = consts.tile(shape, F32, name=f"c_{name}")
        nc.sync.dma_start(t[:], src if src is not None else dram[name])
        if as_f32r:
            r = consts.tile(shape, F32R, name=f"cr_{name}")
            nc.scalar.copy(r[:], t[:])
            return t, r
        return t

    eye_f, eye_r = cst("eye", [P, P], as_f32r=True)
    perm_t = cst("perm", [P, P])
    _, ones1_r = cst("ones1", [1, P], as_f32r=True)
    acol_t = cst("acol", [HPC, 1])
    # wdtvT packed [128, 16*4]: col cc*4+j = wdtvT[cc*128+p, j]
    wdtv_f = consts.tile([P, NT * HPC], F32, name="c_wdtvT")
    nc.sync.dma_start(wdtv_f[:].rearrange("p (c j) -> p c j", c=NT),
                      dram["wdtvT"].rearrange("(c p) j -> p c j", p=P))
    kthc = consts.tile([HPC, 1], F32, name="kthc")
    nc.vector.memset(kthc[:], float(NUM_DYN) - 0.5)

    act = ctx.enter_context(tc.tile_pool(name="act", bufs=1))
    qkro = [act.tile([P, S], F32R, name=f"qro{h}") for h in range(HPC)]
    kro = [act.tile([P, S], F32R, name=f"kro{i}") for i in range(KVPC)]

    with ExitStack() as ctx1:
        vop = ctx1.enter_context(tc.tile_pool(name="vop", bufs=1))
        vT_own = [vop.tile([P, S], F32R, name=f"vTown{i}") for i in range(KVPC)]
        dt_sb = vop.tile([HPC, S], F32, name="dt_sb")
        csp = ctx1.enter_context(tc.tile_pool(name="csp", bufs=1))
        cos_t = csp.tile([D, S], F32, name="cos_t")
        nc.sync.dma_start(cos_t[:], dram["cosT"])
        sin_t = csp.tile([D, S], F32, name="sin_t")
        nc.sync.dma_start(sin_t[:], dram["sinT"])

        # ---------------- dt first (enables early dyn/bisection) --------
        dyq = ctx1.enter_context(tc.tile_pool(name="dyq", bufs=1))
        with tc.tile_pool(name="dts", bufs=4) as dts, \
             tc.tile_pool(name="dps", bufs=2, space="PSUM") as dps:
            for sg in range(4):
                dtp = dps.tile([HPC, QW], F32, name="dtp", tag="dtp")
                for cc in range(NT):
                    x32 = dts.tile([P, QW], F32, name="x32", tag="x32")
                    nc.sync.dma_start(
                        x32[:], dram["xT"][cc * P:(cc + 1) * P,
                                           sg * QW:(sg + 1) * QW])
                    nc.tensor.matmul(dtp[:], wdtv_f[:, cc * HPC:(cc + 1) * HPC],
                                     x32[:], start=(cc == 0), stop=(cc == NT - 1))
                nc.scalar.copy(dt_sb[:, sg * QW:(sg + 1) * QW], dtp[:])

        # ---------------- dyn + kth bisection (overlaps projections) ----
        kth_f = dyq.tile([HPC, 1], I32, name="kth_f")
        dynrow = dyq.tile([HPC, S], F32R, name="dynrow")
        dyn_t = dyq.tile([HPC, S], F32, name="dyn_t")
        work = dyq.tile([HPC, S], F32, name="work")
        scr = dyq.tile([HPC, S], BF16, name="scr")
        scrf = dyq.tile([HPC, S], F32, name="scrf")
        nc.scalar.activation(work[:], dt_sb[:], AF.Exp)
        nc.scalar.activation(work[:], work[:], AF.Ln, bias=1.0)
        nc.scalar.activation(dyn_t[:], work[:], AF.Exp, scale=acol_t[:])
        lo = dyq.tile([HPC, 1], I32, name="lo")
        hi = dyq.tile([HPC, 1], I32, name="hi")
        mid = dyq.tile([HPC, 1], I32, name="mid")
        dlt = dyq.tile([HPC, 1], I32, name="dlt")
        cges = dyq.tile([HPC, 1], I32, name="cges")
        cltv = dyq.tile([HPC, 1], I32, name="cltv")
        cnt = dyq.tile([HPC, 1], F32, name="cnt")
        nc.vector.memset(lo[:], 0)
        nc.vector.memset(hi[:], 0x7F800000)
        for _ in range(31):
            nc.vector.tensor_tensor(dlt[:], hi[:], lo[:], op=AluOpType.subtract)
            nc.vector.tensor_scalar(dlt[:], dlt[:], 1, None,
                                    op0=AluOpType.arith_shift_right)
            nc.vector.tensor_tensor(mid[:], dlt[:], lo[:], op=AluOpType.add)
            nc.vector.tensor_scalar(scr[:], dyn_t[:],
                                    mid[:, 0:1].bitcast(F32), 0.0,
                                    op0=AluOpType.is_lt, op1=AluOpType.add,
                                    accum_out=cnt[:])
            nc.vector.tensor_scalar(cges[:], kthc[:], cnt[:, 0:1], None,
                                    op0=AluOpType.is_lt)
            nc.vector.tensor_scalar(cltv[:], kthc[:], cnt[:, 0:1], None,
                                    op0=AluOpType.is_ge)
            nc.vector.copy_predicated(hi[:], cges[:], mid[:])
            nc.vector.copy_predicated(lo[:], cltv[:], mid[:])
        nc.vector.tensor_copy(kth_f[:], lo[:])
        pen = scrf
        nc.vector.tensor_scalar(pen[:], dyn_t[:],
                                kth_f[:, 0:1].bitcast(F32), -BIG,
                                op0=AluOpType.is_lt, op1=AluOpType.mult)
        nc.vector.tensor_tensor(dynrow[:], dyn_t[:], pen[:], op=AluOpType.add)
        nc.sync.dma_start(dram["dyn_dr"], dynrow[:])

        # ---------------- projections ----------------
        with tc.tile_pool(name="xp", bufs=1) as xp, \
             tc.tile_pool(name="wp", bufs=2) as wp, \
             tc.tile_pool(name="pjp", bufs=5) as pjp, \
             tc.tile_pool(name="pps", bufs=8, space="PSUM") as pps:
            wname = {"v": "wvT", "q": "wqT", "k": "wkT"}
            OT = ([("v", i) for i in range(KVPC)]
                  + [("q", i) for i in range(HPC)]
                  + [("k", i) for i in range(KVPC)])
            for sg in range(4):
                xfull = xp.tile([P, NT * QW], F32R, name="xfull", tag="xf")
                nc.gpsimd.dma_start(
                    xfull[:].rearrange("p (c f) -> p c f", c=NT),
                    dram["xT"][:, sg * QW:(sg + 1) * QW]
                    .rearrange("(c p) f -> p c f", p=P))
                for kind, oi in OT:
                    wfull = wp.tile([P, NT * P], F32R, name="wfull", tag="wf")
                    nc.gpsimd.dma_start(
                        wfull[:].rearrange("p (c f) -> p c f", c=NT),
                        dram[wname[kind]][:, oi * P:(oi + 1) * P]
                        .rearrange("(c p) f -> p c f", p=P))
                    ps = pps.tile([P, QW], F32, name="ps", tag="ps")
                    for cc in range(NT):
                        nc.tensor.matmul(ps[:], wfull[:, cc * P:(cc + 1) * P],
                                         xfull[:, cc * QW:(cc + 1) * QW],
                                         start=(cc == 0), stop=(cc == NT - 1))
                    if kind == "v":
                        dst = vT_own[oi][:, sg * QW:(sg + 1) * QW]
                        nc.scalar.copy(dst, ps[:])
                    else:
                        f32t = pjp.tile([P, QW], F32, name="pj32", tag="pj")
                        nc.scalar.copy(f32t[:], ps[:])
                        dstro = (qkro[oi] if kind == "q" else kro[oi])
                        rh = pps.tile([P, QW], F32, name="rh", tag="ps")
                        nc.tensor.matmul(rh[:], perm_t[:], f32t[:],
                                         start=True, stop=True)
                        t1 = pjp.tile([P, QW], F32, name="t1", tag="pj")
                        nc.vector.tensor_tensor(
                            t1[:], rh[:], sin_t[:, sg * QW:(sg + 1) * QW],
                            op=AluOpType.mult)
                        t2 = pjp.tile([P, QW], F32, name="t2", tag="pj")
                        nc.vector.tensor_tensor(
                            t2[:], f32t[:], cos_t[:, sg * QW:(sg + 1) * QW],
                            op=AluOpType.mult)
                        nc.vector.tensor_tensor(
                            dstro[:, sg * QW:(sg + 1) * QW], t1[:], t2[:],
                            op=AluOpType.add)

        # ---------------- natural-layout v tiles (bounced via DRAM) ------
        with tc.tile_pool(name="vnb", bufs=4) as vnb, \
             tc.tile_pool(name="vps", bufs=4, space="PSUM") as vps:
            for i in range(KVPC):
                for cc in range(NT):
                    pt = vps.tile([P, P], F32, name="vt", tag="vt")
                    nc.tensor.transpose(pt[:].bitcast(F32R),
                                        vT_own[i][:, cc * P:(cc + 1) * P],
                                        eye_r[:])
                    vn = vnb.tile([P, P], F32R, name="vn", tag="vn")
                    nc.scalar.copy(vn[:], pt[:])
                    nc.sync.dma_start(
                        dram["vnat_dr"][(i * NT + cc) * P:(i * NT + cc + 1) * P, :],
                        vn[:])

    # ---------------- attention ----------------
    # blkstate[t][j] in {"Z", "M", "V:<idx>"}: zero / masked-const / varying
    # computed extent per tile: up to last non-M block
    ext = []
    for t in range(NT):
        nz = [j for j in range(NT) if blkstate[t][j] != "M"]
        ext.append((max(nz) + 1) * P if nz else 0)
    ares = ctx.enter_context(tc.tile_pool(name="ares", bufs=1))
    attnT = [ares.tile([P, S], F32R, name=f"attnT{h}") for h in range(HPC)]
    dynrow0 = [ares.tile([1, S], F32R, name=f"dynrow0_{h}") for h in range(HPC)]
    varblk_t = ares.tile([P, NT * P], F32, name="varblk_t")
    nc.sync.dma_start(varblk_t[:], dram["varblk"])
    for h in range(HPC):
        nc.sync.dma_start(dynrow0[h][:], dram["dyn_dr"][h:h + 1, :])
    with tc.tile_pool(name="ppl", bufs=6) as ppl, \
         tc.tile_pool(name="lpl", bufs=16) as lpl, \
         tc.tile_pool(name="ptl", bufs=6) as ptl, \
         tc.tile_pool(name="vnl", bufs=8) as vnl, \
         tc.tile_pool(name="aps", bufs=6, space="PSUM") as aps, \
         tc.tile_pool(name="ovl", bufs=2, space="PSUM") as ovl:
        for h in range(HPC):
            kv = h // GROUPS
            for grp in range(4):
                glim = max(ext[grp * 4 + tq] for tq in range(4))
                glim = ((glim + QW - 1) // QW) * QW  # pad group extent to 512
                ptiles = []
                for tq in range(4):
                    t = grp * 4 + tq
                    ptile = ppl.tile([P, S], F32R, name="ptile", tag="pt")
                    lparts = lpl.tile([P, NQ], F32, name="lparts", tag="lp")
                    nc.vector.memset(lparts[:], 0.0)
                    for qq in range(NQ):
                        q0 = qq * QW
                        e = min(max(ext[t] - q0, 0), QW)
                        if q0 >= glim:
                            break  # rest of group never read
                        if e == 0:
                            nc.vector.memset(ptile[:, q0:min(q0 + QW, glim)].bitcast(F32), 0.0)
                            nc.vector.memset(lparts[:, qq:qq + 1], 0.0)
                            continue
                        sc = aps.tile([P, QW], F32, name="sc", tag="aps")
                        nc.tensor.matmul(
                            sc[:, :e], qkro[h][:, t * P:(t + 1) * P],
                            kro[kv][:, q0:q0 + e],
                            start=True, stop=True, skip_group_check=True)
                        nc.tensor.matmul(
                            sc[:, :e], ones1_r[:], dynrow0[h][:, q0:q0 + e],
                            start=False, stop=True, skip_group_check=True)
                        for j in range(q0 // P, (q0 + e) // P):
                            st = blkstate[t][j]
                            if st.startswith("V"):
                                vi = int(st[2:])
                                off = j * P - q0
                                nc.vector.tensor_tensor(
                                    sc[:, off:off + P], sc[:, off:off + P],
                                    varblk_t[:, vi * P:(vi + 1) * P],
                                    op=AluOpType.add)
                        nc.scalar.activation(
                            ptile[:, q0:q0 + e], sc[:, :e], AF.Exp,
                            accum_out=lparts[:, qq:qq + 1])
                        if e < QW and q0 + e < glim:
                            nc.vector.memset(
                                ptile[:, q0 + e:min(q0 + QW, glim)]
                                .bitcast(F32), 0.0)
                    lsum = lpl.tile([P, 1], F32, name="lsum", tag="ls")
                    nc.vector.reduce_sum(lsum[:], lparts[:], axis=AX)
                    nc.sync.dma_start(
                        l_d[h:h + 1, t * P:(t + 1) * P].rearrange("a b -> b a"),
                        lsum[:])
                    linv = lpl.tile([P, 1], F32, name="linv", tag="ls")
                    nc.vector.reciprocal(linv[:], lsum[:])
                    nc.vector.tensor_scalar(ptile[:, :glim], ptile[:, :glim],
                                            linv[:, 0:1],
                                            None, op0=AluOpType.mult)
                    ptiles.append(ptile)
                ovp = ovl.tile([P, QW], F32, name="ovp", tag="ovp")
                nch = glim // P
                for cc in range(nch):
                    ptt = aps.tile([P, QW], F32, name="ptt", tag="aps")
                    for tq in range(4):
                        nc.tensor.transpose(
                            ptt[:, tq * P:(tq + 1) * P].bitcast(F32R),
                            ptiles[tq][:, cc * P:(cc + 1) * P], eye_r[:])
                    pts = ptl.tile([P, QW], F32R, name="pts", tag="pts")
                    nc.vector.tensor_copy(pts[:], ptt[:])
                    vn = vnl.tile([P, P], F32R, name="vnt", tag="vnt")
                    nc.sync.dma_start(
                        vn[:], dram["vnat_dr"]
                        [(kv * NT + cc) * P:(kv * NT + cc + 1) * P, :])
                    nc.tensor.matmul(ovp[:], vn[:], pts[:],
                                     start=(cc == 0), stop=(cc == nch - 1),
                                     skip_group_check=True)
                nc.scalar.copy(attnT[h][:, grp * QW:(grp + 1) * QW], ovp[:])

    # ---------------- output projection ----------------
    with tc.tile_pool(name="wol", bufs=2) as wol, \
         tc.tile_pool(name="oub", bufs=4) as oub, \
         tc.tile_pool(name="ops", bufs=4, space="PSUM") as ops:
        for ht in range(NT):
            wo = wol.tile([P, HPC * P], F32R, name="wo", tag="wo")
            nc.gpsimd.dma_start(
                wo[:].rearrange("p (h f) -> p h f", h=HPC),
                dram["woT"][:, ht * P:(ht + 1) * P]
                .rearrange("(h p) f -> p h f", p=P))
            for sg in range(4):
                op = ops.tile([P, QW], F32, name="op", tag="op")
                for h in range(HPC):
                    nc.tensor.matmul(op[:], wo[:, h * P:(h + 1) * P],
                                     attnT[h][:, sg * QW:(sg + 1) * QW],
                                     start=(h == 0), stop=(h == HPC - 1))
                ot = oub.tile([P, QW], F32, name="ot", tag="ot")
                nc.scalar.copy(ot[:], op[:])
                nc.sync.dma_start(
                    outT_d[ht * P:(ht + 1) * P, sg * QW:(sg + 1) * QW], ot[:])
    ctx.close()


def _host_prep(hidden_states, cos, sin, attention_mask, Wq, Wk, Wv, A, Wdt, Wo):
    eye = np.eye(P, dtype=np.float32)
    perm = np.zeros((P, P), dtype=np.float32)
    for j in range(64):
        perm[j + 64, j] = -1.0
        perm[j, j + 64] = 1.0
    ones1 = np.ones((1, P), dtype=np.float32)

    in_maps = []
    blkstates = []
    for c in range(NCORES):
        b, g = divmod(c, 4)
        heads = list(range(4 * g, 4 * g + 4))
        wvT = np.ascontiguousarray(Wv[2 * g * D:(2 * g + 2) * D].T)
        wdtvT = np.ascontiguousarray(
            (Wdt[heads].astype(np.float64) @ Wv.astype(np.float64))
            .T.astype(np.float32))
        xT = np.ascontiguousarray(hidden_states[b].T)
        wqT = np.ascontiguousarray(
            (Wq[4 * g * D:(4 * g + 4) * D] * np.float32(SCALING)).T)
        wkT = np.ascontiguousarray(Wk[2 * g * D:(2 * g + 2) * D].T)
        woT = np.ascontiguousarray(Wo[:, 4 * g * D:(4 * g + 4) * D].T)
        acol = A[heads].astype(np.float32).reshape(HPC, 1)
        cosT = np.ascontiguousarray(cos[b].T)
        sinT = np.ascontiguousarray(sin[b].T)
        m = attention_mask[b, 0]
        mb = m.reshape(NT, P, NT, P)
        blkrows = []
        varlist = []
        for t in range(NT):
            row = []
            for j in range(NT):
                blkv = mb[t, :, j, :]
                if np.all(blkv == 0):
                    row.append("Z")
                elif np.all(blkv <= -1e30):
                    row.append("M")
                else:
                    row.append(f"V:{len(varlist)}")
                    varlist.append(np.maximum(blkv, -BIG))
            # interior M blocks (before a later non-M block) become varying
            nz = [j for j in range(NT) if row[j] != "M"]
            lim = (max(nz) + 1) if nz else 0
            for j in range(lim):
                if row[j] == "M":
                    row[j] = f"V:{len(varlist)}"
                    varlist.append(np.full((P, P), -BIG, np.float32))
            blkrows.append(tuple(row))
        if len(varlist) > NT:
            raise NotImplementedError("too many varying mask blocks")
        varblk = np.zeros((P, NT * P), dtype=np.float32)
        for vi, blkv in enumerate(varlist):
            varblk[:, vi * P:(vi + 1) * P] = blkv
        blkstate = tuple(blkrows)
        in_maps.append({
            "xT": xT, "wqT": wqT, "wkT": wkT, "wvT": wvT, "wdtvT": wdtvT,
            "woT": woT, "acol": acol, "cosT": cosT, "sinT": sinT,
            "varblk": varblk, "eye": eye, "perm": perm,
            "ones1": ones1,
        })
        blkstates.append(blkstate)
    if len(set(blkstates)) != 1:
        raise NotImplementedError("mask structure differs across batches")
    return in_maps, blkstates[0]


def _softplus64(x):
    x = x.astype(np.float64)
    return np.log1p(np.exp(-np.abs(x))) + np.maximum(x, 0)


def _repair_rows(out, bad, inputs):
    """Recompute rows flagged bad [B, S] with faithful numpy reference math."""
    if not bad.any():
        return out
    hs = inputs["hidden_states"]; cos = inputs["cos"]; sin = inputs["sin"]
    am = inputs["attention_mask"]; Wq = inputs["Wq"]; Wk = inputs["Wk"]
    Wv = inputs["Wv"]; A = inputs["A"]; Wdt = inputs["Wdt"]; Wo = inputs["Wo"]

    def rope(x, c, s):
        x1, x2 = x[..., :D // 2], x[..., D // 2:]
        return x * c + np.concatenate([-x2, x1], axis=-1) * s

    for b in range(B):
        rows = np.where(bad[b])[0]
        if len(rows) == 0:
            continue
        x = hs[b].astype(np.float32)
        k = (x @ Wk.T).reshape(S, KV, D)
        v = (x @ Wv.T).reshape(S, KV, D)
        k = rope(k, cos[b][:, None, :], sin[b][:, None, :])
        v_flat = v.reshape(S, KV * D)
        dt = v_flat @ Wdt.T
        dyn = np.exp(A[None, :] * _softplus64(dt)).astype(np.float32).T
        kth = np.sort(dyn, axis=-1)[:, NUM_DYN - 1:NUM_DYN]
        dmask = np.where(dyn < kth, MIN, dyn).astype(np.float32)
        for s_i in rows:
            q_row = (x[s_i] @ Wq.T).reshape(H, D)
            q_row = rope(q_row, cos[b][s_i][None, :], sin[b][s_i][None, :])
            attn_row = np.zeros((H, D), dtype=np.float32)
            for h in range(H):
                kvh = h // GROUPS
                sc = ((q_row[h] @ k[:, kvh].T) * np.float32(SCALING)
                      + (dmask[h] + am[b, 0, s_i])).astype(np.float32)
                w = np.exp(sc - sc.max())
                w = (w / w.sum()).astype(np.float32)
                attn_row[h] = w @ v[:, kvh]
            out[b, s_i] = attn_row.reshape(H * D) @ Wo.T
    return out


def kernel(**inputs):
    inputs = {k: np.asarray(v) for k, v in inputs.items()}
    in_maps, blkstate = _host_prep(**inputs)
    nc = _build_program(blkstate)
    res = run_bass_kernel_spmd(nc, in_maps, list(range(NCORES)))
    out = np.zeros((B, S, HID), dtype=np.float32)
    bad = np.zeros((B, S), dtype=bool)
    for c in range(NCORES):
        b = c // 4
        out[b] += res.results[c]["outT"].T
        bad[b] |= (res.results[c]["l_out"] == 0).any(axis=0)
    bad |= ~np.isfinite(out).all(axis=2)
    out = _repair_rows(out, bad, inputs)
    return out



# revision 27
# speedup vs baseline: 1.2686x; 1.2686x over previous
"""DogeDynamicMaskAttention Trainium2 kernel (transposed-scores redesign).

Sharding: 8 cores = 2 batches x 4 head-groups. Core c: batch b=c//4,
head-group g=c%4 -> heads [4g..4g+4), kv heads {2g, 2g+1}.

Design vs previous baseline:
  - scores computed TRANSPOSED [keys, queries]: the dynamic mask row is a
    per-partition (per-key) bias folded into the exp activation for free;
    the P-matrix transposes + f32r casts of the old layout vanish; the
    attn@v matmul consumes exp output directly (keys on partitions).
  - l (softmax denom) via a ones-column stationary matmul accumulated in
    psum; normalize out tiles with reciprocal + gpsimd partition_broadcast
    + one DVE multiply per (head, query-group).
  - projections in bf16 (x and Wq/Wk/Wv/Wdt host-packed contiguous, so
    DMA is large-descriptor); x resident in SBUF, read once.
  - v natural-layout tiles kept in SBUF (no DRAM bounce).
  - dyn/kth bisection identical to baseline (31-step float-bit bisection),
    overlapped under the q/k/v projections; dynT obtained by tiny PE
    transposes instead of a DRAM round trip.
  - fully-masked (degenerate) rows: l==0 detected on host via l output,
    recomputed faithfully in numpy (expected ~1 row per batch*head).
"""
import sys
import numpy as np
import ml_dtypes

BF16NP = ml_dtypes.bfloat16

sys.path.insert(0, "/root/.axon_site/_ro/trn_rl_repo")

import concourse.bass as bass  # noqa: E402,F401
from concourse import bacc  # noqa: E402
import concourse.tile as tile  # noqa: E402
import concourse.mybir as mybir  # noqa: E402
from concourse.bass_utils import run_bass_kernel_spmd  # noqa: E402
from concourse.alu_op_type import AluOpType  # noqa: E402

F32 = mybir.dt.float32
F32R = mybir.dt.float32r
BF16 = mybir.dt.bfloat16
I32 = mybir.dt.int32
AF = mybir.ActivationFunctionType
AX = mybir.AxisListType.X

B, S, HID = 2, 2048, 2048
H, KV, D = 16, 8, 128
HPC, KVPC = 4, 2
GROUPS = H // KV
NUM_DYN = S // 2
SCALING = D ** -0.5
MIN = float(np.finfo(np.float32).min)
BIG = 1.7e38
P = 128
NT = S // P          # 16
NQ = 4
QW = S // NQ         # 512
NCORES = 8

_cache = {}


def _build_program(blkstate):
    key = ("nc", blkstate)
    if key in _cache:
        return _cache[key]
    nvar = _num_varblocks(blkstate)
    nc = bacc.Bacc("TRN2", target_bir_lowering=False, debug=False,
                   num_devices=NCORES)
    dram = {}
    for name, shape, dt in [
            ("xP", [P, NQ * NT * QW], BF16),
            ("xPf", [P, NQ * NT * QW], F32R),
            ("wqP", [P, HPC * NT * P], BF16),
            ("wkP", [P, KVPC * NT * P], BF16),
            ("wvP", [P, KVPC * NT * P], BF16),
            ("wdtvPr", [P, NT * HPC], F32R),
            ("woP", [P, NT * HPC * P], F32),
            ("cosT", [D, S], F32), ("sinT", [D, S], F32),
            ("acol", [HPC, 1], F32),
            ("eye", [P, P], F32), ("perm", [P, P], F32),
            ("varblkT", [P, max(nvar, 1) * P], F32)]:
        dram[name] = nc.dram_tensor(name, shape, dt, kind="ExternalInput").ap()
    outT_d = nc.dram_tensor("outT", [HID, S], F32, kind="ExternalOutput").ap()
    l_d = nc.dram_tensor("l_out", [HPC, S], F32, kind="ExternalOutput").ap()

    with tile.TileContext(nc) as tc:
        _emit(nc, tc, dram, outT_d, l_d, blkstate)
    nc.compile()
    _cache[key] = nc
    return nc


def _num_varblocks(blkstate):
    n = 0
    for t in range(NT):
        for j in range(NT):
            if blkstate[t][j].startswith("V"):
                n = max(n, int(blkstate[t][j][2:]) + 1)
    return n


def _emit(nc, tc, dram, outT_d, l_d, blkstate):
    from contextlib import ExitStack
    ctx = ExitStack()

    # per-tile computed extent (in key chunks): chunks j < extc[t] participate
    extc = []
    for t in range(NT):
        nz = [j for j in range(NT) if blkstate[t][j] != "M"]
        assert nz and min(nz) == 0, "chunk 0 must be active for every tile"
        extc.append(max(nz) + 1)

    consts = ctx.enter_context(tc.tile_pool(name="consts", bufs=1))

    def cst(name, shape):
        t = consts.tile(shape, F32, name=f"c_{name}")
        nc.sync.dma_start(t[:], dram[name])
        return t

    # f32r consts staged through a temp pool so the f32 copies are freed
    eye_r = consts.tile([P, P], F32R, name="cr_eye")
    perm_r = consts.tile([P, P], F32R, name="cr_perm")
    with tc.tile_pool(name="cstg", bufs=2) as cstg:
        for nm, dst in [("eye", eye_r), ("perm", perm_r)]:
            t = cstg.tile([P, P], F32, name=f"s_{nm}", tag="s")
            nc.sync.dma_start(t[:], dram[nm])
            nc.scalar.copy(dst[:], t[:])
    acol_t = cst("acol", [HPC, 1])
    nvar = _num_varblocks(blkstate)
    varblkT = cst("varblkT", [P, max(nvar, 1) * P])
    wdtv = consts.tile([P, NT * HPC], F32R, name="c_wdtv")
    nc.sync.dma_start(wdtv[:], dram["wdtvPr"])
    onescol_b = consts.tile([P, 1], BF16, name="onescol")
    nc.vector.memset(onescol_b[:], 1.0)
    kthc = consts.tile([HPC, 1], F32, name="kthc")
    nc.vector.memset(kthc[:], float(NUM_DYN) - 0.5)

    csp = ctx.enter_context(tc.tile_pool(name="csp", bufs=1))
    cos_t = csp.tile([D, S], F32, name="cos_t")
    nc.sync.dma_start(cos_t[:], dram["cosT"])
    sin_t = csp.tile([D, S], F32, name="sin_t")
    nc.sync.dma_start(sin_t[:], dram["sinT"])

    # x resident in SBUF, bf16, one contiguous load per query-group
    xsp = ctx.enter_context(tc.tile_pool(name="xsp", bufs=1))
    xs = []
    for sg in range(NQ):
        xt = xsp.tile([P, NT * QW], BF16, name=f"xs{sg}")
        nc.sync.dma_start(xt[:], dram["xP"][:, sg * NT * QW:(sg + 1) * NT * QW])
        xs.append(xt)

    act = ctx.enter_context(tc.tile_pool(name="act", bufs=1))
    qkro = [act.tile([P, S], F32R, name=f"qro{h}") for h in range(HPC)]
    kro = [act.tile([P, S], F32R, name=f"kro{i}") for i in range(KVPC)]
    vnat = [act.tile([P, NT * P], BF16, name=f"vnat{i}") for i in range(KVPC)]
    dynT = act.tile([P, NT * HPC], F32, name="dynT")

    with ExitStack() as ctx1:
        vop = ctx1.enter_context(tc.tile_pool(name="vop", bufs=1))
        vT_own = [vop.tile([P, S], F32R, name=f"vTown{i}") for i in range(KVPC)]
        dt_sb = vop.tile([HPC, S], F32, name="dt_sb")

        # ---------------- dt first (enables early dyn/bisection) --------
        # dt must be f32-accurate: it decides the kthvalue mask set, and
        # bf16 dt flips enough borderline keys to breach the error budget.
        with tc.tile_pool(name="dps", bufs=2, space="PSUM") as dps, \
             tc.tile_pool(name="dtx", bufs=4) as dtx:
            for sg in range(NQ):
                dtp = dps.tile([HPC, QW], F32, name="dtp", tag="dtp")
                for cc in range(NT):
                    xf = dtx.tile([P, QW], F32R, name="xf", tag="xf")
                    nc.sync.dma_start(
                        xf[:], dram["xPf"][:, (sg * NT + cc) * QW:
                                           (sg * NT + cc + 1) * QW])
                    nc.tensor.matmul(dtp[:], wdtv[:, cc * HPC:(cc + 1) * HPC],
                                     xf[:],
                                     start=(cc == 0), stop=(cc == NT - 1))
                nc.scalar.copy(dt_sb[:, sg * QW:(sg + 1) * QW], dtp[:])

        # ---------------- dyn + kth bisection (overlaps projections) ----
        dyq = ctx1.enter_context(tc.tile_pool(name="dyq", bufs=1))
        kth_f = dyq.tile([HPC, 1], I32, name="kth_f")
        dynrow = dyq.tile([HPC, S], F32R, name="dynrow")
        dyn_t = dyq.tile([HPC, S], F32, name="dyn_t")
        work = dyq.tile([HPC, S], F32, name="work")
        # work is dead after the dyn chain; reuse its storage for the
        # bisection scratch (bf16 view) and later the penalty tile
        scr = work[:].bitcast(BF16)[:, 0:S]
        pen = work
        nc.scalar.activation(work[:], dt_sb[:], AF.Exp)
        nc.scalar.activation(work[:], work[:], AF.Ln, bias=1.0)
        nc.scalar.activation(dyn_t[:], work[:], AF.Exp, scale=acol_t[:])
        lo = dyq.tile([HPC, 1], I32, name="lo")
        hi = dyq.tile([HPC, 1], I32, name="hi")
        mid = dyq.tile([HPC, 1], I32, name="mid")
        dlt = dyq.tile([HPC, 1], I32, name="dlt")
        cges = dyq.tile([HPC, 1], I32, name="cges")
        cltv = dyq.tile([HPC, 1], I32, name="cltv")
        cnt = dyq.tile([HPC, 1], F32, name="cnt")
        nc.vector.memset(lo[:], 0)
        nc.vector.memset(hi[:], 0x7F800000)
        for _ in range(31):
            nc.vector.tensor_tensor(dlt[:], hi[:], lo[:], op=AluOpType.subtract)
            nc.vector.tensor_scalar(dlt[:], dlt[:], 1, None,
                                    op0=AluOpType.arith_shift_right)
            nc.vector.tensor_tensor(mid[:], dlt[:], lo[:], op=AluOpType.add)
            nc.vector.tensor_scalar(scr, dyn_t[:],
                                    mid[:, 0:1].bitcast(F32), 0.0,
                                    op0=AluOpType.is_lt, op1=AluOpType.add,
                                    accum_out=cnt[:])
            nc.vector.tensor_scalar(cges[:], kthc[:], cnt[:, 0:1], None,
                                    op0=AluOpType.is_lt)
            nc.vector.tensor_scalar(cltv[:], kthc[:], cnt[:, 0:1], None,
                                    op0=AluOpType.is_ge)
            nc.vector.copy_predicated(hi[:], cges[:], mid[:])
            nc.vector.copy_predicated(lo[:], cltv[:], mid[:])
        nc.vector.tensor_copy(kth_f[:], lo[:])
        nc.vector.tensor_scalar(pen[:], dyn_t[:],
                                kth_f[:, 0:1].bitcast(F32), -BIG,
                                op0=AluOpType.is_lt, op1=AluOpType.mult)
        nc.vector.tensor_tensor(dynrow[:], dyn_t[:], pen[:], op=AluOpType.add)
        # transpose dynrow [4, S] -> dynT [S-part, head]: per key chunk c a
        # [4,128] -> [128,4] PE transpose; bias layout dynT[:, c*HPC+h]
        with tc.tile_pool(name="dtp2", bufs=1, space="PSUM") as dtp2:
            dyn_ps = dtp2.tile([P, NT * HPC], F32, name="dyn_ps")
            for cc in range(NT):
                nc.tensor.transpose(
                    dyn_ps[:, cc * HPC:(cc + 1) * HPC].bitcast(F32R),
                    dynrow[:, cc * P:(cc + 1) * P], eye_r[0:HPC, 0:HPC])
            nc.scalar.copy(dynT[:], dyn_ps[:])

        # ---------------- projections (bf16) + RoPE ----------------
        with tc.tile_pool(name="wp", bufs=2) as wp, \
             tc.tile_pool(name="pjp", bufs=4) as pjp, \
             tc.tile_pool(name="pps", bufs=6, space="PSUM") as pps:
            wsrc = {"v": "wvP", "q": "wqP", "k": "wkP"}
            OT = ([("v", i) for i in range(KVPC)]
                  + [("q", i) for i in range(HPC)]
                  + [("k", i) for i in range(KVPC)])
            for kind, oi in OT:
                wfull = wp.tile([P, NT * P], BF16, name="wfull", tag="wf")
                nc.sync.dma_start(
                    wfull[:],
                    dram[wsrc[kind]][:, oi * NT * P:(oi + 1) * NT * P])
                for sg in range(NQ):
                    ps = pps.tile([P, QW], F32, name="ps", tag="ps")
                    for cc in range(NT):
                        nc.tensor.matmul(ps[:], wfull[:, cc * P:(cc + 1) * P],
                                         xs[sg][:, cc * QW:(cc + 1) * QW],
                                         start=(cc == 0), stop=(cc == NT - 1))
                    if kind == "v":
                        nc.scalar.copy(vT_own[oi][:, sg * QW:(sg + 1) * QW],
                                       ps[:])
                    else:
                        pj = pjp.tile([P, QW], F32R, name="pj", tag="pj")
                        nc.scalar.copy(pj[:], ps[:])
                        rh = pps.tile([P, QW], F32, name="rh", tag="ps")
                        nc.tensor.matmul(rh[:], perm_r[:], pj[:],
                                         start=True, stop=True)
                        t1 = pjp.tile([P, QW], F32, name="t1", tag="pj")
                        nc.vector.tensor_tensor(
                            t1[:], rh[:], sin_t[:, sg * QW:(sg + 1) * QW],
                            op=AluOpType.mult)
                        t2 = pjp.tile([P, QW], F32, name="t2", tag="pj")
                        nc.gpsimd.tensor_tensor(
                            t2[:], pj[:], cos_t[:, sg * QW:(sg + 1) * QW],
                            op=AluOpType.mult)
                        dstro = (qkro[oi] if kind == "q" else kro[oi])
                        nc.vector.tensor_tensor(
                            dstro[:, sg * QW:(sg + 1) * QW], t1[:], t2[:],
                            op=AluOpType.add)

        # ---------------- natural-layout v tiles (SBUF resident) --------
        with tc.tile_pool(name="vps", bufs=4, space="PSUM") as vps:
            for i in range(KVPC):
                for cc in range(NT):
                    pt = vps.tile([P, P], F32, name="vt", tag="vt")
                    nc.tensor.transpose(pt[:].bitcast(F32R),
                                        vT_own[i][:, cc * P:(cc + 1) * P],
                                        eye_r[:])
                    nc.scalar.copy(vnat[i][:, cc * P:(cc + 1) * P], pt[:])

    # ---------------- attention (transposed scores) + outproj ----------
    with tc.tile_pool(name="scp", bufs=2, space="PSUM") as scp, \
         tc.tile_pool(name="ovl", bufs=2, space="PSUM") as ovl, \
         tc.tile_pool(name="lpp", bufs=2, space="PSUM") as lpp, \
         tc.tile_pool(name="ptp", bufs=3) as ptp, \
         tc.tile_pool(name="atn", bufs=8) as atn, \
         tc.tile_pool(name="lnb", bufs=2) as lnb, \
         tc.tile_pool(name="lnv", bufs=2) as lnv, \
         tc.tile_pool(name="wol", bufs=2) as wol, \
         tc.tile_pool(name="oub", bufs=3) as oub, \
         tc.tile_pool(name="ops", bufs=2, space="PSUM") as ops:
        for grp in range(NQ):
            base = grp * QW
            tiles = list(range(grp * 4, grp * 4 + 4))
            jmax = max(extc[t] for t in tiles)
            at_grp = {}
            for h in range(HPC):
                kv = h // GROUPS
                ovp = ovl.tile([P, QW], F32, name="ovp", tag="ovp")
                lps = lpp.tile([1, QW], F32, name="lps", tag="lps")
                for j in range(jmax):
                    acts = [t for t in tiles if j < extc[t]]
                    assert acts == tiles[-len(acts):], \
                        "active tiles must be a suffix of the group"
                    qlo = acts[0] * P - base
                    sc = scp.tile([P, QW], F32, name="sc", tag="sc")
                    nc.tensor.matmul(
                        sc[:, qlo:QW], kro[kv][:, j * P:(j + 1) * P],
                        qkro[h][:, base + qlo:base + QW],
                        start=True, stop=True, skip_group_check=True)
                    for t in acts:
                        st = blkstate[t][j]
                        if st.startswith("V"):
                            vi = int(st[2:])
                            off = t * P - base
                            nc.vector.tensor_tensor(
                                sc[:, off:off + P], sc[:, off:off + P],
                                varblkT[:, vi * P:(vi + 1) * P],
                                op=AluOpType.add)
                    pt = ptp.tile([P, QW], BF16, name="pt", tag="pt")
                    nc.scalar.activation(
                        pt[:, qlo:QW], sc[:, qlo:QW], AF.Exp,
                        bias=dynT[:, j * HPC + h:j * HPC + h + 1])
                    nc.tensor.matmul(
                        lps[:, qlo:QW], onescol_b[:], pt[:, qlo:QW],
                        start=(j == 0), stop=(j == jmax - 1),
                        skip_group_check=True)
                    nc.tensor.matmul(
                        ovp[:, qlo:QW], vnat[kv][:, j * P:(j + 1) * P],
                        pt[:, qlo:QW],
                        start=(j == 0), stop=(j == jmax - 1),
                        skip_group_check=True)
                lsb = lnv.tile([1, QW], F32, name="lsb", tag="lv")
                nc.scalar.copy(lsb[:], lps[:])
                nc.sync.dma_start(l_d[h:h + 1, base:base + QW], lsb[:])
                linv = lnv.tile([1, QW], F32, name="linv", tag="lv")
                nc.vector.reciprocal(linv[:], lps[:])
                lbc = lnb.tile([P, QW], F32, name="lbc", tag="lb")
                nc.gpsimd.partition_broadcast(lbc[:], linv[:])
                at = atn.tile([P, QW], F32R, name="at", tag="at")
                nc.vector.tensor_tensor(at[:], ovp[:], lbc[:],
                                        op=AluOpType.mult)
                at_grp[h] = at
            for ht in range(NT):
                wo = wol.tile([P, HPC * P], F32R, name="wo", tag="wo")
                nc.gpsimd.dma_start(
                    wo[:], dram["woP"][:, ht * HPC * P:(ht + 1) * HPC * P])
                op = ops.tile([P, QW], F32, name="op", tag="op")
                for h in range(HPC):
                    nc.tensor.matmul(op[:], wo[:, h * P:(h + 1) * P],
                                     at_grp[h][:], start=(h == 0),
                                     stop=(h == HPC - 1))
                osb = oub.tile([P, QW], F32, name="osb", tag="ob")
                nc.scalar.copy(osb[:], op[:])
                nc.sync.dma_start(
                    outT_d[ht * P:(ht + 1) * P, base:base + QW], osb[:])
    ctx.close()


def _host_prep(hidden_states, cos, sin, attention_mask, Wq, Wk, Wv, A, Wdt, Wo):
    eye = np.eye(P, dtype=np.float32)
    perm = np.zeros((P, P), dtype=np.float32)
    for j in range(64):
        perm[j + 64, j] = -1.0
        perm[j, j + 64] = 1.0

    def pack_w(wT, nblk):
        # wT [HID, nblk*P] f32 -> [P, nblk*NT*P] bf16:
        # [p, (oi*NT+c)*P+f] = wT[c*P+p, oi*P+f]
        w4 = wT.reshape(NT, P, nblk, P)            # [c, p, oi, f]
        return np.ascontiguousarray(
            w4.transpose(1, 2, 0, 3).reshape(P, nblk * NT * P)
        ).astype(np.float32)

    in_maps = []
    blkstates = []
    for c in range(NCORES):
        b, g = divmod(c, 4)
        heads = list(range(4 * g, 4 * g + 4))
        # x packed: [p, ((sg*NT)+c)*QW+f] = x[b][sg*QW+f, c*P+p]
        xb = np.asarray(hidden_states[b], dtype=np.float32)
        xP = np.ascontiguousarray(
            xb.reshape(NQ, QW, NT, P).transpose(3, 0, 2, 1)
            .reshape(P, NQ * NT * QW))
        wqT = (Wq[4 * g * D:(4 * g + 4) * D] * np.float32(SCALING)).T
        wkT = Wk[2 * g * D:(2 * g + 2) * D].T
        wvT = Wv[2 * g * D:(2 * g + 2) * D].T
        wqP = pack_w(np.ascontiguousarray(wqT), HPC)
        wkP = pack_w(np.ascontiguousarray(wkT), KVPC)
        wvP = pack_w(np.ascontiguousarray(wvT), KVPC)
        wdtvT = np.ascontiguousarray(
            (Wdt[heads].astype(np.float64) @ Wv.astype(np.float64))
            .T.astype(np.float32))                 # [HID, 4]
        wdtvP = np.ascontiguousarray(
            wdtvT.reshape(NT, P, HPC).transpose(1, 0, 2).reshape(P, NT * HPC))
        # woP: [p, (ht*HPC+h)*P+f] = WoT[h*P+p, ht*P+f]
        woT = np.ascontiguousarray(Wo[:, 4 * g * D:(4 * g + 4) * D].T)
        woP = np.ascontiguousarray(
            woT.reshape(HPC, P, NT, P).transpose(1, 2, 0, 3)
            .reshape(P, NT * HPC * P))
        acol = A[heads].astype(np.float32).reshape(HPC, 1)
        cosT = np.ascontiguousarray(cos[b].T)
        sinT = np.ascontiguousarray(sin[b].T)
        m = attention_mask[b, 0]
        mb = np.asarray(m).reshape(NT, P, NT, P)
        blkrows = []
        varlist = []
        varkeys = {}
        for t in range(NT):
            row = []
            for j in range(NT):
                blkv = mb[t, :, j, :]
                if np.all(blkv == 0):
                    row.append("Z")
                elif np.all(blkv <= -1e30):
                    row.append("M")
                else:
                    bT = np.ascontiguousarray(
                        np.maximum(blkv, -BIG).T)  # [key, query]
                    kk = bT.tobytes()
                    if kk not in varkeys:
                        varkeys[kk] = len(varlist)
                        varlist.append(bT)
                    row.append(f"V:{varkeys[kk]}")
            # interior M blocks (before a later non-M block) become varying
            nz = [j for j in range(NT) if row[j] != "M"]
            lim = (max(nz) + 1) if nz else 0
            for j in range(lim):
                if row[j] == "M":
                    bT = np.full((P, P), -BIG, np.float32)
                    kk = bT.tobytes()
                    if kk not in varkeys:
                        varkeys[kk] = len(varlist)
                        varlist.append(bT)
                    row[j] = f"V:{varkeys[kk]}"
            blkrows.append(tuple(row))
        if len(varlist) > 8:
            raise NotImplementedError("too many varying mask blocks")
        varblkT = np.zeros((P, max(len(varlist), 1) * P), dtype=np.float32)
        for vi, blkv in enumerate(varlist):
            varblkT[:, vi * P:(vi + 1) * P] = blkv
        blkstate = tuple(blkrows)
        in_maps.append({
            "xP": xP.astype(BF16NP), "xPf": xP,
            "wqP": wqP.astype(BF16NP),
            "wkP": wkP.astype(BF16NP), "wvP": wvP.astype(BF16NP),
            "wdtvPr": wdtvP, "woP": woP, "cosT": cosT,
            "sinT": sinT, "acol": acol, "eye": eye, "perm": perm,
            "varblkT": varblkT,
        })
        blkstates.append(blkstate)
    if len(set(blkstates)) != 1:
        raise NotImplementedError("mask structure differs across batches")
    return in_maps, blkstates[0]


def _softplus64(x):
    x = x.astype(np.float64)
    return np.log1p(np.exp(-np.abs(x))) + np.maximum(x, 0)


def _repair_rows(out, bad, inputs):
    """Recompute rows flagged bad [B, S] with faithful numpy reference math."""
    if not bad.any():
        return out
    hs = inputs["hidden_states"]; cos = inputs["cos"]; sin = inputs["sin"]
    am = inputs["attention_mask"]; Wq = inputs["Wq"]; Wk = inputs["Wk"]
    Wv = inputs["Wv"]; A = inputs["A"]; Wdt = inputs["Wdt"]; Wo = inputs["Wo"]

    def rope(x, c, s):
        x1, x2 = x[..., :D // 2], x[..., D // 2:]
        return x * c + np.concatenate([-x2, x1], axis=-1) * s

    for b in range(B):
        rows = np.where(bad[b])[0]
        if len(rows) == 0:
            continue
        x = hs[b].astype(np.float32)
        k = (x @ Wk.T).reshape(S, KV, D)
        v = (x @ Wv.T).reshape(S, KV, D)
        k = rope(k, cos[b][:, None, :], sin[b][:, None, :])
        v_flat = v.reshape(S, KV * D)
        dt = v_flat @ Wdt.T
        dyn = np.exp(A[None, :] * _softplus64(dt)).astype(np.float32).T
        kth = np.sort(dyn, axis=-1)[:, NUM_DYN - 1:NUM_DYN]
        dmask = np.where(dyn < kth, MIN, dyn).astype(np.float32)
        for s_i in rows:
            q_row = (x[s_i] @ Wq.T).reshape(H, D)
            q_row = rope(q_row, cos[b][s_i][None, :], sin[b][s_i][None, :])
            attn_row = np.zeros((H, D), dtype=np.float32)
            for h in range(H):
                kvh = h // GROUPS
                sc = ((q_row[h] @ k[:, kvh].T) * np.float32(SCALING)
                      + (dmask[h] + am[b, 0, s_i])).astype(np.float32)
                w = np.exp(sc - sc.max())
                w = (w / w.sum()).astype(np.float32)
                attn_row[h] = w @ v[:, kvh]
            out[b, s_i] = attn_row.reshape(H * D) @ Wo.T
    return out


def kernel(**inputs):
    inputs = {k: np.asarray(v) for k, v in inputs.items()}
    in_maps, blkstate = _host_prep(**inputs)
    nc = _build_program(blkstate)
    res = run_bass_kernel_spmd(nc, in_maps, list(range(NCORES)))
    out = np.zeros((B, S, HID), dtype=np.float32)
    bad = np.zeros((B, S), dtype=bool)
    for c in range(NCORES):
        b = c // 4
        out[b] += res.results[c]["outT"].T
        bad[b] |= (res.results[c]["l_out"] == 0).any(axis=0)
    bad |= ~np.isfinite(out).all(axis=2)
    out = _repair_rows(out, bad, inputs)
    return out


# revision 32
# speedup vs baseline: 1.5368x; 1.2114x over previous
"""DogeDynamicMaskAttention Trainium2 kernel (transposed-scores redesign).

Sharding: 8 cores = 2 batches x 4 head-groups. Core c: batch b=c//4,
head-group g=c%4 -> heads [4g..4g+4), kv heads {2g, 2g+1}.

Design vs previous baseline:
  - scores computed TRANSPOSED [keys, queries]: the dynamic mask row is a
    per-partition (per-key) bias folded into the exp activation for free;
    the P-matrix transposes + f32r casts of the old layout vanish; the
    attn@v matmul consumes exp output directly (keys on partitions).
  - l (softmax denom) via a ones-column stationary matmul accumulated in
    psum; normalize out tiles with reciprocal + gpsimd partition_broadcast
    + one DVE multiply per (head, query-group).
  - projections in bf16 (x and Wq/Wk/Wv/Wdt host-packed contiguous, so
    DMA is large-descriptor); x resident in SBUF, read once.
  - v natural-layout tiles kept in SBUF (no DRAM bounce).
  - dyn/kth bisection identical to baseline (31-step float-bit bisection),
    overlapped under the q/k/v projections; dynT obtained by tiny PE
    transposes instead of a DRAM round trip.
  - fully-masked (degenerate) rows: l==0 detected on host via l output,
    recomputed faithfully in numpy (expected ~1 row per batch*head).
"""
import sys
import numpy as np
import ml_dtypes

BF16NP = ml_dtypes.bfloat16

sys.path.insert(0, "/root/.axon_site/_ro/trn_rl_repo")

import concourse.bass as bass  # noqa: E402,F401
from concourse import bacc  # noqa: E402
import concourse.tile as tile  # noqa: E402
import concourse.mybir as mybir  # noqa: E402
from concourse.bass_utils import run_bass_kernel_spmd  # noqa: E402
from concourse.alu_op_type import AluOpType  # noqa: E402

F32 = mybir.dt.float32
F32R = mybir.dt.float32r
BF16 = mybir.dt.bfloat16
I32 = mybir.dt.int32
AF = mybir.ActivationFunctionType
AX = mybir.AxisListType.X

B, S, HID = 2, 2048, 2048
H, KV, D = 16, 8, 128
HPC, KVPC = 4, 2
GROUPS = H // KV
NUM_DYN = S // 2
SCALING = D ** -0.5
MIN = float(np.finfo(np.float32).min)
BIG = 1.7e38
P = 128
NT = S // P          # 16
NQ = 4
QW = S // NQ         # 512
NCORES = 8

_cache = {}


def _build_program(blkstate):
    key = ("nc", blkstate)
    if key in _cache:
        return _cache[key]
    nvar = _num_varblocks(blkstate)
    nc = bacc.Bacc("TRN2", target_bir_lowering=False, debug=False,
                   num_devices=NCORES)
    dram = {}
    for name, shape, dt in [
            ("xP", [P, NQ * NT * QW], BF16),
            ("xPf", [P, NQ * NT * QW], F32R),
            ("wqP", [P, HPC * NT * P], BF16),
            ("wkP", [P, KVPC * NT * P], BF16),
            ("wvP", [P, KVPC * NT * P], BF16),
            ("wdtvPr", [P, NT * HPC], F32R),
            ("woP", [P, NT * HPC * P], F32),
            ("cosT", [D, S], F32), ("sinT", [D, S], F32),
            ("acol", [HPC, 1], F32),
            ("eye", [P, P], F32), ("perm", [P, P], F32),
            ("varblkT", [P, max(nvar, 1) * P], F32)]:
        dram[name] = nc.dram_tensor(name, shape, dt, kind="ExternalInput").ap()
    outT_d = nc.dram_tensor("outT", [HID, S], F32, kind="ExternalOutput").ap()
    l_d = nc.dram_tensor("l_out", [HPC, S], F32, kind="ExternalOutput").ap()

    with tile.TileContext(nc) as tc:
        _emit(nc, tc, dram, outT_d, l_d, blkstate)
    nc.compile()
    _cache[key] = nc
    return nc


def _num_varblocks(blkstate):
    n = 0
    for t in range(NT):
        for j in range(NT):
            if blkstate[t][j].startswith("V"):
                n = max(n, int(blkstate[t][j][2:]) + 1)
    return n


def _emit(nc, tc, dram, outT_d, l_d, blkstate):
    from contextlib import ExitStack
    ctx = ExitStack()

    # per-tile computed extent (in key chunks): chunks j < extc[t] participate
    extc = []
    for t in range(NT):
        nz = [j for j in range(NT) if blkstate[t][j] != "M"]
        assert nz and min(nz) == 0, "chunk 0 must be active for every tile"
        extc.append(max(nz) + 1)

    consts = ctx.enter_context(tc.tile_pool(name="consts", bufs=1))

    # dt-critical consts first on the sync ring so the dt pass starts
    # immediately; all bulk loads go on the Activation DGE ring.
    wdtv = consts.tile([P, NT * HPC], F32R, name="c_wdtv")
    nc.sync.dma_start(wdtv[:], dram["wdtvPr"])
    acol_t = consts.tile([HPC, 1], F32, name="c_acol")
    nc.sync.dma_start(acol_t[:], dram["acol"])
    onescol_b = consts.tile([P, 1], BF16, name="onescol")
    nc.vector.memset(onescol_b[:], 1.0)
    kthc = consts.tile([HPC, 1], F32, name="kthc")
    nc.vector.memset(kthc[:], float(NUM_DYN) - 0.5)

    eye_r = consts.tile([P, P], F32R, name="cr_eye")
    perm_r = consts.tile([P, P], F32R, name="cr_perm")
    nvar = _num_varblocks(blkstate)
    varblkT = consts.tile([P, max(nvar, 1) * P], F32, name="c_varblkT")

    csp = ctx.enter_context(tc.tile_pool(name="csp", bufs=1))
    cos_t = csp.tile([D, S], F32, name="cos_t")
    sin_t = csp.tile([D, S], F32, name="sin_t")
    xsp = ctx.enter_context(tc.tile_pool(name="xsp", bufs=1))
    xs = [xsp.tile([P, NT * QW], BF16, name=f"xs{sg}") for sg in range(NQ)]

    def bulk_loads():
        for sg in range(NQ):
            nc.scalar.dma_start(
                xs[sg][:], dram["xP"][:, sg * NT * QW:(sg + 1) * NT * QW])
        nc.scalar.dma_start(cos_t[:], dram["cosT"])
        nc.scalar.dma_start(sin_t[:], dram["sinT"])
        nc.scalar.dma_start(varblkT[:], dram["varblkT"])
        with tc.tile_pool(name="cstg", bufs=2) as cstg:
            for nm, dst in [("eye", eye_r), ("perm", perm_r)]:
                t = cstg.tile([P, P], F32, name=f"s_{nm}", tag="s")
                nc.scalar.dma_start(t[:], dram[nm])
                nc.scalar.copy(dst[:], t[:])

    act = ctx.enter_context(tc.tile_pool(name="act", bufs=1))
    qkro = [act.tile([P, S], F32R, name=f"qro{h}") for h in range(HPC)]
    kro = [act.tile([P, S], F32R, name=f"kro{i}") for i in range(KVPC)]
    vnat = [act.tile([P, NT * P], BF16, name=f"vnat{i}") for i in range(KVPC)]
    dynT = act.tile([P, NT * HPC], F32, name="dynT")

    with ExitStack() as ctx1:
        vop = ctx1.enter_context(tc.tile_pool(name="vop", bufs=1))
        vT_own = [vop.tile([P, S], F32R, name=f"vTown{i}") for i in range(KVPC)]
        dt_sb = vop.tile([HPC, S], F32, name="dt_sb")

        # ---------------- dt first (enables early dyn/bisection) --------
        # dt must be f32-accurate: it decides the kthvalue mask set, and
        # bf16 dt flips enough borderline keys to breach the error budget.
        with tc.tile_pool(name="dps", bufs=2, space="PSUM") as dps, \
             tc.tile_pool(name="dtx", bufs=4) as dtx:
            for sg in range(NQ):
                dtp = dps.tile([HPC, QW], F32, name="dtp", tag="dtp")
                for cc in range(NT):
                    xf = dtx.tile([P, QW], F32R, name="xf", tag="xf")
                    nc.sync.dma_start(
                        xf[:], dram["xPf"][:, (sg * NT + cc) * QW:
                                           (sg * NT + cc + 1) * QW])
                    nc.tensor.matmul(dtp[:], wdtv[:, cc * HPC:(cc + 1) * HPC],
                                     xf[:],
                                     start=(cc == 0), stop=(cc == NT - 1))
                nc.scalar.copy(dt_sb[:, sg * QW:(sg + 1) * QW], dtp[:])
        bulk_loads()

        # ---------------- dyn + kth bisection (overlaps projections) ----
        dyq = ctx1.enter_context(tc.tile_pool(name="dyq", bufs=1))
        kth_f = dyq.tile([HPC, 1], I32, name="kth_f")
        dynrow = dyq.tile([HPC, S], F32R, name="dynrow")
        dyn_t = dyq.tile([HPC, S], F32, name="dyn_t")
        work = dyq.tile([HPC, S], F32, name="work")
        # work is dead after the dyn chain; reuse its storage for the
        # bisection scratch (bf16 view) and later the penalty tile
        scr = work[:].bitcast(BF16)[:, 0:S]
        pen = work
        nc.scalar.activation(work[:], dt_sb[:], AF.Exp)
        nc.scalar.activation(work[:], work[:], AF.Ln, bias=1.0)
        nc.scalar.activation(dyn_t[:], work[:], AF.Exp, scale=acol_t[:])
        lo = dyq.tile([HPC, 1], I32, name="lo")
        hi = dyq.tile([HPC, 1], I32, name="hi")
        mid = dyq.tile([HPC, 1], I32, name="mid")
        dlt = dyq.tile([HPC, 1], I32, name="dlt")
        cges = dyq.tile([HPC, 1], I32, name="cges")
        cltv = dyq.tile([HPC, 1], I32, name="cltv")
        cnt = dyq.tile([HPC, 1], F32, name="cnt")
        nc.vector.memset(lo[:], 0)
        nc.vector.memset(hi[:], 0x7F800000)
        for _ in range(31):
            nc.vector.tensor_tensor(dlt[:], hi[:], lo[:], op=AluOpType.subtract)
            nc.vector.tensor_scalar(dlt[:], dlt[:], 1, None,
                                    op0=AluOpType.arith_shift_right)
            nc.vector.tensor_tensor(mid[:], dlt[:], lo[:], op=AluOpType.add)
            nc.vector.tensor_scalar(scr, dyn_t[:],
                                    mid[:, 0:1].bitcast(F32), 0.0,
                                    op0=AluOpType.is_lt, op1=AluOpType.add,
                                    accum_out=cnt[:])
            nc.vector.tensor_scalar(cges[:], kthc[:], cnt[:, 0:1], None,
                                    op0=AluOpType.is_lt)
            nc.vector.tensor_scalar(cltv[:], kthc[:], cnt[:, 0:1], None,
                                    op0=AluOpType.is_ge)
            nc.vector.copy_predicated(hi[:], cges[:], mid[:])
            nc.vector.copy_predicated(lo[:], cltv[:], mid[:])
        nc.vector.tensor_copy(kth_f[:], lo[:])
        nc.vector.tensor_scalar(pen[:], dyn_t[:],
                                kth_f[:, 0:1].bitcast(F32), -BIG,
                                op0=AluOpType.is_lt, op1=AluOpType.mult)
        nc.vector.tensor_tensor(dynrow[:], dyn_t[:], pen[:], op=AluOpType.add)

        # ---------------- projections (bf16) + RoPE ----------------
        with tc.tile_pool(name="wp", bufs=2) as wp, \
             tc.tile_pool(name="pjp", bufs=4) as pjp, \
             tc.tile_pool(name="pps", bufs=6, space="PSUM") as pps:
            wsrc = {"v": "wvP", "q": "wqP", "k": "wkP"}
            OT = ([("v", i) for i in range(KVPC)]
                  + [("q", i) for i in range(HPC)]
                  + [("k", i) for i in range(KVPC)])
            for kind, oi in OT:
                wfull = wp.tile([P, NT * P], BF16, name="wfull", tag="wf")
                nc.sync.dma_start(
                    wfull[:],
                    dram[wsrc[kind]][:, oi * NT * P:(oi + 1) * NT * P])
                for sg in range(NQ):
                    ps = pps.tile([P, QW], F32, name="ps", tag="ps")
                    for cc in range(NT):
                        nc.tensor.matmul(ps[:], wfull[:, cc * P:(cc + 1) * P],
                                         xs[sg][:, cc * QW:(cc + 1) * QW],
                                         start=(cc == 0), stop=(cc == NT - 1))
                    if kind == "v":
                        nc.scalar.copy(vT_own[oi][:, sg * QW:(sg + 1) * QW],
                                       ps[:])
                    else:
                        pj = pjp.tile([P, QW], F32R, name="pj", tag="pj")
                        nc.scalar.copy(pj[:], ps[:])
                        rh = pps.tile([P, QW], F32, name="rh", tag="ps")
                        nc.tensor.matmul(rh[:], perm_r[:], pj[:],
                                         start=True, stop=True)
                        t1 = pjp.tile([P, QW], F32, name="t1", tag="pj")
                        nc.vector.tensor_tensor(
                            t1[:], rh[:], sin_t[:, sg * QW:(sg + 1) * QW],
                            op=AluOpType.mult)
                        t2 = pjp.tile([P, QW], F32, name="t2", tag="pj")
                        nc.gpsimd.tensor_tensor(
                            t2[:], pj[:], cos_t[:, sg * QW:(sg + 1) * QW],
                            op=AluOpType.mult)
                        dstro = (qkro[oi] if kind == "q" else kro[oi])
                        nc.vector.tensor_tensor(
                            dstro[:, sg * QW:(sg + 1) * QW], t1[:], t2[:],
                            op=AluOpType.add)

        # ---------------- natural-layout v tiles (SBUF resident) --------
        with tc.tile_pool(name="vps", bufs=4, space="PSUM") as vps:
            for i in range(KVPC):
                for cc in range(NT):
                    pt = vps.tile([P, P], F32, name="vt", tag="vt")
                    nc.tensor.transpose(pt[:].bitcast(F32R),
                                        vT_own[i][:, cc * P:(cc + 1) * P],
                                        eye_r[:])
                    nc.scalar.copy(vnat[i][:, cc * P:(cc + 1) * P], pt[:])

        # dynT transposes last in the PE queue before attention: they wait
        # on the DVE bisection, so anything emitted after them would stall
        # the in-order PE queue (cost a 122us bubble when emitted early).
        with tc.tile_pool(name="dtp2", bufs=1, space="PSUM") as dtp2:
            dyn_ps = dtp2.tile([P, NT * HPC], F32, name="dyn_ps")
            for cc in range(NT):
                nc.tensor.transpose(
                    dyn_ps[:, cc * HPC:(cc + 1) * HPC].bitcast(F32R),
                    dynrow[:, cc * P:(cc + 1) * P], eye_r[0:HPC, 0:HPC])
            nc.scalar.copy(dynT[:], dyn_ps[:])

    # ---------------- attention (transposed scores) + outproj ----------
    with tc.tile_pool(name="scp", bufs=2, space="PSUM") as scp, \
         tc.tile_pool(name="ovl", bufs=2, space="PSUM") as ovl, \
         tc.tile_pool(name="lpp", bufs=2, space="PSUM") as lpp, \
         tc.tile_pool(name="ptp", bufs=3) as ptp, \
         tc.tile_pool(name="atn", bufs=8) as atn, \
         tc.tile_pool(name="lnb", bufs=2) as lnb, \
         tc.tile_pool(name="lnv", bufs=2) as lnv, \
         tc.tile_pool(name="wol", bufs=2) as wol, \
         tc.tile_pool(name="oub", bufs=3) as oub, \
         tc.tile_pool(name="ops", bufs=2, space="PSUM") as ops:
        for grp in range(NQ):
            base = grp * QW
            tiles = list(range(grp * 4, grp * 4 + 4))
            jmax = max(extc[t] for t in tiles)
            at_grp = {}
            for h in range(HPC):
                kv = h // GROUPS
                ovp = ovl.tile([P, QW], F32, name="ovp", tag="ovp")
                lps = lpp.tile([1, QW], F32, name="lps", tag="lps")
                for j in range(jmax):
                    acts = [t for t in tiles if j < extc[t]]
                    assert acts == tiles[-len(acts):], \
                        "active tiles must be a suffix of the group"
                    qlo = acts[0] * P - base
                    sc = scp.tile([P, QW], F32, name="sc", tag="sc")
                    nc.tensor.matmul(
                        sc[:, qlo:QW], kro[kv][:, j * P:(j + 1) * P],
                        qkro[h][:, base + qlo:base + QW],
                        start=True, stop=True, skip_group_check=True)
                    for t in acts:
                        st = blkstate[t][j]
                        if st.startswith("V"):
                            vi = int(st[2:])
                            off = t * P - base
                            nc.vector.tensor_tensor(
                                sc[:, off:off + P], sc[:, off:off + P],
                                varblkT[:, vi * P:(vi + 1) * P],
                                op=AluOpType.add)
                    pt = ptp.tile([P, QW], BF16, name="pt", tag="pt")
                    nc.scalar.activation(
                        pt[:, qlo:QW], sc[:, qlo:QW], AF.Exp,
                        bias=dynT[:, j * HPC + h:j * HPC + h + 1])
                    nc.tensor.matmul(
                        lps[:, qlo:QW], onescol_b[:], pt[:, qlo:QW],
                        start=(j == 0), stop=(j == jmax - 1),
                        skip_group_check=True)
                    nc.tensor.matmul(
                        ovp[:, qlo:QW], vnat[kv][:, j * P:(j + 1) * P],
                        pt[:, qlo:QW],
                        start=(j == 0), stop=(j == jmax - 1),
                        skip_group_check=True)
                lsb = lnv.tile([1, QW], F32, name="lsb", tag="lv")
                nc.scalar.copy(lsb[:], lps[:])
                nc.sync.dma_start(l_d[h:h + 1, base:base + QW], lsb[:])
                linv = lnv.tile([1, QW], F32, name="linv", tag="lv")
                nc.vector.reciprocal_approx_fast(linv[:], lps[:])
                lbc = lnb.tile([P, QW], F32, name="lbc", tag="lb")
                nc.gpsimd.partition_broadcast(lbc[:], linv[:])
                at = atn.tile([P, QW], F32R, name="at", tag="at")
                nc.vector.tensor_tensor(at[:], ovp[:], lbc[:],
                                        op=AluOpType.mult)
                at_grp[h] = at
            for ht in range(NT):
                wo = wol.tile([P, HPC * P], F32R, name="wo", tag="wo")
                nc.gpsimd.dma_start(
                    wo[:], dram["woP"][:, ht * HPC * P:(ht + 1) * HPC * P])
                op = ops.tile([P, QW], F32, name="op", tag="op")
                for h in range(HPC):
                    nc.tensor.matmul(op[:], wo[:, h * P:(h + 1) * P],
                                     at_grp[h][:], start=(h == 0),
                                     stop=(h == HPC - 1))
                osb = oub.tile([P, QW], F32, name="osb", tag="ob")
                nc.scalar.copy(osb[:], op[:])
                nc.sync.dma_start(
                    outT_d[ht * P:(ht + 1) * P, base:base + QW], osb[:])
    ctx.close()


def _host_prep(hidden_states, cos, sin, attention_mask, Wq, Wk, Wv, A, Wdt, Wo):
    eye = np.eye(P, dtype=np.float32)
    perm = np.zeros((P, P), dtype=np.float32)
    for j in range(64):
        perm[j + 64, j] = -1.0
        perm[j, j + 64] = 1.0

    def pack_w(wT, nblk):
        # wT [HID, nblk*P] f32 -> [P, nblk*NT*P] bf16:
        # [p, (oi*NT+c)*P+f] = wT[c*P+p, oi*P+f]
        w4 = wT.reshape(NT, P, nblk, P)            # [c, p, oi, f]
        return np.ascontiguousarray(
            w4.transpose(1, 2, 0, 3).reshape(P, nblk * NT * P)
        ).astype(np.float32)

    in_maps = []
    blkstates = []
    for c in range(NCORES):
        b, g = divmod(c, 4)
        heads = list(range(4 * g, 4 * g + 4))
        # x packed: [p, ((sg*NT)+c)*QW+f] = x[b][sg*QW+f, c*P+p]
        xb = np.asarray(hidden_states[b], dtype=np.float32)
        xP = np.ascontiguousarray(
            xb.reshape(NQ, QW, NT, P).transpose(3, 0, 2, 1)
            .reshape(P, NQ * NT * QW))
        wqT = (Wq[4 * g * D:(4 * g + 4) * D] * np.float32(SCALING)).T
        wkT = Wk[2 * g * D:(2 * g + 2) * D].T
        wvT = Wv[2 * g * D:(2 * g + 2) * D].T
        wqP = pack_w(np.ascontiguousarray(wqT), HPC)
        wkP = pack_w(np.ascontiguousarray(wkT), KVPC)
        wvP = pack_w(np.ascontiguousarray(wvT), KVPC)
        wdtvT = np.ascontiguousarray(
            (Wdt[heads].astype(np.float64) @ Wv.astype(np.float64))
            .T.astype(np.float32))                 # [HID, 4]
        wdtvP = np.ascontiguousarray(
            wdtvT.reshape(NT, P, HPC).transpose(1, 0, 2).reshape(P, NT * HPC))
        # woP: [p, (ht*HPC+h)*P+f] = WoT[h*P+p, ht*P+f]
        woT = np.ascontiguousarray(Wo[:, 4 * g * D:(4 * g + 4) * D].T)
        woP = np.ascontiguousarray(
            woT.reshape(HPC, P, NT, P).transpose(1, 2, 0, 3)
            .reshape(P, NT * HPC * P))
        acol = A[heads].astype(np.float32).reshape(HPC, 1)
        cosT = np.ascontiguousarray(cos[b].T)
        sinT = np.ascontiguousarray(sin[b].T)
        m = attention_mask[b, 0]
        mb = np.asarray(m).reshape(NT, P, NT, P)
        blkrows = []
        varlist = []
        varkeys = {}
        for t in range(NT):
            row = []
            for j in range(NT):
                blkv = mb[t, :, j, :]
                if np.all(blkv == 0):
                    row.append("Z")
                elif np.all(blkv <= -1e30):
                    row.append("M")
                else:
                    bT = np.ascontiguousarray(
                        np.maximum(blkv, -BIG).T)  # [key, query]
                    kk = bT.tobytes()
                    if kk not in varkeys:
                        varkeys[kk] = len(varlist)
                        varlist.append(bT)
                    row.append(f"V:{varkeys[kk]}")
            # interior M blocks (before a later non-M block) become varying
            nz = [j for j in range(NT) if row[j] != "M"]
            lim = (max(nz) + 1) if nz else 0
            for j in range(lim):
                if row[j] == "M":
                    bT = np.full((P, P), -BIG, np.float32)
                    kk = bT.tobytes()
                    if kk not in varkeys:
                        varkeys[kk] = len(varlist)
                        varlist.append(bT)
                    row[j] = f"V:{varkeys[kk]}"
            blkrows.append(tuple(row))
        if len(varlist) > 8:
            raise NotImplementedError("too many varying mask blocks")
        varblkT = np.zeros((P, max(len(varlist), 1) * P), dtype=np.float32)
        for vi, blkv in enumerate(varlist):
            varblkT[:, vi * P:(vi + 1) * P] = blkv
        blkstate = tuple(blkrows)
        in_maps.append({
            "xP": xP.astype(BF16NP), "xPf": xP,
            "wqP": wqP.astype(BF16NP),
            "wkP": wkP.astype(BF16NP), "wvP": wvP.astype(BF16NP),
            "wdtvPr": wdtvP, "woP": woP, "cosT": cosT,
            "sinT": sinT, "acol": acol, "eye": eye, "perm": perm,
            "varblkT": varblkT,
        })
        blkstates.append(blkstate)
    if len(set(blkstates)) != 1:
        raise NotImplementedError("mask structure differs across batches")
    return in_maps, blkstates[0]


def _softplus64(x):
    x = x.astype(np.float64)
    return np.log1p(np.exp(-np.abs(x))) + np.maximum(x, 0)


def _repair_rows(out, bad, inputs):
    """Recompute rows flagged bad [B, S] with faithful numpy reference math."""
    if not bad.any():
        return out
    hs = inputs["hidden_states"]; cos = inputs["cos"]; sin = inputs["sin"]
    am = inputs["attention_mask"]; Wq = inputs["Wq"]; Wk = inputs["Wk"]
    Wv = inputs["Wv"]; A = inputs["A"]; Wdt = inputs["Wdt"]; Wo = inputs["Wo"]

    def rope(x, c, s):
        x1, x2 = x[..., :D // 2], x[..., D // 2:]
        return x * c + np.concatenate([-x2, x1], axis=-1) * s

    for b in range(B):
        rows = np.where(bad[b])[0]
        if len(rows) == 0:
            continue
        x = hs[b].astype(np.float32)
        k = (x @ Wk.T).reshape(S, KV, D)
        v = (x @ Wv.T).reshape(S, KV, D)
        k = rope(k, cos[b][:, None, :], sin[b][:, None, :])
        v_flat = v.reshape(S, KV * D)
        dt = v_flat @ Wdt.T
        dyn = np.exp(A[None, :] * _softplus64(dt)).astype(np.float32).T
        kth = np.sort(dyn, axis=-1)[:, NUM_DYN - 1:NUM_DYN]
        dmask = np.where(dyn < kth, MIN, dyn).astype(np.float32)
        for s_i in rows:
            q_row = (x[s_i] @ Wq.T).reshape(H, D)
            q_row = rope(q_row, cos[b][s_i][None, :], sin[b][s_i][None, :])
            attn_row = np.zeros((H, D), dtype=np.float32)
            for h in range(H):
                kvh = h // GROUPS
                sc = ((q_row[h] @ k[:, kvh].T) * np.float32(SCALING)
                      + (dmask[h] + am[b, 0, s_i])).astype(np.float32)
                w = np.exp(sc - sc.max())
                w = (w / w.sum()).astype(np.float32)
                attn_row[h] = w @ v[:, kvh]
            out[b, s_i] = attn_row.reshape(H * D) @ Wo.T
    return out


def kernel(**inputs):
    inputs = {k: np.asarray(v) for k, v in inputs.items()}
    in_maps, blkstate = _host_prep(**inputs)
    nc = _build_program(blkstate)
    res = run_bass_kernel_spmd(nc, in_maps, list(range(NCORES)))
    out = np.zeros((B, S, HID), dtype=np.float32)
    bad = np.zeros((B, S), dtype=bool)
    for c in range(NCORES):
        b = c // 4
        out[b] += res.results[c]["outT"].T
        bad[b] |= (res.results[c]["l_out"] == 0).any(axis=0)
    bad |= ~np.isfinite(out).all(axis=2)
    out = _repair_rows(out, bad, inputs)
    return out


# revision 43
# speedup vs baseline: 1.5422x; 1.0035x over previous
"""DogeDynamicMaskAttention Trainium2 kernel (transposed-scores redesign).

Sharding: 8 cores = 2 batches x 4 head-groups. Core c: batch b=c//4,
head-group g=c%4 -> heads [4g..4g+4), kv heads {2g, 2g+1}.

Design vs previous baseline:
  - scores computed TRANSPOSED [keys, queries]: the dynamic mask row is a
    per-partition (per-key) bias folded into the exp activation for free;
    the P-matrix transposes + f32r casts of the old layout vanish; the
    attn@v matmul consumes exp output directly (keys on partitions).
  - l (softmax denom) via a ones-column stationary matmul accumulated in
    psum; normalize out tiles with reciprocal + gpsimd partition_broadcast
    + one DVE multiply per (head, query-group).
  - projections in bf16 (x and Wq/Wk/Wv/Wdt host-packed contiguous, so
    DMA is large-descriptor); x resident in SBUF, read once.
  - v natural-layout tiles kept in SBUF (no DRAM bounce).
  - dyn/kth bisection identical to baseline (31-step float-bit bisection),
    overlapped under the q/k/v projections; dynT obtained by tiny PE
    transposes instead of a DRAM round trip.
  - fully-masked (degenerate) rows: l==0 detected on host via l output,
    recomputed faithfully in numpy (expected ~1 row per batch*head).
"""
import sys
import numpy as np
import ml_dtypes

BF16NP = ml_dtypes.bfloat16

sys.path.insert(0, "/root/.axon_site/_ro/trn_rl_repo")

import concourse.bass as bass  # noqa: E402,F401
from concourse import bacc  # noqa: E402
import concourse.tile as tile  # noqa: E402
import concourse.mybir as mybir  # noqa: E402
from concourse.bass_utils import run_bass_kernel_spmd  # noqa: E402
from concourse.alu_op_type import AluOpType  # noqa: E402

F32 = mybir.dt.float32
F32R = mybir.dt.float32r
BF16 = mybir.dt.bfloat16
I32 = mybir.dt.int32
AF = mybir.ActivationFunctionType
AX = mybir.AxisListType.X

B, S, HID = 2, 2048, 2048
H, KV, D = 16, 8, 128
HPC, KVPC = 4, 2
GROUPS = H // KV
NUM_DYN = S // 2
SCALING = D ** -0.5
MIN = float(np.finfo(np.float32).min)
BIG = 1.7e38
P = 128
NT = S // P          # 16
NQ = 4
QW = S // NQ         # 512
NCORES = 8

_cache = {}


def _build_program(blkstate):
    key = ("nc", blkstate)
    if key in _cache:
        return _cache[key]
    nvar = _num_varblocks(blkstate)
    nc = bacc.Bacc("TRN2", target_bir_lowering=False, debug=False,
                   num_devices=NCORES)
    dram = {}
    for name, shape, dt in [
            ("xP", [P, NQ * NT * QW], BF16),
            ("xPf", [P, NQ * NT * QW], F32R),
            ("wqP", [P, HPC * NT * P], BF16),
            ("wkP", [P, KVPC * NT * P], BF16),
            ("wvP", [P, KVPC * NT * P], BF16),
            ("wdtvPr", [P, NT * HPC], F32R),
            ("woP", [P, NT * HPC * P], F32),
            ("cosT", [D, S], F32), ("sinT", [D, S], F32),
            ("acol", [HPC, 1], F32),
            ("eye", [P, P], F32), ("perm", [P, P], F32),
            ("varblkQ", [P, max(nvar, 1) * P], BF16)]:
        dram[name] = nc.dram_tensor(name, shape, dt, kind="ExternalInput").ap()
    outT_d = nc.dram_tensor("outT", [HID, S], F32, kind="ExternalOutput").ap()
    l_d = nc.dram_tensor("l_out", [HPC, S], F32, kind="ExternalOutput").ap()

    with tile.TileContext(nc) as tc:
        _emit(nc, tc, dram, outT_d, l_d, blkstate)
    nc.compile()
    _cache[key] = nc
    return nc


def _num_varblocks(blkstate):
    n = 0
    for t in range(NT):
        for j in range(NT):
            if blkstate[t][j].startswith("V"):
                n = max(n, int(blkstate[t][j][2:]) + 1)
    return n


def _emit(nc, tc, dram, outT_d, l_d, blkstate):
    from contextlib import ExitStack
    ctx = ExitStack()

    # per-tile computed extent (in key chunks): chunks j < extc[t] participate
    extc = []
    for t in range(NT):
        nz = [j for j in range(NT) if blkstate[t][j] != "M"]
        assert nz and min(nz) == 0, "chunk 0 must be active for every tile"
        extc.append(max(nz) + 1)

    consts = ctx.enter_context(tc.tile_pool(name="consts", bufs=1))

    # dt-critical consts first on the sync ring so the dt pass starts
    # immediately; all bulk loads go on the Activation DGE ring.
    wdtv = consts.tile([P, NT * HPC], F32R, name="c_wdtv")
    nc.sync.dma_start(wdtv[:], dram["wdtvPr"])
    acol_t = consts.tile([HPC, 1], F32, name="c_acol")
    nc.sync.dma_start(acol_t[:], dram["acol"])
    onescol_b = consts.tile([P, 1], BF16, name="onescol")
    nc.vector.memset(onescol_b[:], 1.0)
    kthc = consts.tile([HPC, 1], F32, name="kthc")
    nc.vector.memset(kthc[:], float(NUM_DYN) - 0.5)

    eye_r = consts.tile([P, P], F32R, name="cr_eye")
    eye_b = consts.tile([P, P], BF16, name="cr_eye_b")
    perm_r = consts.tile([P, P], F32R, name="cr_perm")
    nvar = _num_varblocks(blkstate)
    # untransposed mask blocks, bf16, used as PE matmul-add stationaries
    varblkQ = consts.tile([P, max(nvar, 1) * P], BF16, name="c_varblkQ")

    csp = ctx.enter_context(tc.tile_pool(name="csp", bufs=1))
    cos_t = csp.tile([D, S], F32, name="cos_t")
    sin_t = csp.tile([D, S], F32, name="sin_t")
    xsp = ctx.enter_context(tc.tile_pool(name="xsp", bufs=1))
    xs = [xsp.tile([P, NT * QW], BF16, name=f"xs{sg}") for sg in range(NQ)]

    def bulk_loads():
        for sg in range(NQ):
            nc.scalar.dma_start(
                xs[sg][:], dram["xP"][:, sg * NT * QW:(sg + 1) * NT * QW])
        nc.scalar.dma_start(cos_t[:], dram["cosT"])
        nc.scalar.dma_start(sin_t[:], dram["sinT"])
        nc.scalar.dma_start(varblkQ[:], dram["varblkQ"])
        with tc.tile_pool(name="cstg", bufs=2) as cstg:
            for nm, dst in [("eye", eye_r), ("perm", perm_r)]:
                t = cstg.tile([P, P], F32, name=f"s_{nm}", tag="s")
                nc.scalar.dma_start(t[:], dram[nm])
                nc.scalar.copy(dst[:], t[:])
                if nm == "eye":
                    nc.scalar.copy(eye_b[:], t[:])

    act = ctx.enter_context(tc.tile_pool(name="act", bufs=1))
    qkro = [act.tile([P, S], F32R, name=f"qro{h}") for h in range(HPC)]
    kro = [act.tile([P, S], F32R, name=f"kro{i}") for i in range(KVPC)]
    vnat = [act.tile([P, NT * P], BF16, name=f"vnat{i}") for i in range(KVPC)]
    dynT = act.tile([P, NT * HPC], F32, name="dynT")

    with ExitStack() as ctx1:
        vop = ctx1.enter_context(tc.tile_pool(name="vop", bufs=1))
        vT_own = [vop.tile([P, S], F32R, name=f"vTown{i}") for i in range(KVPC)]
        dt_sb = vop.tile([HPC, S], F32, name="dt_sb")

        # ---------------- dt first (enables early dyn/bisection) --------
        # dt must be f32-accurate: it decides the kthvalue mask set, and
        # bf16 dt flips enough borderline keys to breach the error budget.
        with tc.tile_pool(name="dps", bufs=2, space="PSUM") as dps, \
             tc.tile_pool(name="dtx", bufs=4) as dtx:
            for sg in range(NQ):
                dtp = dps.tile([HPC, QW], F32, name="dtp", tag="dtp")
                for cc in range(NT):
                    xf = dtx.tile([P, QW], F32R, name="xf", tag="xf")
                    ring = nc.sync if cc % 2 == 0 else nc.scalar
                    ring.dma_start(
                        xf[:], dram["xPf"][:, (sg * NT + cc) * QW:
                                           (sg * NT + cc + 1) * QW])
                    nc.tensor.matmul(dtp[:], wdtv[:, cc * HPC:(cc + 1) * HPC],
                                     xf[:],
                                     start=(cc == 0), stop=(cc == NT - 1))
                nc.scalar.copy(dt_sb[:, sg * QW:(sg + 1) * QW], dtp[:])
        bulk_loads()

        # ---------------- dyn + kth bisection (overlaps projections) ----
        dyq = ctx1.enter_context(tc.tile_pool(name="dyq", bufs=1))
        kth_f = dyq.tile([HPC, 1], I32, name="kth_f")
        dynrow = dyq.tile([HPC, S], F32R, name="dynrow")
        dyn_t = dyq.tile([HPC, S], F32, name="dyn_t")
        work = dyq.tile([HPC, S], F32, name="work")
        # work is dead after the dyn chain; reuse its storage for the
        # bisection scratch (bf16 view) and later the penalty tile
        scr = work[:].bitcast(BF16)[:, 0:S]
        pen = work
        nc.scalar.activation(work[:], dt_sb[:], AF.Exp)
        nc.scalar.activation(work[:], work[:], AF.Ln, bias=1.0)
        nc.scalar.activation(dyn_t[:], work[:], AF.Exp, scale=acol_t[:])
        lo = dyq.tile([HPC, 1], I32, name="lo")
        hi = dyq.tile([HPC, 1], I32, name="hi")
        mid = dyq.tile([HPC, 1], I32, name="mid")
        dlt = dyq.tile([HPC, 1], I32, name="dlt")
        cges = dyq.tile([HPC, 1], I32, name="cges")
        cltv = dyq.tile([HPC, 1], I32, name="cltv")
        cnt = dyq.tile([HPC, 1], F32, name="cnt")
        nc.vector.memset(lo[:], 0)
        nc.vector.memset(hi[:], 0x7F800000)
        for _ in range(31):
            nc.vector.tensor_tensor(dlt[:], hi[:], lo[:], op=AluOpType.subtract)
            nc.vector.tensor_scalar(dlt[:], dlt[:], 1, None,
                                    op0=AluOpType.arith_shift_right)
            nc.vector.tensor_tensor(mid[:], dlt[:], lo[:], op=AluOpType.add)
            nc.vector.tensor_scalar(scr, dyn_t[:],
                                    mid[:, 0:1].bitcast(F32), 0.0,
                                    op0=AluOpType.is_lt, op1=AluOpType.add,
                                    accum_out=cnt[:])
            nc.vector.tensor_scalar(cges[:], kthc[:], cnt[:, 0:1], None,
                                    op0=AluOpType.is_lt)
            nc.vector.tensor_scalar(cltv[:], kthc[:], cnt[:, 0:1], None,
                                    op0=AluOpType.is_ge)
            nc.vector.copy_predicated(hi[:], cges[:], mid[:])
            nc.vector.copy_predicated(lo[:], cltv[:], mid[:])
        nc.vector.tensor_copy(kth_f[:], lo[:])
        nc.vector.tensor_scalar(pen[:], dyn_t[:],
                                kth_f[:, 0:1].bitcast(F32), -BIG,
                                op0=AluOpType.is_lt, op1=AluOpType.mult)
        nc.vector.tensor_tensor(dynrow[:], dyn_t[:], pen[:], op=AluOpType.add)

        # ---------------- projections (bf16) + RoPE ----------------
        with tc.tile_pool(name="wp", bufs=2) as wp, \
             tc.tile_pool(name="pjp", bufs=6) as pjp, \
             tc.tile_pool(name="pps", bufs=6, space="PSUM") as pps:
            wsrc = {"v": "wvP", "q": "wqP", "k": "wkP"}
            OT = ([("v", i) for i in range(KVPC)]
                  + [("q", i) for i in range(HPC)]
                  + [("k", i) for i in range(KVPC)])
            for kind, oi in OT:
                wfull = wp.tile([P, NT * P], BF16, name="wfull", tag="wf")
                nc.sync.dma_start(
                    wfull[:],
                    dram[wsrc[kind]][:, oi * NT * P:(oi + 1) * NT * P])
                for sg in range(NQ):
                    ps = pps.tile([P, QW], F32, name="ps", tag="ps")
                    for cc in range(NT):
                        nc.tensor.matmul(ps[:], wfull[:, cc * P:(cc + 1) * P],
                                         xs[sg][:, cc * QW:(cc + 1) * QW],
                                         start=(cc == 0), stop=(cc == NT - 1))
                    if kind == "v":
                        nc.scalar.copy(vT_own[oi][:, sg * QW:(sg + 1) * QW],
                                       ps[:])
                    else:
                        pj = pjp.tile([P, QW], F32R, name="pj", tag="pj")
                        nc.scalar.copy(pj[:], ps[:])
                        rh = pps.tile([P, QW], F32, name="rh", tag="ps")
                        nc.tensor.matmul(rh[:], perm_r[:], pj[:],
                                         start=True, stop=True)
                        # drain rh psum via scalar immediately: the DVE ops
                        # below queue behind the long bisection on the
                        # in-order DVE engine, and holding psum that long
                        # stalls the PE on psum bufs.
                        rhs = pjp.tile([P, QW], F32, name="rhs", tag="pj")
                        nc.scalar.copy(rhs[:], rh[:])
                        t1 = pjp.tile([P, QW], F32, name="t1", tag="pj")
                        nc.vector.tensor_tensor(
                            t1[:], rhs[:], sin_t[:, sg * QW:(sg + 1) * QW],
                            op=AluOpType.mult)
                        t2 = pjp.tile([P, QW], F32, name="t2", tag="pj")
                        nc.gpsimd.tensor_tensor(
                            t2[:], pj[:], cos_t[:, sg * QW:(sg + 1) * QW],
                            op=AluOpType.mult)
                        dstro = (qkro[oi] if kind == "q" else kro[oi])
                        nc.vector.tensor_tensor(
                            dstro[:, sg * QW:(sg + 1) * QW], t1[:], t2[:],
                            op=AluOpType.add)

        # ---------------- natural-layout v tiles (SBUF resident) --------
        with tc.tile_pool(name="vps", bufs=4, space="PSUM") as vps:
            for i in range(KVPC):
                for cc in range(NT):
                    pt = vps.tile([P, P], F32, name="vt", tag="vt")
                    nc.tensor.transpose(pt[:].bitcast(F32R),
                                        vT_own[i][:, cc * P:(cc + 1) * P],
                                        eye_r[:])
                    nc.scalar.copy(vnat[i][:, cc * P:(cc + 1) * P], pt[:])

        # dynT transposes last in the PE queue before attention: they wait
        # on the DVE bisection, so anything emitted after them would stall
        # the in-order PE queue (cost a 122us bubble when emitted early).
        with tc.tile_pool(name="dtp2", bufs=1, space="PSUM") as dtp2:
            dyn_ps = dtp2.tile([P, NT * HPC], F32, name="dyn_ps")
            for cc in range(NT):
                nc.tensor.transpose(
                    dyn_ps[:, cc * HPC:(cc + 1) * HPC].bitcast(F32R),
                    dynrow[:, cc * P:(cc + 1) * P], eye_r[0:HPC, 0:HPC])
            nc.scalar.copy(dynT[:], dyn_ps[:])

    # ---------------- attention (transposed scores) + outproj ----------
    with tc.tile_pool(name="scp", bufs=2, space="PSUM") as scp, \
         tc.tile_pool(name="ovl", bufs=2, space="PSUM") as ovl, \
         tc.tile_pool(name="lpp", bufs=2, space="PSUM") as lpp, \
         tc.tile_pool(name="ptp", bufs=3) as ptp, \
         tc.tile_pool(name="atn", bufs=8) as atn, \
         tc.tile_pool(name="lnb", bufs=2) as lnb, \
         tc.tile_pool(name="lnv", bufs=2) as lnv, \
         tc.tile_pool(name="wol", bufs=2) as wol, \
         tc.tile_pool(name="oub", bufs=3) as oub, \
         tc.tile_pool(name="ops", bufs=2, space="PSUM") as ops:
        for grp in range(NQ):
            base = grp * QW
            tiles = list(range(grp * 4, grp * 4 + 4))
            jmax = max(extc[t] for t in tiles)
            at_grp = {}
            for h in range(HPC):
                kv = h // GROUPS
                ovp = ovl.tile([P, QW], F32, name="ovp", tag="ovp")
                lps = lpp.tile([1, QW], F32, name="lps", tag="lps")

                qlos = []
                for j in range(jmax):
                    acts = [t for t in tiles if j < extc[t]]
                    assert acts == tiles[-len(acts):], \
                        "active tiles must be a suffix of the group"
                    qlos.append(acts[0] * P - base)

                def emit_score(j):
                    qlo = qlos[j]
                    vts = [t for t in tiles
                           if j < extc[t] and blkstate[t][j].startswith("V")]
                    sc = scp.tile([P, QW], F32, name="sc", tag="sc")
                    nc.tensor.matmul(
                        sc[:, qlo:QW], kro[kv][:, j * P:(j + 1) * P],
                        qkro[h][:, base + qlo:base + QW],
                        start=True, stop=not vts, skip_group_check=True)
                    # mask blocks added on the PE (psum accumulate) via a
                    # moving-identity matmul-add: psum[i,j] += stat[j,i]
                    for n, t in enumerate(vts):
                        vi = int(blkstate[t][j][2:])
                        off = t * P - base
                        nc.tensor.matmul(
                            sc[:, off:off + P],
                            varblkQ[:, vi * P:(vi + 1) * P], eye_b[:],
                            start=False, stop=(n == len(vts) - 1),
                            skip_group_check=True)
                    pt = ptp.tile([P, QW], BF16, name="pt", tag="pt")
                    nc.scalar.activation(
                        pt[:, qlo:QW], sc[:, qlo:QW], AF.Exp,
                        bias=dynT[:, j * HPC + h:j * HPC + h + 1])
                    return pt

                # software-pipeline by one chunk: emit chunk j+1's score
                # matmul before chunk j's l/av matmuls so the PE works
                # through the exp latency instead of waiting on it.
                pts = emit_score(0)
                for j in range(jmax):
                    pt, qlo = pts, qlos[j]
                    if j + 1 < jmax:
                        pts = emit_score(j + 1)
                    nc.tensor.matmul(
                        lps[:, qlo:QW], onescol_b[:], pt[:, qlo:QW],
                        start=(j == 0), stop=(j == jmax - 1),
                        skip_group_check=True)
                    nc.tensor.matmul(
                        ovp[:, qlo:QW], vnat[kv][:, j * P:(j + 1) * P],
                        pt[:, qlo:QW],
                        start=(j == 0), stop=(j == jmax - 1),
                        skip_group_check=True)
                lsb = lnv.tile([1, QW], F32, name="lsb", tag="lv")
                nc.scalar.copy(lsb[:], lps[:])
                nc.sync.dma_start(l_d[h:h + 1, base:base + QW], lsb[:])
                linv = lnv.tile([1, QW], F32, name="linv", tag="lv")
                nc.vector.reciprocal_approx_fast(linv[:], lps[:])
                lbc = lnb.tile([P, QW], F32, name="lbc", tag="lb")
                nc.gpsimd.partition_broadcast(lbc[:], linv[:])
                at = atn.tile([P, QW], F32R, name="at", tag="at")
                nc.vector.tensor_tensor(at[:], ovp[:], lbc[:],
                                        op=AluOpType.mult)
                at_grp[h] = at
            for ht in range(NT):
                wo = wol.tile([P, HPC * P], F32R, name="wo", tag="wo")
                nc.gpsimd.dma_start(
                    wo[:], dram["woP"][:, ht * HPC * P:(ht + 1) * HPC * P])
                op = ops.tile([P, QW], F32, name="op", tag="op")
                for h in range(HPC):
                    nc.tensor.matmul(op[:], wo[:, h * P:(h + 1) * P],
                                     at_grp[h][:], start=(h == 0),
                                     stop=(h == HPC - 1))
                osb = oub.tile([P, QW], F32, name="osb", tag="ob")
                nc.scalar.copy(osb[:], op[:])
                nc.sync.dma_start(
                    outT_d[ht * P:(ht + 1) * P, base:base + QW], osb[:])
    ctx.close()


def _host_prep(hidden_states, cos, sin, attention_mask, Wq, Wk, Wv, A, Wdt, Wo):
    eye = np.eye(P, dtype=np.float32)
    perm = np.zeros((P, P), dtype=np.float32)
    for j in range(64):
        perm[j + 64, j] = -1.0
        perm[j, j + 64] = 1.0

    def pack_w(wT, nblk):
        # wT [HID, nblk*P] f32 -> [P, nblk*NT*P] bf16:
        # [p, (oi*NT+c)*P+f] = wT[c*P+p, oi*P+f]
        w4 = wT.reshape(NT, P, nblk, P)            # [c, p, oi, f]
        return np.ascontiguousarray(
            w4.transpose(1, 2, 0, 3).reshape(P, nblk * NT * P)
        ).astype(np.float32)

    in_maps = []
    blkstates = []
    for c in range(NCORES):
        b, g = divmod(c, 4)
        heads = list(range(4 * g, 4 * g + 4))
        # x packed: [p, ((sg*NT)+c)*QW+f] = x[b][sg*QW+f, c*P+p]
        xb = np.asarray(hidden_states[b], dtype=np.float32)
        xP = np.ascontiguousarray(
            xb.reshape(NQ, QW, NT, P).transpose(3, 0, 2, 1)
            .reshape(P, NQ * NT * QW))
        wqT = (Wq[4 * g * D:(4 * g + 4) * D] * np.float32(SCALING)).T
        wkT = Wk[2 * g * D:(2 * g + 2) * D].T
        wvT = Wv[2 * g * D:(2 * g + 2) * D].T
        wqP = pack_w(np.ascontiguousarray(wqT), HPC)
        wkP = pack_w(np.ascontiguousarray(wkT), KVPC)
        wvP = pack_w(np.ascontiguousarray(wvT), KVPC)
        wdtvT = np.ascontiguousarray(
            (Wdt[heads].astype(np.float64) @ Wv.astype(np.float64))
            .T.astype(np.float32))                 # [HID, 4]
        wdtvP = np.ascontiguousarray(
            wdtvT.reshape(NT, P, HPC).transpose(1, 0, 2).reshape(P, NT * HPC))
        # woP: [p, (ht*HPC+h)*P+f] = WoT[h*P+p, ht*P+f]
        woT = np.ascontiguousarray(Wo[:, 4 * g * D:(4 * g + 4) * D].T)
        woP = np.ascontiguousarray(
            woT.reshape(HPC, P, NT, P).transpose(1, 2, 0, 3)
            .reshape(P, NT * HPC * P))
        acol = A[heads].astype(np.float32).reshape(HPC, 1)
        cosT = np.ascontiguousarray(cos[b].T)
        sinT = np.ascontiguousarray(sin[b].T)
        m = attention_mask[b, 0]
        mb = np.asarray(m).reshape(NT, P, NT, P)
        blkrows = []
        varlist = []
        varkeys = {}
        for t in range(NT):
            row = []
            for j in range(NT):
                blkv = mb[t, :, j, :]
                if np.all(blkv == 0):
                    row.append("Z")
                elif np.all(blkv <= -1e30):
                    row.append("M")
                else:
                    bQ = np.ascontiguousarray(
                        np.maximum(blkv, -BIG))    # [query, key]
                    kk = bQ.tobytes()
                    if kk not in varkeys:
                        varkeys[kk] = len(varlist)
                        varlist.append(bQ)
                    row.append(f"V:{varkeys[kk]}")
            # interior M blocks (before a later non-M block) become varying
            nz = [j for j in range(NT) if row[j] != "M"]
            lim = (max(nz) + 1) if nz else 0
            for j in range(lim):
                if row[j] == "M":
                    bQ = np.full((P, P), -BIG, np.float32)
                    kk = bQ.tobytes()
                    if kk not in varkeys:
                        varkeys[kk] = len(varlist)
                        varlist.append(bQ)
                    row[j] = f"V:{varkeys[kk]}"
            blkrows.append(tuple(row))
        if len(varlist) > 8:
            raise NotImplementedError("too many varying mask blocks")
        varblkQ = np.zeros((P, max(len(varlist), 1) * P), dtype=np.float32)
        for vi, blkv in enumerate(varlist):
            varblkQ[:, vi * P:(vi + 1) * P] = blkv
        blkstate = tuple(blkrows)
        in_maps.append({
            "xP": xP.astype(BF16NP), "xPf": xP,
            "wqP": wqP.astype(BF16NP),
            "wkP": wkP.astype(BF16NP), "wvP": wvP.astype(BF16NP),
            "wdtvPr": wdtvP, "woP": woP, "cosT": cosT,
            "sinT": sinT, "acol": acol, "eye": eye, "perm": perm,
            "varblkQ": varblkQ.astype(BF16NP),
        })
        blkstates.append(blkstate)
    if len(set(blkstates)) != 1:
        raise NotImplementedError("mask structure differs across batches")
    return in_maps, blkstates[0]


def _softplus64(x):
    x = x.astype(np.float64)
    return np.log1p(np.exp(-np.abs(x))) + np.maximum(x, 0)


def _repair_rows(out, bad, inputs):
    """Recompute rows flagged bad [B, S] with faithful numpy reference math."""
    if not bad.any():
        return out
    hs = inputs["hidden_states"]; cos = inputs["cos"]; sin = inputs["sin"]
    am = inputs["attention_mask"]; Wq = inputs["Wq"]; Wk = inputs["Wk"]
    Wv = inputs["Wv"]; A = inputs["A"]; Wdt = inputs["Wdt"]; Wo = inputs["Wo"]

    def rope(x, c, s):
        x1, x2 = x[..., :D // 2], x[..., D // 2:]
        return x * c + np.concatenate([-x2, x1], axis=-1) * s

    for b in range(B):
        rows = np.where(bad[b])[0]
        if len(rows) == 0:
            continue
        x = hs[b].astype(np.float32)
        k = (x @ Wk.T).reshape(S, KV, D)
        v = (x @ Wv.T).reshape(S, KV, D)
        k = rope(k, cos[b][:, None, :], sin[b][:, None, :])
        v_flat = v.reshape(S, KV * D)
        dt = v_flat @ Wdt.T
        dyn = np.exp(A[None, :] * _softplus64(dt)).astype(np.float32).T
        kth = np.sort(dyn, axis=-1)[:, NUM_DYN - 1:NUM_DYN]
        dmask = np.where(dyn < kth, MIN, dyn).astype(np.float32)
        for s_i in rows:
            q_row = (x[s_i] @ Wq.T).reshape(H, D)
            q_row = rope(q_row, cos[b][s_i][None, :], sin[b][s_i][None, :])
            attn_row = np.zeros((H, D), dtype=np.float32)
            for h in range(H):
                kvh = h // GROUPS
                sc = ((q_row[h] @ k[:, kvh].T) * np.float32(SCALING)
                      + (dmask[h] + am[b, 0, s_i])).astype(np.float32)
                w = np.exp(sc - sc.max())
                w = (w / w.sum()).astype(np.float32)
                attn_row[h] = w @ v[:, kvh]
            out[b, s_i] = attn_row.reshape(H * D) @ Wo.T
    return out


def kernel(**inputs):
    inputs = {k: np.asarray(v) for k, v in inputs.items()}
    in_maps, blkstate = _host_prep(**inputs)
    nc = _build_program(blkstate)
    res = run_bass_kernel_spmd(nc, in_maps, list(range(NCORES)))
    out = np.zeros((B, S, HID), dtype=np.float32)
    bad = np.zeros((B, S), dtype=bool)
    for c in range(NCORES):
        b = c // 4
        out[b] += res.results[c]["outT"].T
        bad[b] |= (res.results[c]["l_out"] == 0).any(axis=0)
    bad |= ~np.isfinite(out).all(axis=2)
    out = _repair_rows(out, bad, inputs)
    return out


# revision 58
# speedup vs baseline: 1.7565x; 1.1390x over previous
"""DogeDynamicMaskAttention Trainium2 kernel (transposed-scores redesign).

Sharding: 8 cores = 2 batches x 4 head-groups. Core c: batch b=c//4,
head-group g=c%4 -> heads [4g..4g+4), kv heads {2g, 2g+1}.

Design vs previous baseline:
  - scores computed TRANSPOSED [keys, queries]: the dynamic mask row is a
    per-partition (per-key) bias folded into the exp activation for free;
    the P-matrix transposes + f32r casts of the old layout vanish; the
    attn@v matmul consumes exp output directly (keys on partitions).
  - l (softmax denom) via a ones-column stationary matmul accumulated in
    psum; normalize out tiles with reciprocal + gpsimd partition_broadcast
    + one DVE multiply per (head, query-group).
  - projections in bf16 (x and Wq/Wk/Wv/Wdt host-packed contiguous, so
    DMA is large-descriptor); x resident in SBUF, read once.
  - v natural-layout tiles kept in SBUF (no DRAM bounce).
  - dyn/kth bisection identical to baseline (31-step float-bit bisection),
    overlapped under the q/k/v projections; dynT obtained by tiny PE
    transposes instead of a DRAM round trip.
  - fully-masked (degenerate) rows: l==0 detected on host via l output,
    recomputed faithfully in numpy (expected ~1 row per batch*head).
"""
import sys
import numpy as np
import ml_dtypes

BF16NP = ml_dtypes.bfloat16

sys.path.insert(0, "/root/.axon_site/_ro/trn_rl_repo")

import concourse.bass as bass  # noqa: E402,F401
from concourse import bacc  # noqa: E402
import concourse.tile as tile  # noqa: E402
import concourse.mybir as mybir  # noqa: E402
from concourse.bass_utils import run_bass_kernel_spmd  # noqa: E402
from concourse.alu_op_type import AluOpType  # noqa: E402

F32 = mybir.dt.float32
F32R = mybir.dt.float32r
BF16 = mybir.dt.bfloat16
I32 = mybir.dt.int32
AF = mybir.ActivationFunctionType
AX = mybir.AxisListType.X

B, S, HID = 2, 2048, 2048
H, KV, D = 16, 8, 128
HPC, KVPC = 4, 2
GROUPS = H // KV
NUM_DYN = S // 2
SCALING = D ** -0.5
MIN = float(np.finfo(np.float32).min)
BIG = 1.7e38
P = 128
NT = S // P          # 16
NQ = 4
QW = S // NQ         # 512
NCORES = 8

_cache = {}


def _build_program(blkstate):
    key = ("nc", blkstate)
    if key in _cache:
        return _cache[key]
    nvar = _num_varblocks(blkstate)
    nc = bacc.Bacc("TRN2", target_bir_lowering=False, debug=False,
                   num_devices=NCORES)
    dram = {}
    for name, shape, dt in [
            ("xP", [P, NQ * NT * QW], BF16),
            ("xPf", [P, NQ * NT * QW], F32R),
            ("wqP", [P, HPC * NT * P], BF16),
            ("wkP", [P, KVPC * NT * P], BF16),
            ("wvP", [P, KVPC * NT * P], BF16),
            ("wdtvPr", [P, NT * HPC], F32R),
            ("woP", [P, NT * HPC * P], F32),
            ("cosT", [D, S], F32), ("sinT", [D, S], F32),
            ("acol", [HPC, 1], F32),
            ("eye", [P, P], F32), ("perm", [P, P], F32),
            ("varblkT", [P, max(nvar, 1) * P], F32)]:
        dram[name] = nc.dram_tensor(name, shape, dt, kind="ExternalInput").ap()
    outT_d = nc.dram_tensor("outT", [HID, S], F32, kind="ExternalOutput").ap()
    l_d = nc.dram_tensor("l_out", [HPC, S], F32, kind="ExternalOutput").ap()

    with tile.TileContext(nc) as tc:
        _emit(nc, tc, dram, outT_d, l_d, blkstate)
    nc.compile()
    _cache[key] = nc
    return nc


def _num_varblocks(blkstate):
    n = 0
    for t in range(NT):
        for j in range(NT):
            if blkstate[t][j].startswith("V"):
                n = max(n, int(blkstate[t][j][2:]) + 1)
    return n


def _emit(nc, tc, dram, outT_d, l_d, blkstate):
    from contextlib import ExitStack
    ctx = ExitStack()

    # per-tile computed extent (in key chunks): chunks j < extc[t] participate
    extc = []
    for t in range(NT):
        nz = [j for j in range(NT) if blkstate[t][j] != "M"]
        assert nz and min(nz) == 0, "chunk 0 must be active for every tile"
        extc.append(max(nz) + 1)

    consts = ctx.enter_context(tc.tile_pool(name="consts", bufs=1))

    # dt-critical consts first on the sync ring so the dt pass starts
    # immediately; all bulk loads go on the Activation DGE ring.
    wdtv = consts.tile([P, NT * HPC], F32R, name="c_wdtv")
    nc.sync.dma_start(wdtv[:], dram["wdtvPr"])
    acol_t = consts.tile([HPC, 1], F32, name="c_acol")
    nc.sync.dma_start(acol_t[:], dram["acol"])
    onescol_b = consts.tile([P, 1], BF16, name="onescol")
    nc.vector.memset(onescol_b[:], 1.0)
    kthc = consts.tile([HPC, 1], F32, name="kthc")
    nc.vector.memset(kthc[:], float(NUM_DYN) - 0.5)

    eye_r = consts.tile([P, P], F32R, name="cr_eye")
    perm_r = consts.tile([P, P], F32R, name="cr_perm")
    nvar = _num_varblocks(blkstate)
    varblkT = consts.tile([P, max(nvar, 1) * P], F32, name="c_varblkT")

    csp = ctx.enter_context(tc.tile_pool(name="csp", bufs=1))
    cos_t = csp.tile([D, S], F32, name="cos_t")
    sin_t = csp.tile([D, S], F32, name="sin_t")

    # side consts on the gpsimd software-DGE ring, in parallel with the
    # dt-critical sync/Activation traffic
    nc.gpsimd.dma_start(cos_t[:], dram["cosT"])
    nc.gpsimd.dma_start(sin_t[:], dram["sinT"])
    nc.gpsimd.dma_start(varblkT[:], dram["varblkT"])
    cstg = ctx.enter_context(tc.tile_pool(name="cstg", bufs=2))
    for nm, dst in [("eye", eye_r), ("perm", perm_r)]:
        t = cstg.tile([P, P], F32, name=f"s_{nm}", tag="s")
        nc.gpsimd.dma_start(t[:], dram[nm])
        nc.scalar.copy(dst[:], t[:])

    act = ctx.enter_context(tc.tile_pool(name="act", bufs=1))
    qkro = [act.tile([P, S], F32R, name=f"qro{h}") for h in range(HPC)]
    kro = [act.tile([P, S], F32R, name=f"kro{i}") for i in range(KVPC)]
    vnat = [act.tile([P, NT * P], BF16, name=f"vnat{i}") for i in range(KVPC)]
    dynT = act.tile([P, NT * HPC], F32, name="dynT")

    with ExitStack() as ctx1:
        xsp = ctx1.enter_context(tc.tile_pool(name="xsp", bufs=1))
        xs = [xsp.tile([P, NT * QW], BF16, name=f"xs{sg}")
              for sg in range(NQ)]
        vop = ctx1.enter_context(tc.tile_pool(name="vop", bufs=1))
        vT_own = [vop.tile([P, S], F32R, name=f"vTown{i}") for i in range(KVPC)]
        dt_sb = vop.tile([HPC, S], F32, name="dt_sb")

        # ---- dt pass (f32-accurate: decides the kth mask set) merged ----
        # with the projections; dt chains interleave with v-projections so
        # the PE stays fed while dt's x stream arrives. The dyn chain +
        # bisection is emitted right after the last dt chain so the scalar
        # and DVE queues reach it early (both are in-order engines).
        dyq = ctx1.enter_context(tc.tile_pool(name="dyq", bufs=1))
        kth_f = dyq.tile([HPC, 1], I32, name="kth_f")
        dynrow = dyq.tile([HPC, S], F32R, name="dynrow")
        dyn_t = dyq.tile([HPC, S], F32, name="dyn_t")
        work = dyq.tile([HPC, S], F32, name="work")
        # work is dead after the dyn chain; reuse its storage for the
        # bisection scratch (bf16 view) and later the penalty tile
        scr = work[:].bitcast(BF16)[:, 0:S]
        pen = work

        def emit_dyn_bisect():
            nc.scalar.activation(work[:], dt_sb[:], AF.Exp)
            nc.scalar.activation(work[:], work[:], AF.Ln, bias=1.0)
            nc.scalar.activation(dyn_t[:], work[:], AF.Exp, scale=acol_t[:])
            lo = dyq.tile([HPC, 1], I32, name="lo")
            hi = dyq.tile([HPC, 1], I32, name="hi")
            mid = dyq.tile([HPC, 1], I32, name="mid")
            dlt = dyq.tile([HPC, 1], I32, name="dlt")
            cges = dyq.tile([HPC, 1], I32, name="cges")
            cltv = dyq.tile([HPC, 1], I32, name="cltv")
            cnt = dyq.tile([HPC, 1], F32, name="cnt")
            nc.vector.memset(lo[:], 0)
            nc.vector.memset(hi[:], 0x7F800000)
            for _ in range(31):
                nc.vector.tensor_tensor(dlt[:], hi[:], lo[:],
                                        op=AluOpType.subtract)
                nc.vector.tensor_scalar(dlt[:], dlt[:], 1, None,
                                        op0=AluOpType.arith_shift_right)
                nc.vector.tensor_tensor(mid[:], dlt[:], lo[:],
                                        op=AluOpType.add)
                nc.vector.tensor_scalar(scr, dyn_t[:],
                                        mid[:, 0:1].bitcast(F32), 0.0,
                                        op0=AluOpType.is_lt,
                                        op1=AluOpType.add,
                                        accum_out=cnt[:])
                nc.vector.tensor_scalar(cges[:], kthc[:], cnt[:, 0:1], None,
                                        op0=AluOpType.is_lt)
                nc.vector.tensor_scalar(cltv[:], kthc[:], cnt[:, 0:1], None,
                                        op0=AluOpType.is_ge)
                nc.vector.copy_predicated(hi[:], cges[:], mid[:])
                nc.vector.copy_predicated(lo[:], cltv[:], mid[:])
            nc.vector.tensor_copy(kth_f[:], lo[:])
            nc.vector.tensor_scalar(pen[:], dyn_t[:],
                                    kth_f[:, 0:1].bitcast(F32), -BIG,
                                    op0=AluOpType.is_lt, op1=AluOpType.mult)
            nc.vector.tensor_tensor(dynrow[:], dyn_t[:], pen[:],
                                    op=AluOpType.add)

        with tc.tile_pool(name="dps", bufs=2, space="PSUM") as dps, \
             tc.tile_pool(name="dtx", bufs=2) as dtx, \
             tc.tile_pool(name="wp", bufs=2) as wp, \
             tc.tile_pool(name="pjp", bufs=4) as pjp, \
             tc.tile_pool(name="pps", bufs=6, space="PSUM") as pps:

            def emit_dt(sg):
                dtp = dps.tile([HPC, QW], F32, name="dtp", tag="dtp")
                for cc in range(NT):
                    xf = dtx.tile([P, QW], F32R, name="xf", tag="xf")
                    ring = nc.sync if cc % 2 == 0 else nc.scalar
                    ring.dma_start(
                        xf[:], dram["xPf"][:, (sg * NT + cc) * QW:
                                           (sg * NT + cc + 1) * QW])
                    nc.tensor.matmul(dtp[:], wdtv[:, cc * HPC:(cc + 1) * HPC],
                                     xf[:],
                                     start=(cc == 0), stop=(cc == NT - 1))
                # DVE copy: keeps the in-order scalar queue free for DMA
                # issues and the dyn chain
                nc.vector.tensor_copy(dt_sb[:, sg * QW:(sg + 1) * QW], dtp[:])
                nc.scalar.dma_start(
                    xs[sg][:], dram["xP"][:, sg * NT * QW:(sg + 1) * NT * QW])

            wsrc = {"v": "wvP", "q": "wqP", "k": "wkP"}

            def emit_proj(kind, oi):
                wfull = wp.tile([P, NT * P], BF16, name="wfull", tag="wf")
                nc.sync.dma_start(
                    wfull[:],
                    dram[wsrc[kind]][:, oi * NT * P:(oi + 1) * NT * P])
                for sg in range(NQ):
                    ps = pps.tile([P, QW], F32, name="ps", tag="ps")
                    for cc in range(NT):
                        nc.tensor.matmul(ps[:], wfull[:, cc * P:(cc + 1) * P],
                                         xs[sg][:, cc * QW:(cc + 1) * QW],
                                         start=(cc == 0), stop=(cc == NT - 1))
                    if kind == "v":
                        nc.vector.tensor_copy(
                            vT_own[oi][:, sg * QW:(sg + 1) * QW], ps[:])
                    else:
                        pj = pjp.tile([P, QW], F32R, name="pj", tag="pj")
                        nc.scalar.copy(pj[:], ps[:])
                        rh = pps.tile([P, QW], F32, name="rh", tag="ps")
                        nc.tensor.matmul(rh[:], perm_r[:], pj[:],
                                         start=True, stop=True)
                        # gpsimd cannot read PSUM: stage rh through SBUF on
                        # the scalar engine, then do all RoPE elementwise
                        # work on gpsimd (DVE is busy with the bisection and
                        # its in-order queue would pin pjp tiles for ~70us)
                        rhs = pjp.tile([P, QW], F32, name="rhs", tag="pj")
                        nc.scalar.copy(rhs[:], rh[:])
                        t1 = pjp.tile([P, QW], F32, name="t1", tag="pj")
                        nc.gpsimd.tensor_tensor(
                            t1[:], rhs[:], sin_t[:, sg * QW:(sg + 1) * QW],
                            op=AluOpType.mult)
                        t2 = pjp.tile([P, QW], F32, name="t2", tag="pj")
                        nc.gpsimd.tensor_tensor(
                            t2[:], pj[:], cos_t[:, sg * QW:(sg + 1) * QW],
                            op=AluOpType.mult)
                        dstro = (qkro[oi] if kind == "q" else kro[oi])
                        nc.gpsimd.tensor_tensor(
                            dstro[:, sg * QW:(sg + 1) * QW], t1[:], t2[:],
                            op=AluOpType.add)

            emit_dt(0)
            emit_dt(1)
            emit_proj("v", 0)
            emit_dt(2)
            emit_proj("v", 1)
            emit_dt(3)
            emit_dyn_bisect()
            for kind, oi in [("q", 0), ("q", 1), ("q", 2), ("q", 3),
                             ("k", 0), ("k", 1)]:
                emit_proj(kind, oi)

        # ---------------- natural-layout v tiles (SBUF resident) --------
        with tc.tile_pool(name="vps", bufs=4, space="PSUM") as vps:
            for i in range(KVPC):
                for cc in range(NT):
                    pt = vps.tile([P, P], F32, name="vt", tag="vt")
                    nc.tensor.transpose(pt[:].bitcast(F32R),
                                        vT_own[i][:, cc * P:(cc + 1) * P],
                                        eye_r[:])
                    nc.scalar.copy(vnat[i][:, cc * P:(cc + 1) * P], pt[:])

        # dynT transposes last in the PE queue before attention: they wait
        # on the DVE bisection, so anything emitted after them would stall
        # the in-order PE queue (cost a 122us bubble when emitted early).
        with tc.tile_pool(name="dtp2", bufs=1, space="PSUM") as dtp2:
            dyn_ps = dtp2.tile([P, NT * HPC], F32, name="dyn_ps")
            for cc in range(NT):
                nc.tensor.transpose(
                    dyn_ps[:, cc * HPC:(cc + 1) * HPC].bitcast(F32R),
                    dynrow[:, cc * P:(cc + 1) * P], eye_r[0:HPC, 0:HPC])
            nc.scalar.copy(dynT[:], dyn_ps[:])

    # ---------------- attention (transposed scores) + outproj ----------
    # wo resident: loaded once (not once per query-group), via the
    # Activation DGE ring while the first group's attention runs
    wop = ctx.enter_context(tc.tile_pool(name="wop", bufs=1))
    wos = []
    for ht in range(NT):
        wo = wop.tile([P, HPC * P], F32R, name=f"wo{ht}")
        nc.gpsimd.dma_start(
            wo[:], dram["woP"][:, ht * HPC * P:(ht + 1) * HPC * P])
        wos.append(wo)
    with tc.tile_pool(name="scp", bufs=2, space="PSUM") as scp, \
         tc.tile_pool(name="ovl", bufs=2, space="PSUM") as ovl, \
         tc.tile_pool(name="lpp", bufs=2, space="PSUM") as lpp, \
         tc.tile_pool(name="ptp", bufs=3) as ptp, \
         tc.tile_pool(name="atn", bufs=8) as atn, \
         tc.tile_pool(name="lnb", bufs=2) as lnb, \
         tc.tile_pool(name="lnv", bufs=2) as lnv, \
         tc.tile_pool(name="oub", bufs=4) as oub, \
         tc.tile_pool(name="ops", bufs=2, space="PSUM") as ops:
        for grp in range(NQ):
            base = grp * QW
            tiles = list(range(grp * 4, grp * 4 + 4))
            jmax = max(extc[t] for t in tiles)
            at_grp = {}
            for h in range(HPC):
                kv = h // GROUPS
                ovp = ovl.tile([P, QW], F32, name="ovp", tag="ovp")
                lps = lpp.tile([1, QW], F32, name="lps", tag="lps")

                qlos = []
                for j in range(jmax):
                    acts = [t for t in tiles if j < extc[t]]
                    assert acts == tiles[-len(acts):], \
                        "active tiles must be a suffix of the group"
                    qlos.append(acts[0] * P - base)

                def emit_score(j):
                    qlo = qlos[j]
                    sc = scp.tile([P, QW], F32, name="sc", tag="sc")
                    nc.tensor.matmul(
                        sc[:, qlo:QW], kro[kv][:, j * P:(j + 1) * P],
                        qkro[h][:, base + qlo:base + QW],
                        start=True, stop=True, skip_group_check=True)
                    for t in tiles:
                        if j >= extc[t]:
                            continue
                        st = blkstate[t][j]
                        if st.startswith("V"):
                            vi = int(st[2:])
                            off = t * P - base
                            nc.vector.tensor_tensor(
                                sc[:, off:off + P], sc[:, off:off + P],
                                varblkT[:, vi * P:(vi + 1) * P],
                                op=AluOpType.add)
                    pt = ptp.tile([P, QW], BF16, name="pt", tag="pt")
                    nc.scalar.activation(
                        pt[:, qlo:QW], sc[:, qlo:QW], AF.Exp,
                        bias=dynT[:, j * HPC + h:j * HPC + h + 1])
                    return pt

                # software-pipeline by one chunk: emit chunk j+1's score
                # matmul before chunk j's l/av matmuls so the PE works
                # through the exp latency instead of waiting on it.
                pts = emit_score(0)
                for j in range(jmax):
                    pt, qlo = pts, qlos[j]
                    if j + 1 < jmax:
                        pts = emit_score(j + 1)
                    nc.tensor.matmul(
                        lps[:, qlo:QW], onescol_b[:], pt[:, qlo:QW],
                        start=(j == 0), stop=(j == jmax - 1),
                        skip_group_check=True)
                    nc.tensor.matmul(
                        ovp[:, qlo:QW], vnat[kv][:, j * P:(j + 1) * P],
                        pt[:, qlo:QW],
                        start=(j == 0), stop=(j == jmax - 1),
                        skip_group_check=True)
                lsb = lnv.tile([1, QW], F32, name="lsb", tag="lv")
                nc.scalar.copy(lsb[:], lps[:])
                nc.sync.dma_start(l_d[h:h + 1, base:base + QW], lsb[:])
                linv = lnv.tile([1, QW], F32, name="linv", tag="lv")
                nc.vector.reciprocal_approx_fast(linv[:], lps[:])
                lbc = lnb.tile([P, QW], F32, name="lbc", tag="lb")
                nc.gpsimd.partition_broadcast(lbc[:], linv[:])
                at = atn.tile([P, QW], F32R, name="at", tag="at")
                nc.vector.tensor_tensor(at[:], ovp[:], lbc[:],
                                        op=AluOpType.mult)
                at_grp[h] = at
            for ht in range(NT):
                op = ops.tile([P, QW], F32, name="op", tag="op")
                for h in range(HPC):
                    nc.tensor.matmul(op[:], wos[ht][:, h * P:(h + 1) * P],
                                     at_grp[h][:], start=(h == 0),
                                     stop=(h == HPC - 1))
                osb = oub.tile([P, QW], F32, name="osb", tag="ob")
                # alternate the psum drain between scalar and DVE so
                # neither in-order queue delays the next group's exps
                if ht % 2 == 0:
                    nc.scalar.copy(osb[:], op[:])
                else:
                    nc.vector.tensor_copy(osb[:], op[:])
                nc.sync.dma_start(
                    outT_d[ht * P:(ht + 1) * P, base:base + QW], osb[:])
    ctx.close()


def _host_prep(hidden_states, cos, sin, attention_mask, Wq, Wk, Wv, A, Wdt, Wo):
    eye = np.eye(P, dtype=np.float32)
    perm = np.zeros((P, P), dtype=np.float32)
    for j in range(64):
        perm[j + 64, j] = -1.0
        perm[j, j + 64] = 1.0

    def pack_w(wT, nblk):
        # wT [HID, nblk*P] f32 -> [P, nblk*NT*P] bf16:
        # [p, (oi*NT+c)*P+f] = wT[c*P+p, oi*P+f]
        w4 = wT.reshape(NT, P, nblk, P)            # [c, p, oi, f]
        return np.ascontiguousarray(
            w4.transpose(1, 2, 0, 3).reshape(P, nblk * NT * P)
        ).astype(np.float32)

    in_maps = []
    blkstates = []
    for c in range(NCORES):
        b, g = divmod(c, 4)
        heads = list(range(4 * g, 4 * g + 4))
        # x packed: [p, ((sg*NT)+c)*QW+f] = x[b][sg*QW+f, c*P+p]
        xb = np.asarray(hidden_states[b], dtype=np.float32)
        xP = np.ascontiguousarray(
            xb.reshape(NQ, QW, NT, P).transpose(3, 0, 2, 1)
            .reshape(P, NQ * NT * QW))
        wqT = (Wq[4 * g * D:(4 * g + 4) * D] * np.float32(SCALING)).T
        wkT = Wk[2 * g * D:(2 * g + 2) * D].T
        wvT = Wv[2 * g * D:(2 * g + 2) * D].T
        wqP = pack_w(np.ascontiguousarray(wqT), HPC)
        wkP = pack_w(np.ascontiguousarray(wkT), KVPC)
        wvP = pack_w(np.ascontiguousarray(wvT), KVPC)
        wdtvT = np.ascontiguousarray(
            (Wdt[heads].astype(np.float64) @ Wv.astype(np.float64))
            .T.astype(np.float32))                 # [HID, 4]
        wdtvP = np.ascontiguousarray(
            wdtvT.reshape(NT, P, HPC).transpose(1, 0, 2).reshape(P, NT * HPC))
        # woP: [p, (ht*HPC+h)*P+f] = WoT[h*P+p, ht*P+f]
        woT = np.ascontiguousarray(Wo[:, 4 * g * D:(4 * g + 4) * D].T)
        woP = np.ascontiguousarray(
            woT.reshape(HPC, P, NT, P).transpose(1, 2, 0, 3)
            .reshape(P, NT * HPC * P))
        acol = A[heads].astype(np.float32).reshape(HPC, 1)
        cosT = np.ascontiguousarray(cos[b].T)
        sinT = np.ascontiguousarray(sin[b].T)
        m = attention_mask[b, 0]
        mb = np.asarray(m).reshape(NT, P, NT, P)
        blkrows = []
        varlist = []
        varkeys = {}
        for t in range(NT):
            row = []
            for j in range(NT):
                blkv = mb[t, :, j, :]
                if np.all(blkv == 0):
                    row.append("Z")
                elif np.all(blkv <= -1e30):
                    row.append("M")
                else:
                    bT = np.ascontiguousarray(
                        np.maximum(blkv, -BIG).T)  # [key, query]
                    kk = bT.tobytes()
                    if kk not in varkeys:
                        varkeys[kk] = len(varlist)
                        varlist.append(bT)
                    row.append(f"V:{varkeys[kk]}")
            # interior M blocks (before a later non-M block) become varying
            nz = [j for j in range(NT) if row[j] != "M"]
            lim = (max(nz) + 1) if nz else 0
            for j in range(lim):
                if row[j] == "M":
                    bT = np.full((P, P), -BIG, np.float32)
                    kk = bT.tobytes()
                    if kk not in varkeys:
                        varkeys[kk] = len(varlist)
                        varlist.append(bT)
                    row[j] = f"V:{varkeys[kk]}"
            blkrows.append(tuple(row))
        if len(varlist) > 8:
            raise NotImplementedError("too many varying mask blocks")
        varblkT = np.zeros((P, max(len(varlist), 1) * P), dtype=np.float32)
        for vi, blkv in enumerate(varlist):
            varblkT[:, vi * P:(vi + 1) * P] = blkv
        blkstate = tuple(blkrows)
        in_maps.append({
            "xP": xP.astype(BF16NP), "xPf": xP,
            "wqP": wqP.astype(BF16NP),
            "wkP": wkP.astype(BF16NP), "wvP": wvP.astype(BF16NP),
            "wdtvPr": wdtvP, "woP": woP, "cosT": cosT,
            "sinT": sinT, "acol": acol, "eye": eye, "perm": perm,
            "varblkT": varblkT,
        })
        blkstates.append(blkstate)
    if len(set(blkstates)) != 1:
        raise NotImplementedError("mask structure differs across batches")
    return in_maps, blkstates[0]


def _softplus64(x):
    x = x.astype(np.float64)
    return np.log1p(np.exp(-np.abs(x))) + np.maximum(x, 0)


def _repair_rows(out, bad, inputs):
    """Recompute rows flagged bad [B, S] with faithful numpy reference math."""
    if not bad.any():
        return out
    hs = inputs["hidden_states"]; cos = inputs["cos"]; sin = inputs["sin"]
    am = inputs["attention_mask"]; Wq = inputs["Wq"]; Wk = inputs["Wk"]
    Wv = inputs["Wv"]; A = inputs["A"]; Wdt = inputs["Wdt"]; Wo = inputs["Wo"]

    def rope(x, c, s):
        x1, x2 = x[..., :D // 2], x[..., D // 2:]
        return x * c + np.concatenate([-x2, x1], axis=-1) * s

    for b in range(B):
        rows = np.where(bad[b])[0]
        if len(rows) == 0:
            continue
        x = hs[b].astype(np.float32)
        k = (x @ Wk.T).reshape(S, KV, D)
        v = (x @ Wv.T).reshape(S, KV, D)
        k = rope(k, cos[b][:, None, :], sin[b][:, None, :])
        v_flat = v.reshape(S, KV * D)
        dt = v_flat @ Wdt.T
        dyn = np.exp(A[None, :] * _softplus64(dt)).astype(np.float32).T
        kth = np.sort(dyn, axis=-1)[:, NUM_DYN - 1:NUM_DYN]
        dmask = np.where(dyn < kth, MIN, dyn).astype(np.float32)
        for s_i in rows:
            q_row = (x[s_i] @ Wq.T).reshape(H, D)
            q_row = rope(q_row, cos[b][s_i][None, :], sin[b][s_i][None, :])
            attn_row = np.zeros((H, D), dtype=np.float32)
            for h in range(H):
                kvh = h // GROUPS
                sc = ((q_row[h] @ k[:, kvh].T) * np.float32(SCALING)
                      + (dmask[h] + am[b, 0, s_i])).astype(np.float32)
                w = np.exp(sc - sc.max())
                w = (w / w.sum()).astype(np.float32)
                attn_row[h] = w @ v[:, kvh]
            out[b, s_i] = attn_row.reshape(H * D) @ Wo.T
    return out


def kernel(**inputs):
    inputs = {k: np.asarray(v) for k, v in inputs.items()}
    in_maps, blkstate = _host_prep(**inputs)
    nc = _build_program(blkstate)
    res = run_bass_kernel_spmd(nc, in_maps, list(range(NCORES)))
    out = np.zeros((B, S, HID), dtype=np.float32)
    bad = np.zeros((B, S), dtype=bool)
    for c in range(NCORES):
        b = c // 4
        out[b] += res.results[c]["outT"].T
        bad[b] |= (res.results[c]["l_out"] == 0).any(axis=0)
    bad |= ~np.isfinite(out).all(axis=2)
    out = _repair_rows(out, bad, inputs)
    return out


# revision 61
# speedup vs baseline: 1.8254x; 1.0392x over previous
"""DogeDynamicMaskAttention Trainium2 kernel (transposed-scores redesign).

Sharding: 8 cores = 2 batches x 4 head-groups. Core c: batch b=c//4,
head-group g=c%4 -> heads [4g..4g+4), kv heads {2g, 2g+1}.

Design vs previous baseline:
  - scores computed TRANSPOSED [keys, queries]: the dynamic mask row is a
    per-partition (per-key) bias folded into the exp activation for free;
    the P-matrix transposes + f32r casts of the old layout vanish; the
    attn@v matmul consumes exp output directly (keys on partitions).
  - l (softmax denom) via a ones-column stationary matmul accumulated in
    psum; normalize out tiles with reciprocal + gpsimd partition_broadcast
    + one DVE multiply per (head, query-group).
  - projections in bf16 (x and Wq/Wk/Wv/Wdt host-packed contiguous, so
    DMA is large-descriptor); x resident in SBUF, read once.
  - v natural-layout tiles kept in SBUF (no DRAM bounce).
  - dyn/kth bisection identical to baseline (31-step float-bit bisection),
    overlapped under the q/k/v projections; dynT obtained by tiny PE
    transposes instead of a DRAM round trip.
  - fully-masked (degenerate) rows: l==0 detected on host via l output,
    recomputed faithfully in numpy (expected ~1 row per batch*head).
"""
import sys
import numpy as np
import ml_dtypes

BF16NP = ml_dtypes.bfloat16

sys.path.insert(0, "/root/.axon_site/_ro/trn_rl_repo")

import concourse.bass as bass  # noqa: E402,F401
from concourse import bacc  # noqa: E402
import concourse.tile as tile  # noqa: E402
import concourse.mybir as mybir  # noqa: E402
from concourse.bass_utils import run_bass_kernel_spmd  # noqa: E402
from concourse.alu_op_type import AluOpType  # noqa: E402

F32 = mybir.dt.float32
F32R = mybir.dt.float32r
BF16 = mybir.dt.bfloat16
I32 = mybir.dt.int32
AF = mybir.ActivationFunctionType
AX = mybir.AxisListType.X

B, S, HID = 2, 2048, 2048
H, KV, D = 16, 8, 128
HPC, KVPC = 4, 2
GROUPS = H // KV
NUM_DYN = S // 2
SCALING = D ** -0.5
MIN = float(np.finfo(np.float32).min)
BIG = 1.7e38
P = 128
NT = S // P          # 16
NQ = 4
QW = S // NQ         # 512
NCORES = 8

_cache = {}


def _build_program(blkstate):
    key = ("nc", blkstate)
    if key in _cache:
        return _cache[key]
    nvar = _num_varblocks(blkstate)
    nc = bacc.Bacc("TRN2", target_bir_lowering=False, debug=False,
                   num_devices=NCORES)
    dram = {}
    for name, shape, dt in [
            ("xP", [P, NQ * NT * QW], BF16),
            ("xPf", [P, NQ * NT * QW], F32R),
            ("wqP", [P, HPC * NT * P], BF16),
            ("wkP", [P, KVPC * NT * P], BF16),
            ("wvP", [P, KVPC * NT * P], BF16),
            ("wdtvPr", [P, NT * HPC], F32R),
            ("woP", [P, NT * HPC * P], F32),
            ("cosT", [D, S], F32), ("sinT", [D, S], F32),
            ("acol", [HPC, 1], F32),
            ("eye", [P, P], F32), ("perm", [P, P], F32),
            ("varblkT", [P, max(nvar, 1) * P], F32)]:
        dram[name] = nc.dram_tensor(name, shape, dt, kind="ExternalInput").ap()
    outT_d = nc.dram_tensor("outT", [HID, S], F32, kind="ExternalOutput").ap()
    l_d = nc.dram_tensor("l_out", [HPC, S], F32, kind="ExternalOutput").ap()

    with tile.TileContext(nc) as tc:
        _emit(nc, tc, dram, outT_d, l_d, blkstate)
    nc.compile()
    _cache[key] = nc
    return nc


def _num_varblocks(blkstate):
    n = 0
    for t in range(NT):
        for j in range(NT):
            if blkstate[t][j].startswith("V"):
                n = max(n, int(blkstate[t][j][2:]) + 1)
    return n


def _emit(nc, tc, dram, outT_d, l_d, blkstate):
    from contextlib import ExitStack
    ctx = ExitStack()

    # per-tile computed extent (in key chunks): chunks j < extc[t] participate
    extc = []
    for t in range(NT):
        nz = [j for j in range(NT) if blkstate[t][j] != "M"]
        assert nz and min(nz) == 0, "chunk 0 must be active for every tile"
        extc.append(max(nz) + 1)

    consts = ctx.enter_context(tc.tile_pool(name="consts", bufs=1))

    # dt-critical consts first on the sync ring so the dt pass starts
    # immediately; all bulk loads go on the Activation DGE ring.
    wdtv = consts.tile([P, NT * HPC], F32R, name="c_wdtv")
    nc.sync.dma_start(wdtv[:], dram["wdtvPr"])
    acol_t = consts.tile([HPC, 1], F32, name="c_acol")
    nc.sync.dma_start(acol_t[:], dram["acol"])
    onescol_b = consts.tile([P, 1], BF16, name="onescol")
    nc.vector.memset(onescol_b[:], 1.0)
    kthc = consts.tile([HPC, 1], F32, name="kthc")
    nc.vector.memset(kthc[:], float(NUM_DYN) - 0.5)

    eye_r = consts.tile([P, P], F32R, name="cr_eye")
    perm_r = consts.tile([P, P], F32R, name="cr_perm")
    nvar = _num_varblocks(blkstate)
    varblkT = consts.tile([P, max(nvar, 1) * P], F32, name="c_varblkT")

    csp = ctx.enter_context(tc.tile_pool(name="csp", bufs=1))
    cos_t = csp.tile([D, S], F32, name="cos_t")
    sin_t = csp.tile([D, S], F32, name="sin_t")

    # side consts on the gpsimd software-DGE ring, in parallel with the
    # dt-critical sync/Activation traffic
    nc.gpsimd.dma_start(cos_t[:], dram["cosT"])
    nc.gpsimd.dma_start(sin_t[:], dram["sinT"])
    nc.gpsimd.dma_start(varblkT[:], dram["varblkT"])
    cstg = ctx.enter_context(tc.tile_pool(name="cstg", bufs=2))
    for nm, dst in [("eye", eye_r), ("perm", perm_r)]:
        t = cstg.tile([P, P], F32, name=f"s_{nm}", tag="s")
        nc.gpsimd.dma_start(t[:], dram[nm])
        nc.scalar.copy(dst[:], t[:])

    act = ctx.enter_context(tc.tile_pool(name="act", bufs=1))
    qkro = [act.tile([P, S], F32R, name=f"qro{h}") for h in range(HPC)]
    kro = [act.tile([P, S], F32R, name=f"kro{i}") for i in range(KVPC)]
    vnat = [act.tile([P, NT * P], BF16, name=f"vnat{i}") for i in range(KVPC)]
    dynT = act.tile([P, NT * HPC], F32, name="dynT")

    with ExitStack() as ctx1:
        xsp = ctx1.enter_context(tc.tile_pool(name="xsp", bufs=1))
        xs = [xsp.tile([P, NT * QW], BF16, name=f"xs{sg}")
              for sg in range(NQ)]
        vop = ctx1.enter_context(tc.tile_pool(name="vop", bufs=1))
        vT_own = [vop.tile([P, S], F32R, name=f"vTown{i}") for i in range(KVPC)]
        dt_sb = vop.tile([HPC, S], F32, name="dt_sb")

        # ---- dt pass (f32-accurate: decides the kth mask set) merged ----
        # with the projections; dt chains interleave with v-projections so
        # the PE stays fed while dt's x stream arrives. The dyn chain +
        # bisection is emitted right after the last dt chain so the scalar
        # and DVE queues reach it early (both are in-order engines).
        dyq = ctx1.enter_context(tc.tile_pool(name="dyq", bufs=1))
        kth_f = dyq.tile([HPC, 1], I32, name="kth_f")
        dynrow = dyq.tile([HPC, S], F32R, name="dynrow")
        dyn_t = dyq.tile([HPC, S], F32, name="dyn_t")
        work = dyq.tile([HPC, S], F32, name="work")
        # work is dead after the dyn chain; reuse its storage for the
        # bisection scratch (bf16 view) and later the penalty tile
        scr = work[:].bitcast(BF16)[:, 0:S]
        pen = work

        def emit_dyn_bisect():
            nc.scalar.activation(work[:], dt_sb[:], AF.Exp)
            nc.scalar.activation(work[:], work[:], AF.Ln, bias=1.0)
            nc.scalar.activation(dyn_t[:], work[:], AF.Exp, scale=acol_t[:])
            lo = dyq.tile([HPC, 1], I32, name="lo")
            hi = dyq.tile([HPC, 1], I32, name="hi")
            mid = dyq.tile([HPC, 1], I32, name="mid")
            dlt = dyq.tile([HPC, 1], I32, name="dlt")
            cges = dyq.tile([HPC, 1], I32, name="cges")
            cltv = dyq.tile([HPC, 1], I32, name="cltv")
            cnt = dyq.tile([HPC, 1], F32, name="cnt")
            nc.vector.memset(lo[:], 0)
            nc.vector.memset(hi[:], 0x7F800000)
            for _ in range(31):
                # mid = (lo + hi) >>> 1 (bit values < 2^31 so the unsigned
                # average is exact under logical shift)
                nc.vector.tensor_tensor(mid[:], hi[:], lo[:],
                                        op=AluOpType.add)
                nc.vector.tensor_scalar(mid[:], mid[:], 1, None,
                                        op0=AluOpType.logical_shift_right)
                nc.vector.tensor_scalar(scr, dyn_t[:],
                                        mid[:, 0:1].bitcast(F32), 0.0,
                                        op0=AluOpType.is_lt,
                                        op1=AluOpType.add,
                                        accum_out=cnt[:])
                nc.vector.tensor_scalar(cges[:], kthc[:], cnt[:, 0:1], None,
                                        op0=AluOpType.is_lt)
                nc.vector.tensor_scalar(cltv[:], kthc[:], cnt[:, 0:1], None,
                                        op0=AluOpType.is_ge)
                nc.vector.copy_predicated(hi[:], cges[:], mid[:])
                nc.vector.copy_predicated(lo[:], cltv[:], mid[:])
            nc.vector.tensor_copy(kth_f[:], lo[:])
            nc.vector.tensor_scalar(pen[:], dyn_t[:],
                                    kth_f[:, 0:1].bitcast(F32), -BIG,
                                    op0=AluOpType.is_lt, op1=AluOpType.mult)
            nc.vector.tensor_tensor(dynrow[:], dyn_t[:], pen[:],
                                    op=AluOpType.add)

        with tc.tile_pool(name="dps", bufs=2, space="PSUM") as dps, \
             tc.tile_pool(name="dtx", bufs=2) as dtx, \
             tc.tile_pool(name="wp", bufs=2) as wp, \
             tc.tile_pool(name="pjp", bufs=4) as pjp, \
             tc.tile_pool(name="pps", bufs=6, space="PSUM") as pps:

            def emit_dt(sg):
                dtp = dps.tile([HPC, QW], F32, name="dtp", tag="dtp")
                for cc in range(NT):
                    xf = dtx.tile([P, QW], F32R, name="xf", tag="xf")
                    ring = nc.sync if cc % 2 == 0 else nc.scalar
                    ring.dma_start(
                        xf[:], dram["xPf"][:, (sg * NT + cc) * QW:
                                           (sg * NT + cc + 1) * QW])
                    nc.tensor.matmul(dtp[:], wdtv[:, cc * HPC:(cc + 1) * HPC],
                                     xf[:],
                                     start=(cc == 0), stop=(cc == NT - 1))
                # DVE copy: keeps the in-order scalar queue free for DMA
                # issues and the dyn chain
                nc.vector.tensor_copy(dt_sb[:, sg * QW:(sg + 1) * QW], dtp[:])
                nc.scalar.dma_start(
                    xs[sg][:], dram["xP"][:, sg * NT * QW:(sg + 1) * NT * QW])

            wsrc = {"v": "wvP", "q": "wqP", "k": "wkP"}

            def emit_proj(kind, oi):
                wfull = wp.tile([P, NT * P], BF16, name="wfull", tag="wf")
                nc.sync.dma_start(
                    wfull[:],
                    dram[wsrc[kind]][:, oi * NT * P:(oi + 1) * NT * P])
                for sg in range(NQ):
                    ps = pps.tile([P, QW], F32, name="ps", tag="ps")
                    for cc in range(NT):
                        nc.tensor.matmul(ps[:], wfull[:, cc * P:(cc + 1) * P],
                                         xs[sg][:, cc * QW:(cc + 1) * QW],
                                         start=(cc == 0), stop=(cc == NT - 1))
                    if kind == "v":
                        # scalar engine: its queue reaches these after the
                        # dyn chain, so they never gate the bisection
                        nc.scalar.copy(
                            vT_own[oi][:, sg * QW:(sg + 1) * QW], ps[:])
                    else:
                        pj = pjp.tile([P, QW], F32R, name="pj", tag="pj")
                        nc.scalar.copy(pj[:], ps[:])
                        rh = pps.tile([P, QW], F32, name="rh", tag="ps")
                        nc.tensor.matmul(rh[:], perm_r[:], pj[:],
                                         start=True, stop=True)
                        # gpsimd cannot read PSUM: stage rh through SBUF on
                        # the scalar engine, then do all RoPE elementwise
                        # work on gpsimd (DVE is busy with the bisection and
                        # its in-order queue would pin pjp tiles for ~70us)
                        rhs = pjp.tile([P, QW], F32, name="rhs", tag="pj")
                        nc.scalar.copy(rhs[:], rh[:])
                        t1 = pjp.tile([P, QW], F32, name="t1", tag="pj")
                        nc.gpsimd.tensor_tensor(
                            t1[:], rhs[:], sin_t[:, sg * QW:(sg + 1) * QW],
                            op=AluOpType.mult)
                        t2 = pjp.tile([P, QW], F32, name="t2", tag="pj")
                        nc.gpsimd.tensor_tensor(
                            t2[:], pj[:], cos_t[:, sg * QW:(sg + 1) * QW],
                            op=AluOpType.mult)
                        dstro = (qkro[oi] if kind == "q" else kro[oi])
                        nc.gpsimd.tensor_tensor(
                            dstro[:, sg * QW:(sg + 1) * QW], t1[:], t2[:],
                            op=AluOpType.add)

            # dt chains first (DMA-paced), then the dyn chain + bisection so
            # its scalar/DVE ops sit ahead of all projection copies in the
            # in-order queues; projections follow and overlap the bisection.
            for sg in range(NQ):
                emit_dt(sg)
            emit_dyn_bisect()
            for kind, oi in [("v", 0), ("v", 1),
                             ("q", 0), ("q", 1), ("q", 2), ("q", 3),
                             ("k", 0), ("k", 1)]:
                emit_proj(kind, oi)

        # ---------------- natural-layout v tiles (SBUF resident) --------
        with tc.tile_pool(name="vps", bufs=4, space="PSUM") as vps:
            for i in range(KVPC):
                for cc in range(NT):
                    pt = vps.tile([P, P], F32, name="vt", tag="vt")
                    nc.tensor.transpose(pt[:].bitcast(F32R),
                                        vT_own[i][:, cc * P:(cc + 1) * P],
                                        eye_r[:])
                    nc.scalar.copy(vnat[i][:, cc * P:(cc + 1) * P], pt[:])

        # dynT transposes last in the PE queue before attention: they wait
        # on the DVE bisection, so anything emitted after them would stall
        # the in-order PE queue (cost a 122us bubble when emitted early).
        with tc.tile_pool(name="dtp2", bufs=1, space="PSUM") as dtp2:
            dyn_ps = dtp2.tile([P, NT * HPC], F32, name="dyn_ps")
            for cc in range(NT):
                nc.tensor.transpose(
                    dyn_ps[:, cc * HPC:(cc + 1) * HPC].bitcast(F32R),
                    dynrow[:, cc * P:(cc + 1) * P], eye_r[0:HPC, 0:HPC])
            nc.scalar.copy(dynT[:], dyn_ps[:])

    # ---------------- attention (transposed scores) + outproj ----------
    # wo resident: loaded once (not once per query-group), via the
    # Activation DGE ring while the first group's attention runs
    wop = ctx.enter_context(tc.tile_pool(name="wop", bufs=1))
    wos = []
    for ht in range(NT):
        wo = wop.tile([P, HPC * P], F32R, name=f"wo{ht}")
        nc.gpsimd.dma_start(
            wo[:], dram["woP"][:, ht * HPC * P:(ht + 1) * HPC * P])
        wos.append(wo)
    with tc.tile_pool(name="scp", bufs=2, space="PSUM") as scp, \
         tc.tile_pool(name="ovl", bufs=2, space="PSUM") as ovl, \
         tc.tile_pool(name="lpp", bufs=2, space="PSUM") as lpp, \
         tc.tile_pool(name="ptp", bufs=3) as ptp, \
         tc.tile_pool(name="atn", bufs=8) as atn, \
         tc.tile_pool(name="lnb", bufs=2) as lnb, \
         tc.tile_pool(name="lnv", bufs=2) as lnv, \
         tc.tile_pool(name="oub", bufs=4) as oub, \
         tc.tile_pool(name="ops", bufs=2, space="PSUM") as ops:
        for grp in range(NQ):
            base = grp * QW
            tiles = list(range(grp * 4, grp * 4 + 4))
            jmax = max(extc[t] for t in tiles)
            at_grp = {}
            for h in range(HPC):
                kv = h // GROUPS
                ovp = ovl.tile([P, QW], F32, name="ovp", tag="ovp")
                lps = lpp.tile([1, QW], F32, name="lps", tag="lps")

                qlos = []
                for j in range(jmax):
                    acts = [t for t in tiles if j < extc[t]]
                    assert acts == tiles[-len(acts):], \
                        "active tiles must be a suffix of the group"
                    qlos.append(acts[0] * P - base)

                def emit_score(j):
                    qlo = qlos[j]
                    sc = scp.tile([P, QW], F32, name="sc", tag="sc")
                    nc.tensor.matmul(
                        sc[:, qlo:QW], kro[kv][:, j * P:(j + 1) * P],
                        qkro[h][:, base + qlo:base + QW],
                        start=True, stop=True, skip_group_check=True)
                    for t in tiles:
                        if j >= extc[t]:
                            continue
                        st = blkstate[t][j]
                        if st.startswith("V"):
                            vi = int(st[2:])
                            off = t * P - base
                            nc.vector.tensor_tensor(
                                sc[:, off:off + P], sc[:, off:off + P],
                                varblkT[:, vi * P:(vi + 1) * P],
                                op=AluOpType.add)
                    pt = ptp.tile([P, QW], BF16, name="pt", tag="pt")
                    nc.scalar.activation(
                        pt[:, qlo:QW], sc[:, qlo:QW], AF.Exp,
                        bias=dynT[:, j * HPC + h:j * HPC + h + 1])
                    return pt

                # software-pipeline by one chunk: emit chunk j+1's score
                # matmul before chunk j's l/av matmuls so the PE works
                # through the exp latency instead of waiting on it.
                pts = emit_score(0)
                for j in range(jmax):
                    pt, qlo = pts, qlos[j]
                    if j + 1 < jmax:
                        pts = emit_score(j + 1)
                    nc.tensor.matmul(
                        lps[:, qlo:QW], onescol_b[:], pt[:, qlo:QW],
                        start=(j == 0), stop=(j == jmax - 1),
                        skip_group_check=True)
                    nc.tensor.matmul(
                        ovp[:, qlo:QW], vnat[kv][:, j * P:(j + 1) * P],
                        pt[:, qlo:QW],
                        start=(j == 0), stop=(j == jmax - 1),
                        skip_group_check=True)
                lsb = lnv.tile([1, QW], F32, name="lsb", tag="lv")
                nc.scalar.copy(lsb[:], lps[:])
                nc.sync.dma_start(l_d[h:h + 1, base:base + QW], lsb[:])
                linv = lnv.tile([1, QW], F32, name="linv", tag="lv")
                nc.vector.reciprocal_approx_fast(linv[:], lps[:])
                lbc = lnb.tile([P, QW], F32, name="lbc", tag="lb")
                nc.gpsimd.partition_broadcast(lbc[:], linv[:])
                at = atn.tile([P, QW], F32R, name="at", tag="at")
                nc.vector.tensor_tensor(at[:], ovp[:], lbc[:],
                                        op=AluOpType.mult)
                at_grp[h] = at
            for ht in range(NT):
                op = ops.tile([P, QW], F32, name="op", tag="op")
                for h in range(HPC):
                    nc.tensor.matmul(op[:], wos[ht][:, h * P:(h + 1) * P],
                                     at_grp[h][:], start=(h == 0),
                                     stop=(h == HPC - 1))
                osb = oub.tile([P, QW], F32, name="osb", tag="ob")
                # alternate the psum drain between scalar and DVE so
                # neither in-order queue delays the next group's exps
                if ht % 2 == 0:
                    nc.scalar.copy(osb[:], op[:])
                else:
                    nc.vector.tensor_copy(osb[:], op[:])
                nc.sync.dma_start(
                    outT_d[ht * P:(ht + 1) * P, base:base + QW], osb[:])
    ctx.close()


def _host_prep(hidden_states, cos, sin, attention_mask, Wq, Wk, Wv, A, Wdt, Wo):
    eye = np.eye(P, dtype=np.float32)
    perm = np.zeros((P, P), dtype=np.float32)
    for j in range(64):
        perm[j + 64, j] = -1.0
        perm[j, j + 64] = 1.0

    def pack_w(wT, nblk):
        # wT [HID, nblk*P] f32 -> [P, nblk*NT*P] bf16:
        # [p, (oi*NT+c)*P+f] = wT[c*P+p, oi*P+f]
        w4 = wT.reshape(NT, P, nblk, P)            # [c, p, oi, f]
        return np.ascontiguousarray(
            w4.transpose(1, 2, 0, 3).reshape(P, nblk * NT * P)
        ).astype(np.float32)

    in_maps = []
    blkstates = []
    for c in range(NCORES):
        b, g = divmod(c, 4)
        heads = list(range(4 * g, 4 * g + 4))
        # x packed: [p, ((sg*NT)+c)*QW+f] = x[b][sg*QW+f, c*P+p]
        xb = np.asarray(hidden_states[b], dtype=np.float32)
        xP = np.ascontiguousarray(
            xb.reshape(NQ, QW, NT, P).transpose(3, 0, 2, 1)
            .reshape(P, NQ * NT * QW))
        wqT = (Wq[4 * g * D:(4 * g + 4) * D] * np.float32(SCALING)).T
        wkT = Wk[2 * g * D:(2 * g + 2) * D].T
        wvT = Wv[2 * g * D:(2 * g + 2) * D].T
        wqP = pack_w(np.ascontiguousarray(wqT), HPC)
        wkP = pack_w(np.ascontiguousarray(wkT), KVPC)
        wvP = pack_w(np.ascontiguousarray(wvT), KVPC)
        wdtvT = np.ascontiguousarray(
            (Wdt[heads].astype(np.float64) @ Wv.astype(np.float64))
            .T.astype(np.float32))                 # [HID, 4]
        wdtvP = np.ascontiguousarray(
            wdtvT.reshape(NT, P, HPC).transpose(1, 0, 2).reshape(P, NT * HPC))
        # woP: [p, (ht*HPC+h)*P+f] = WoT[h*P+p, ht*P+f]
        woT = np.ascontiguousarray(Wo[:, 4 * g * D:(4 * g + 4) * D].T)
        woP = np.ascontiguousarray(
            woT.reshape(HPC, P, NT, P).transpose(1, 2, 0, 3)
            .reshape(P, NT * HPC * P))
        acol = A[heads].astype(np.float32).reshape(HPC, 1)
        cosT = np.ascontiguousarray(cos[b].T)
        sinT = np.ascontiguousarray(sin[b].T)
        m = attention_mask[b, 0]
        mb = np.asarray(m).reshape(NT, P, NT, P)
        blkrows = []
        varlist = []
        varkeys = {}
        for t in range(NT):
            row = []
            for j in range(NT):
                blkv = mb[t, :, j, :]
                if np.all(blkv == 0):
                    row.append("Z")
                elif np.all(blkv <= -1e30):
                    row.append("M")
                else:
                    bT = np.ascontiguousarray(
                        np.maximum(blkv, -BIG).T)  # [key, query]
                    kk = bT.tobytes()
                    if kk not in varkeys:
                        varkeys[kk] = len(varlist)
                        varlist.append(bT)
                    row.append(f"V:{varkeys[kk]}")
            # interior M blocks (before a later non-M block) become varying
            nz = [j for j in range(NT) if row[j] != "M"]
            lim = (max(nz) + 1) if nz else 0
            for j in range(lim):
                if row[j] == "M":
                    bT = np.full((P, P), -BIG, np.float32)
                    kk = bT.tobytes()
                    if kk not in varkeys:
                        varkeys[kk] = len(varlist)
                        varlist.append(bT)
                    row[j] = f"V:{varkeys[kk]}"
            blkrows.append(tuple(row))
        if len(varlist) > 8:
            raise NotImplementedError("too many varying mask blocks")
        varblkT = np.zeros((P, max(len(varlist), 1) * P), dtype=np.float32)
        for vi, blkv in enumerate(varlist):
            varblkT[:, vi * P:(vi + 1) * P] = blkv
        blkstate = tuple(blkrows)
        in_maps.append({
            "xP": xP.astype(BF16NP), "xPf": xP,
            "wqP": wqP.astype(BF16NP),
            "wkP": wkP.astype(BF16NP), "wvP": wvP.astype(BF16NP),
            "wdtvPr": wdtvP, "woP": woP, "cosT": cosT,
            "sinT": sinT, "acol": acol, "eye": eye, "perm": perm,
            "varblkT": varblkT,
        })
        blkstates.append(blkstate)
    if len(set(blkstates)) != 1:
        raise NotImplementedError("mask structure differs across batches")
    return in_maps, blkstates[0]


def _softplus64(x):
    x = x.astype(np.float64)
    return np.log1p(np.exp(-np.abs(x))) + np.maximum(x, 0)


def _repair_rows(out, bad, inputs):
    """Recompute rows flagged bad [B, S] with faithful numpy reference math."""
    if not bad.any():
        return out
    hs = inputs["hidden_states"]; cos = inputs["cos"]; sin = inputs["sin"]
    am = inputs["attention_mask"]; Wq = inputs["Wq"]; Wk = inputs["Wk"]
    Wv = inputs["Wv"]; A = inputs["A"]; Wdt = inputs["Wdt"]; Wo = inputs["Wo"]

    def rope(x, c, s):
        x1, x2 = x[..., :D // 2], x[..., D // 2:]
        return x * c + np.concatenate([-x2, x1], axis=-1) * s

    for b in range(B):
        rows = np.where(bad[b])[0]
        if len(rows) == 0:
            continue
        x = hs[b].astype(np.float32)
        k = (x @ Wk.T).reshape(S, KV, D)
        v = (x @ Wv.T).reshape(S, KV, D)
        k = rope(k, cos[b][:, None, :], sin[b][:, None, :])
        v_flat = v.reshape(S, KV * D)
        dt = v_flat @ Wdt.T
        dyn = np.exp(A[None, :] * _softplus64(dt)).astype(np.float32).T
        kth = np.sort(dyn, axis=-1)[:, NUM_DYN - 1:NUM_DYN]
        dmask = np.where(dyn < kth, MIN, dyn).astype(np.float32)
        for s_i in rows:
            q_row = (x[s_i] @ Wq.T).reshape(H, D)
            q_row = rope(q_row, cos[b][s_i][None, :], sin[b][s_i][None, :])
            attn_row = np.zeros((H, D), dtype=np.float32)
            for h in range(H):
                kvh = h // GROUPS
                sc = ((q_row[h] @ k[:, kvh].T) * np.float32(SCALING)
                      + (dmask[h] + am[b, 0, s_i])).astype(np.float32)
                w = np.exp(sc - sc.max())
                w = (w / w.sum()).astype(np.float32)
                attn_row[h] = w @ v[:, kvh]
            out[b, s_i] = attn_row.reshape(H * D) @ Wo.T
    return out


def kernel(**inputs):
    inputs = {k: np.asarray(v) for k, v in inputs.items()}
    in_maps, blkstate = _host_prep(**inputs)
    nc = _build_program(blkstate)
    res = run_bass_kernel_spmd(nc, in_maps, list(range(NCORES)))
    out = np.zeros((B, S, HID), dtype=np.float32)
    bad = np.zeros((B, S), dtype=bool)
    for c in range(NCORES):
        b = c // 4
        out[b] += res.results[c]["outT"].T
        bad[b] |= (res.results[c]["l_out"] == 0).any(axis=0)
    bad |= ~np.isfinite(out).all(axis=2)
    out = _repair_rows(out, bad, inputs)
    return out


# revision 65
# speedup vs baseline: 1.8257x; 1.0001x over previous
"""DogeDynamicMaskAttention Trainium2 kernel (transposed-scores redesign).

Sharding: 8 cores = 2 batches x 4 head-groups. Core c: batch b=c//4,
head-group g=c%4 -> heads [4g..4g+4), kv heads {2g, 2g+1}.

Design vs previous baseline:
  - scores computed TRANSPOSED [keys, queries]: the dynamic mask row is a
    per-partition (per-key) bias folded into the exp activation for free;
    the P-matrix transposes + f32r casts of the old layout vanish; the
    attn@v matmul consumes exp output directly (keys on partitions).
  - l (softmax denom) via a ones-column stationary matmul accumulated in
    psum; normalize out tiles with reciprocal + gpsimd partition_broadcast
    + one DVE multiply per (head, query-group).
  - projections in bf16 (x and Wq/Wk/Wv/Wdt host-packed contiguous, so
    DMA is large-descriptor); x resident in SBUF, read once.
  - v natural-layout tiles kept in SBUF (no DRAM bounce).
  - dyn/kth bisection identical to baseline (31-step float-bit bisection),
    overlapped under the q/k/v projections; dynT obtained by tiny PE
    transposes instead of a DRAM round trip.
  - fully-masked (degenerate) rows: l==0 detected on host via l output,
    recomputed faithfully in numpy (expected ~1 row per batch*head).
"""
import sys
import numpy as np
import ml_dtypes

BF16NP = ml_dtypes.bfloat16

sys.path.insert(0, "/root/.axon_site/_ro/trn_rl_repo")

import concourse.bass as bass  # noqa: E402,F401
from concourse import bacc  # noqa: E402
import concourse.tile as tile  # noqa: E402
import concourse.mybir as mybir  # noqa: E402
from concourse.bass_utils import run_bass_kernel_spmd  # noqa: E402
from concourse.alu_op_type import AluOpType  # noqa: E402

F32 = mybir.dt.float32
F32R = mybir.dt.float32r
BF16 = mybir.dt.bfloat16
I32 = mybir.dt.int32
AF = mybir.ActivationFunctionType
AX = mybir.AxisListType.X

B, S, HID = 2, 2048, 2048
H, KV, D = 16, 8, 128
HPC, KVPC = 4, 2
GROUPS = H // KV
NUM_DYN = S // 2
SCALING = D ** -0.5
MIN = float(np.finfo(np.float32).min)
BIG = 1.7e38
P = 128
NT = S // P          # 16
NQ = 4
QW = S // NQ         # 512
NCORES = 8

_cache = {}


def _build_program(blkstate):
    key = ("nc", blkstate)
    if key in _cache:
        return _cache[key]
    nvar = _num_varblocks(blkstate)
    nc = bacc.Bacc("TRN2", target_bir_lowering=False, debug=False,
                   num_devices=NCORES)
    dram = {}
    for name, shape, dt in [
            ("xP", [P, NQ * NT * QW], BF16),
            ("xPf", [P, NQ * NT * QW], F32R),
            ("wqP", [P, HPC * NT * P], BF16),
            ("wkP", [P, KVPC * NT * P], BF16),
            ("wvP", [P, KVPC * NT * P], BF16),
            ("wdtvPr", [P, NT * HPC], F32R),
            ("woP", [P, NT * HPC * P], F32),
            ("cosT", [D, S], F32), ("sinT", [D, S], F32),
            ("acol", [HPC, 1], F32),
            ("eye", [P, P], F32), ("perm", [P, P], F32),
            ("varblkT", [P, max(nvar, 1) * P], F32)]:
        dram[name] = nc.dram_tensor(name, shape, dt, kind="ExternalInput").ap()
    outT_d = nc.dram_tensor("outT", [HID, S], F32, kind="ExternalOutput").ap()
    l_d = nc.dram_tensor("l_out", [HPC, S], F32, kind="ExternalOutput").ap()

    with tile.TileContext(nc) as tc:
        _emit(nc, tc, dram, outT_d, l_d, blkstate)
    nc.compile()
    _cache[key] = nc
    return nc


def _num_varblocks(blkstate):
    n = 0
    for t in range(NT):
        for j in range(NT):
            if blkstate[t][j].startswith("V"):
                n = max(n, int(blkstate[t][j][2:]) + 1)
    return n


def _emit(nc, tc, dram, outT_d, l_d, blkstate):
    from contextlib import ExitStack
    ctx = ExitStack()

    # per-tile computed extent (in key chunks): chunks j < extc[t] participate
    extc = []
    for t in range(NT):
        nz = [j for j in range(NT) if blkstate[t][j] != "M"]
        assert nz and min(nz) == 0, "chunk 0 must be active for every tile"
        extc.append(max(nz) + 1)

    consts = ctx.enter_context(tc.tile_pool(name="consts", bufs=1))

    # dt-critical consts first on the sync ring so the dt pass starts
    # immediately; all bulk loads go on the Activation DGE ring.
    wdtv = consts.tile([P, NT * HPC], F32R, name="c_wdtv")
    nc.sync.dma_start(wdtv[:], dram["wdtvPr"])
    acol_t = consts.tile([HPC, 1], F32, name="c_acol")
    nc.sync.dma_start(acol_t[:], dram["acol"])
    onescol_b = consts.tile([P, 1], BF16, name="onescol")
    nc.vector.memset(onescol_b[:], 1.0)
    kthc = consts.tile([HPC, 1], F32, name="kthc")
    nc.vector.memset(kthc[:], float(NUM_DYN) - 0.5)

    eye_r = consts.tile([P, P], F32R, name="cr_eye")
    perm_r = consts.tile([P, P], F32R, name="cr_perm")
    nvar = _num_varblocks(blkstate)
    varblkT = consts.tile([P, max(nvar, 1) * P], F32, name="c_varblkT")

    csp = ctx.enter_context(tc.tile_pool(name="csp", bufs=1))
    cos_t = csp.tile([D, S], F32, name="cos_t")
    sin_t = csp.tile([D, S], F32, name="sin_t")

    # side consts on the gpsimd software-DGE ring, in parallel with the
    # dt-critical sync/Activation traffic
    nc.gpsimd.dma_start(cos_t[:], dram["cosT"])
    nc.gpsimd.dma_start(sin_t[:], dram["sinT"])
    nc.gpsimd.dma_start(varblkT[:], dram["varblkT"])
    cstg = ctx.enter_context(tc.tile_pool(name="cstg", bufs=2))
    for nm, dst in [("eye", eye_r), ("perm", perm_r)]:
        t = cstg.tile([P, P], F32, name=f"s_{nm}", tag="s")
        nc.gpsimd.dma_start(t[:], dram[nm])
        nc.scalar.copy(dst[:], t[:])

    act = ctx.enter_context(tc.tile_pool(name="act", bufs=1))
    qkro = [act.tile([P, S], F32R, name=f"qro{h}") for h in range(HPC)]
    kro = [act.tile([P, S], F32R, name=f"kro{i}") for i in range(KVPC)]
    vnat = [act.tile([P, NT * P], BF16, name=f"vnat{i}") for i in range(KVPC)]
    dynT = act.tile([P, NT * HPC], F32, name="dynT")

    with ExitStack() as ctx1:
        xsp = ctx1.enter_context(tc.tile_pool(name="xsp", bufs=1))
        xs = [xsp.tile([P, NT * QW], BF16, name=f"xs{sg}")
              for sg in range(NQ)]
        vop = ctx1.enter_context(tc.tile_pool(name="vop", bufs=1))
        vT_own = [vop.tile([P, S], F32R, name=f"vTown{i}") for i in range(KVPC)]
        dt_sb = vop.tile([HPC, S], F32, name="dt_sb")

        # ---- dt pass (f32-accurate: decides the kth mask set) merged ----
        # with the projections; dt chains interleave with v-projections so
        # the PE stays fed while dt's x stream arrives. The dyn chain +
        # bisection is emitted right after the last dt chain so the scalar
        # and DVE queues reach it early (both are in-order engines).
        dyq = ctx1.enter_context(tc.tile_pool(name="dyq", bufs=1))
        kth_f = dyq.tile([HPC, 1], I32, name="kth_f")
        dynrow = dyq.tile([HPC, S], F32R, name="dynrow")
        dyn_t = dyq.tile([HPC, S], F32, name="dyn_t")
        work = dyq.tile([HPC, S], F32, name="work")
        # work is dead after the dyn chain; reuse its storage for the
        # bisection scratch (bf16 view) and later the penalty tile
        scr = work[:].bitcast(BF16)[:, 0:S]
        pen = work

        def emit_dyn_bisect():
            nc.scalar.activation(work[:], dt_sb[:], AF.Exp)
            nc.scalar.activation(work[:], work[:], AF.Ln, bias=1.0)
            nc.scalar.activation(dyn_t[:], work[:], AF.Exp, scale=acol_t[:])
            lo = dyq.tile([HPC, 1], I32, name="lo")
            hi = dyq.tile([HPC, 1], I32, name="hi")
            mid = dyq.tile([HPC, 1], I32, name="mid")
            dlt = dyq.tile([HPC, 1], I32, name="dlt")
            cges = dyq.tile([HPC, 1], I32, name="cges")
            cltv = dyq.tile([HPC, 1], I32, name="cltv")
            cnt = dyq.tile([HPC, 1], F32, name="cnt")
            nc.vector.memset(lo[:], 0)
            nc.vector.memset(hi[:], 0x7F800000)
            for _ in range(31):
                # mid = (lo + hi) >>> 1 (bit values < 2^31 so the unsigned
                # average is exact under logical shift)
                nc.vector.tensor_tensor(mid[:], hi[:], lo[:],
                                        op=AluOpType.add)
                nc.vector.tensor_scalar(mid[:], mid[:], 1, None,
                                        op0=AluOpType.logical_shift_right)
                nc.vector.tensor_scalar(scr, dyn_t[:],
                                        mid[:, 0:1].bitcast(F32), 0.0,
                                        op0=AluOpType.is_lt,
                                        op1=AluOpType.add,
                                        accum_out=cnt[:])
                nc.vector.tensor_scalar(cges[:], kthc[:], cnt[:, 0:1], None,
                                        op0=AluOpType.is_lt)
                nc.vector.tensor_scalar(cltv[:], kthc[:], cnt[:, 0:1], None,
                                        op0=AluOpType.is_ge)
                nc.vector.copy_predicated(hi[:], cges[:], mid[:])
                nc.vector.copy_predicated(lo[:], cltv[:], mid[:])
            nc.vector.tensor_copy(kth_f[:], lo[:])
            nc.vector.tensor_scalar(pen[:], dyn_t[:],
                                    kth_f[:, 0:1].bitcast(F32), -BIG,
                                    op0=AluOpType.is_lt, op1=AluOpType.mult)
            nc.vector.tensor_tensor(dynrow[:], dyn_t[:], pen[:],
                                    op=AluOpType.add)

        with tc.tile_pool(name="dps", bufs=2, space="PSUM") as dps, \
             tc.tile_pool(name="dtx", bufs=2) as dtx, \
             tc.tile_pool(name="wp", bufs=2) as wp, \
             tc.tile_pool(name="pjp", bufs=4) as pjp, \
             tc.tile_pool(name="pps", bufs=6, space="PSUM") as pps:

            def emit_dt(sg):
                dtp = dps.tile([HPC, QW], F32, name="dtp", tag="dtp")
                for cc in range(NT):
                    xf = dtx.tile([P, QW], F32R, name="xf", tag="xf")
                    ring = nc.sync if cc % 2 == 0 else nc.scalar
                    ring.dma_start(
                        xf[:], dram["xPf"][:, (sg * NT + cc) * QW:
                                           (sg * NT + cc + 1) * QW])
                    nc.tensor.matmul(dtp[:], wdtv[:, cc * HPC:(cc + 1) * HPC],
                                     xf[:],
                                     start=(cc == 0), stop=(cc == NT - 1))
                # DVE copy: keeps the in-order scalar queue free for DMA
                # issues and the dyn chain
                nc.vector.tensor_copy(dt_sb[:, sg * QW:(sg + 1) * QW], dtp[:])
                nc.scalar.dma_start(
                    xs[sg][:], dram["xP"][:, sg * NT * QW:(sg + 1) * NT * QW])

            wsrc = {"v": "wvP", "q": "wqP", "k": "wkP"}

            def emit_proj(kind, oi):
                wfull = wp.tile([P, NT * P], BF16, name="wfull", tag="wf")
                nc.sync.dma_start(
                    wfull[:],
                    dram[wsrc[kind]][:, oi * NT * P:(oi + 1) * NT * P])
                for sg in range(NQ):
                    ps = pps.tile([P, QW], F32, name="ps", tag="ps")
                    for cc in range(NT):
                        nc.tensor.matmul(ps[:], wfull[:, cc * P:(cc + 1) * P],
                                         xs[sg][:, cc * QW:(cc + 1) * QW],
                                         start=(cc == 0), stop=(cc == NT - 1))
                    if kind == "v":
                        # scalar engine: its queue reaches these after the
                        # dyn chain, so they never gate the bisection
                        nc.scalar.copy(
                            vT_own[oi][:, sg * QW:(sg + 1) * QW], ps[:])
                    else:
                        pj = pjp.tile([P, QW], F32R, name="pj", tag="pj")
                        nc.scalar.copy(pj[:], ps[:])
                        rh = pps.tile([P, QW], F32, name="rh", tag="ps")
                        nc.tensor.matmul(rh[:], perm_r[:], pj[:],
                                         start=True, stop=True)
                        # gpsimd cannot read PSUM: stage rh through SBUF on
                        # the scalar engine, then do all RoPE elementwise
                        # work on gpsimd (DVE is busy with the bisection and
                        # its in-order queue would pin pjp tiles for ~70us)
                        rhs = pjp.tile([P, QW], F32, name="rhs", tag="pj")
                        nc.scalar.copy(rhs[:], rh[:])
                        t1 = pjp.tile([P, QW], F32, name="t1", tag="pj")
                        nc.gpsimd.tensor_tensor(
                            t1[:], rhs[:], sin_t[:, sg * QW:(sg + 1) * QW],
                            op=AluOpType.mult)
                        t2 = pjp.tile([P, QW], F32, name="t2", tag="pj")
                        nc.gpsimd.tensor_tensor(
                            t2[:], pj[:], cos_t[:, sg * QW:(sg + 1) * QW],
                            op=AluOpType.mult)
                        dstro = (qkro[oi] if kind == "q" else kro[oi])
                        nc.gpsimd.tensor_tensor(
                            dstro[:, sg * QW:(sg + 1) * QW], t1[:], t2[:],
                            op=AluOpType.add)

            # dt chains first (DMA-paced), then the dyn chain + bisection so
            # its scalar/DVE ops sit ahead of all projection copies in the
            # in-order queues; projections follow and overlap the bisection.
            # (Do NOT wrap this in tc.high_priority(): duplicate priorities
            # desync the psum pool-allocation pass from the schedule and
            # produce garbage results.)
            for sg in range(NQ):
                emit_dt(sg)
            emit_dyn_bisect()
            for kind, oi in [("v", 0), ("v", 1),
                             ("q", 0), ("q", 1), ("q", 2), ("q", 3),
                             ("k", 0), ("k", 1)]:
                emit_proj(kind, oi)

        # ---------------- natural-layout v tiles (SBUF resident) --------
        with tc.tile_pool(name="vps", bufs=4, space="PSUM") as vps:
            for i in range(KVPC):
                for cc in range(NT):
                    pt = vps.tile([P, P], F32, name="vt", tag="vt")
                    nc.tensor.transpose(pt[:].bitcast(F32R),
                                        vT_own[i][:, cc * P:(cc + 1) * P],
                                        eye_r[:])
                    nc.scalar.copy(vnat[i][:, cc * P:(cc + 1) * P], pt[:])

        # dynT transposes last in the PE queue before attention: they wait
        # on the DVE bisection, so anything emitted after them would stall
        # the in-order PE queue (cost a 122us bubble when emitted early).
        with tc.tile_pool(name="dtp2", bufs=1, space="PSUM") as dtp2:
            dyn_ps = dtp2.tile([P, NT * HPC], F32, name="dyn_ps")
            for cc in range(NT):
                nc.tensor.transpose(
                    dyn_ps[:, cc * HPC:(cc + 1) * HPC].bitcast(F32R),
                    dynrow[:, cc * P:(cc + 1) * P], eye_r[0:HPC, 0:HPC])
            nc.scalar.copy(dynT[:], dyn_ps[:])

    # ---------------- attention (transposed scores) + outproj ----------
    # wo resident: loaded once (not once per query-group), via the
    # Activation DGE ring while the first group's attention runs
    wop = ctx.enter_context(tc.tile_pool(name="wop", bufs=1))
    wos = []
    for ht in range(NT):
        wo = wop.tile([P, HPC * P], F32R, name=f"wo{ht}")
        nc.gpsimd.dma_start(
            wo[:], dram["woP"][:, ht * HPC * P:(ht + 1) * HPC * P])
        wos.append(wo)
    with tc.tile_pool(name="scp", bufs=3, space="PSUM") as scp, \
         tc.tile_pool(name="ovl", bufs=2, space="PSUM") as ovl, \
         tc.tile_pool(name="lpp", bufs=1, space="PSUM") as lpp, \
         tc.tile_pool(name="ptp", bufs=3) as ptp, \
         tc.tile_pool(name="atn", bufs=8) as atn, \
         tc.tile_pool(name="lnb", bufs=2) as lnb, \
         tc.tile_pool(name="lnv", bufs=2) as lnv, \
         tc.tile_pool(name="oub", bufs=4) as oub, \
         tc.tile_pool(name="ops", bufs=2, space="PSUM") as ops:
        for grp in range(NQ):
            base = grp * QW
            tiles = list(range(grp * 4, grp * 4 + 4))
            jmax = max(extc[t] for t in tiles)
            at_grp = {}
            for h in range(HPC):
                kv = h // GROUPS
                ovp = ovl.tile([P, QW], F32, name="ovp", tag="ovp")
                lps = lpp.tile([1, QW], F32, name="lps", tag="lps")

                qlos = []
                for j in range(jmax):
                    acts = [t for t in tiles if j < extc[t]]
                    assert acts == tiles[-len(acts):], \
                        "active tiles must be a suffix of the group"
                    qlos.append(acts[0] * P - base)

                def emit_score(j):
                    qlo = qlos[j]
                    sc = scp.tile([P, QW], F32, name="sc", tag="sc")
                    nc.tensor.matmul(
                        sc[:, qlo:QW], kro[kv][:, j * P:(j + 1) * P],
                        qkro[h][:, base + qlo:base + QW],
                        start=True, stop=True, skip_group_check=True)
                    for t in tiles:
                        if j >= extc[t]:
                            continue
                        st = blkstate[t][j]
                        if st.startswith("V"):
                            vi = int(st[2:])
                            off = t * P - base
                            nc.vector.tensor_tensor(
                                sc[:, off:off + P], sc[:, off:off + P],
                                varblkT[:, vi * P:(vi + 1) * P],
                                op=AluOpType.add)
                    pt = ptp.tile([P, QW], BF16, name="pt", tag="pt")
                    nc.scalar.activation(
                        pt[:, qlo:QW], sc[:, qlo:QW], AF.Exp,
                        bias=dynT[:, j * HPC + h:j * HPC + h + 1])
                    return pt

                # software-pipeline by two chunks: emit chunk j+1/j+2's
                # score matmuls before chunk j's l/av matmuls so the PE
                # works through the exp latency instead of waiting on it.
                ptq = [emit_score(0)]
                if jmax > 1:
                    ptq.append(emit_score(1))
                for j in range(jmax):
                    pt, qlo = ptq.pop(0), qlos[j]
                    if j + 2 < jmax:
                        ptq.append(emit_score(j + 2))
                    nc.tensor.matmul(
                        lps[:, qlo:QW], onescol_b[:], pt[:, qlo:QW],
                        start=(j == 0), stop=(j == jmax - 1),
                        skip_group_check=True)
                    nc.tensor.matmul(
                        ovp[:, qlo:QW], vnat[kv][:, j * P:(j + 1) * P],
                        pt[:, qlo:QW],
                        start=(j == 0), stop=(j == jmax - 1),
                        skip_group_check=True)
                lsb = lnv.tile([1, QW], F32, name="lsb", tag="lv")
                nc.scalar.copy(lsb[:], lps[:])
                nc.sync.dma_start(l_d[h:h + 1, base:base + QW], lsb[:])
                linv = lnv.tile([1, QW], F32, name="linv", tag="lv")
                nc.vector.reciprocal_approx_fast(linv[:], lps[:])
                lbc = lnb.tile([P, QW], F32, name="lbc", tag="lb")
                nc.gpsimd.partition_broadcast(lbc[:], linv[:])
                at = atn.tile([P, QW], F32R, name="at", tag="at")
                nc.vector.tensor_tensor(at[:], ovp[:], lbc[:],
                                        op=AluOpType.mult)
                at_grp[h] = at
            for ht in range(NT):
                op = ops.tile([P, QW], F32, name="op", tag="op")
                for h in range(HPC):
                    nc.tensor.matmul(op[:], wos[ht][:, h * P:(h + 1) * P],
                                     at_grp[h][:], start=(h == 0),
                                     stop=(h == HPC - 1))
                osb = oub.tile([P, QW], F32, name="osb", tag="ob")
                # alternate the psum drain between scalar and DVE so
                # neither in-order queue delays the next group's exps
                if ht % 2 == 0:
                    nc.scalar.copy(osb[:], op[:])
                else:
                    nc.vector.tensor_copy(osb[:], op[:])
                nc.sync.dma_start(
                    outT_d[ht * P:(ht + 1) * P, base:base + QW], osb[:])
    ctx.close()


def _host_prep(hidden_states, cos, sin, attention_mask, Wq, Wk, Wv, A, Wdt, Wo):
    eye = np.eye(P, dtype=np.float32)
    perm = np.zeros((P, P), dtype=np.float32)
    for j in range(64):
        perm[j + 64, j] = -1.0
        perm[j, j + 64] = 1.0

    def pack_w(wT, nblk):
        # wT [HID, nblk*P] f32 -> [P, nblk*NT*P] bf16:
        # [p, (oi*NT+c)*P+f] = wT[c*P+p, oi*P+f]
        w4 = wT.reshape(NT, P, nblk, P)            # [c, p, oi, f]
        return np.ascontiguousarray(
            w4.transpose(1, 2, 0, 3).reshape(P, nblk * NT * P)
        ).astype(np.float32)

    in_maps = []
    blkstates = []
    for c in range(NCORES):
        b, g = divmod(c, 4)
        heads = list(range(4 * g, 4 * g + 4))
        # x packed: [p, ((sg*NT)+c)*QW+f] = x[b][sg*QW+f, c*P+p]
        xb = np.asarray(hidden_states[b], dtype=np.float32)
        xP = np.ascontiguousarray(
            xb.reshape(NQ, QW, NT, P).transpose(3, 0, 2, 1)
            .reshape(P, NQ * NT * QW))
        wqT = (Wq[4 * g * D:(4 * g + 4) * D] * np.float32(SCALING)).T
        wkT = Wk[2 * g * D:(2 * g + 2) * D].T
        wvT = Wv[2 * g * D:(2 * g + 2) * D].T
        wqP = pack_w(np.ascontiguousarray(wqT), HPC)
        wkP = pack_w(np.ascontiguousarray(wkT), KVPC)
        wvP = pack_w(np.ascontiguousarray(wvT), KVPC)
        wdtvT = np.ascontiguousarray(
            (Wdt[heads].astype(np.float64) @ Wv.astype(np.float64))
            .T.astype(np.float32))                 # [HID, 4]
        wdtvP = np.ascontiguousarray(
            wdtvT.reshape(NT, P, HPC).transpose(1, 0, 2).reshape(P, NT * HPC))
        # woP: [p, (ht*HPC+h)*P+f] = WoT[h*P+p, ht*P+f]
        woT = np.ascontiguousarray(Wo[:, 4 * g * D:(4 * g + 4) * D].T)
        woP = np.ascontiguousarray(
            woT.reshape(HPC, P, NT, P).transpose(1, 2, 0, 3)
            .reshape(P, NT * HPC * P))
        acol = A[heads].astype(np.float32).reshape(HPC, 1)
        cosT = np.ascontiguousarray(cos[b].T)
        sinT = np.ascontiguousarray(sin[b].T)
        m = attention_mask[b, 0]
        mb = np.asarray(m).reshape(NT, P, NT, P)
        blkrows = []
        varlist = []
        varkeys = {}
        for t in range(NT):
            row = []
            for j in range(NT):
                blkv = mb[t, :, j, :]
                if np.all(blkv == 0):
                    row.append("Z")
                elif np.all(blkv <= -1e30):
                    row.append("M")
                else:
                    bT = np.ascontiguousarray(
                        np.maximum(blkv, -BIG).T)  # [key, query]
                    kk = bT.tobytes()
                    if kk not in varkeys:
                        varkeys[kk] = len(varlist)
                        varlist.append(bT)
                    row.append(f"V:{varkeys[kk]}")
            # interior M blocks (before a later non-M block) become varying
            nz = [j for j in range(NT) if row[j] != "M"]
            lim = (max(nz) + 1) if nz else 0
            for j in range(lim):
                if row[j] == "M":
                    bT = np.full((P, P), -BIG, np.float32)
                    kk = bT.tobytes()
                    if kk not in varkeys:
                        varkeys[kk] = len(varlist)
                        varlist.append(bT)
                    row[j] = f"V:{varkeys[kk]}"
            blkrows.append(tuple(row))
        if len(varlist) > 8:
            raise NotImplementedError("too many varying mask blocks")
        varblkT = np.zeros((P, max(len(varlist), 1) * P), dtype=np.float32)
        for vi, blkv in enumerate(varlist):
            varblkT[:, vi * P:(vi + 1) * P] = blkv
        blkstate = tuple(blkrows)
        in_maps.append({
            "xP": xP.astype(BF16NP), "xPf": xP,
            "wqP": wqP.astype(BF16NP),
            "wkP": wkP.astype(BF16NP), "wvP": wvP.astype(BF16NP),
            "wdtvPr": wdtvP, "woP": woP, "cosT": cosT,
            "sinT": sinT, "acol": acol, "eye": eye, "perm": perm,
            "varblkT": varblkT,
        })
        blkstates.append(blkstate)
    if len(set(blkstates)) != 1:
        raise NotImplementedError("mask structure differs across batches")
    return in_maps, blkstates[0]


def _softplus64(x):
    x = x.astype(np.float64)
    return np.log1p(np.exp(-np.abs(x))) + np.maximum(x, 0)


def _repair_rows(out, bad, inputs):
    """Recompute rows flagged bad [B, S] with faithful numpy reference math."""
    if not bad.any():
        return out
    hs = inputs["hidden_states"]; cos = inputs["cos"]; sin = inputs["sin"]
    am = inputs["attention_mask"]; Wq = inputs["Wq"]; Wk = inputs["Wk"]
    Wv = inputs["Wv"]; A = inputs["A"]; Wdt = inputs["Wdt"]; Wo = inputs["Wo"]

    def rope(x, c, s):
        x1, x2 = x[..., :D // 2], x[..., D // 2:]
        return x * c + np.concatenate([-x2, x1], axis=-1) * s

    for b in range(B):
        rows = np.where(bad[b])[0]
        if len(rows) == 0:
            continue
        x = hs[b].astype(np.float32)
        k = (x @ Wk.T).reshape(S, KV, D)
        v = (x @ Wv.T).reshape(S, KV, D)
        k = rope(k, cos[b][:, None, :], sin[b][:, None, :])
        v_flat = v.reshape(S, KV * D)
        dt = v_flat @ Wdt.T
        dyn = np.exp(A[None, :] * _softplus64(dt)).astype(np.float32).T
        kth = np.sort(dyn, axis=-1)[:, NUM_DYN - 1:NUM_DYN]
        dmask = np.where(dyn < kth, MIN, dyn).astype(np.float32)
        for s_i in rows:
            q_row = (x[s_i] @ Wq.T).reshape(H, D)
            q_row = rope(q_row, cos[b][s_i][None, :], sin[b][s_i][None, :])
            attn_row = np.zeros((H, D), dtype=np.float32)
            for h in range(H):
                kvh = h // GROUPS
                sc = ((q_row[h] @ k[:, kvh].T) * np.float32(SCALING)
                      + (dmask[h] + am[b, 0, s_i])).astype(np.float32)
                w = np.exp(sc - sc.max())
                w = (w / w.sum()).astype(np.float32)
                attn_row[h] = w @ v[:, kvh]
            out[b, s_i] = attn_row.reshape(H * D) @ Wo.T
    return out


def kernel(**inputs):
    inputs = {k: np.asarray(v) for k, v in inputs.items()}
    in_maps, blkstate = _host_prep(**inputs)
    nc = _build_program(blkstate)
    res = run_bass_kernel_spmd(nc, in_maps, list(range(NCORES)))
    out = np.zeros((B, S, HID), dtype=np.float32)
    bad = np.zeros((B, S), dtype=bool)
    for c in range(NCORES):
        b = c // 4
        out[b] += res.results[c]["outT"].T
        bad[b] |= (res.results[c]["l_out"] == 0).any(axis=0)
    bad |= ~np.isfinite(out).all(axis=2)
    out = _repair_rows(out, bad, inputs)
    return out


# revision 68
# speedup vs baseline: 1.9430x; 1.0643x over previous
"""DogeDynamicMaskAttention Trainium2 kernel (transposed-scores redesign).

Sharding: 8 cores = 2 batches x 4 head-groups. Core c: batch b=c//4,
head-group g=c%4 -> heads [4g..4g+4), kv heads {2g, 2g+1}.

Design vs previous baseline:
  - scores computed TRANSPOSED [keys, queries]: the dynamic mask row is a
    per-partition (per-key) bias folded into the exp activation for free;
    the P-matrix transposes + f32r casts of the old layout vanish; the
    attn@v matmul consumes exp output directly (keys on partitions).
  - l (softmax denom) via a ones-column stationary matmul accumulated in
    psum; normalize out tiles with reciprocal + gpsimd partition_broadcast
    + one DVE multiply per (head, query-group).
  - projections in bf16 (x and Wq/Wk/Wv/Wdt host-packed contiguous, so
    DMA is large-descriptor); x resident in SBUF, read once.
  - v natural-layout tiles kept in SBUF (no DRAM bounce).
  - dyn/kth bisection identical to baseline (31-step float-bit bisection),
    overlapped under the q/k/v projections; dynT obtained by tiny PE
    transposes instead of a DRAM round trip.
  - fully-masked (degenerate) rows: l==0 detected on host via l output,
    recomputed faithfully in numpy (expected ~1 row per batch*head).
"""
import sys
import numpy as np
import ml_dtypes

BF16NP = ml_dtypes.bfloat16

sys.path.insert(0, "/root/.axon_site/_ro/trn_rl_repo")

import concourse.bass as bass  # noqa: E402,F401
from concourse import bacc  # noqa: E402
import concourse.tile as tile  # noqa: E402
import concourse.mybir as mybir  # noqa: E402
from concourse.bass_utils import run_bass_kernel_spmd  # noqa: E402
from concourse.alu_op_type import AluOpType  # noqa: E402

F32 = mybir.dt.float32
F32R = mybir.dt.float32r
BF16 = mybir.dt.bfloat16
I32 = mybir.dt.int32
AF = mybir.ActivationFunctionType
AX = mybir.AxisListType.X

B, S, HID = 2, 2048, 2048
H, KV, D = 16, 8, 128
HPC, KVPC = 4, 2
GROUPS = H // KV
NUM_DYN = S // 2
SCALING = D ** -0.5
MIN = float(np.finfo(np.float32).min)
BIG = 1.7e38
P = 128
NT = S // P          # 16
NQ = 4
QW = S // NQ         # 512
NCORES = 8

_cache = {}


def _build_program(blkstate):
    key = ("nc", blkstate)
    if key in _cache:
        return _cache[key]
    nvar = _num_varblocks(blkstate)
    nc = bacc.Bacc("TRN2", target_bir_lowering=False, debug=False,
                   num_devices=NCORES)
    dram = {}
    for name, shape, dt in [
            ("xP", [P, NQ * NT * QW], BF16),
            ("xPf", [P, NQ * NT * QW], F32R),
            ("wqP", [P, HPC * NT * P], BF16),
            ("wkP", [P, KVPC * NT * P], BF16),
            ("wvP", [P, KVPC * NT * P], BF16),
            ("wdtvPr", [P, NT * HPC], F32R),
            ("woP", [P, NT * HPC * P], F32),
            ("cosT", [D, S], F32), ("sinT", [D, S], F32),
            ("acol", [HPC, 1], F32),
            ("eye", [P, P], F32), ("perm", [P, P], F32),
            ("varblkT", [P, max(nvar, 1) * P], F32)]:
        dram[name] = nc.dram_tensor(name, shape, dt, kind="ExternalInput").ap()
    outT_d = nc.dram_tensor("outT", [HID, S], F32, kind="ExternalOutput").ap()
    l_d = nc.dram_tensor("l_out", [HPC, S], F32, kind="ExternalOutput").ap()

    with tile.TileContext(nc) as tc:
        _emit(nc, tc, dram, outT_d, l_d, blkstate)
    nc.compile()
    _cache[key] = nc
    return nc


def _num_varblocks(blkstate):
    n = 0
    for t in range(NT):
        for j in range(NT):
            if blkstate[t][j].startswith("V"):
                n = max(n, int(blkstate[t][j][2:]) + 1)
    return n


def _emit(nc, tc, dram, outT_d, l_d, blkstate):
    from contextlib import ExitStack
    ctx = ExitStack()

    # per-tile computed extent (in key chunks): chunks j < extc[t] participate
    extc = []
    for t in range(NT):
        nz = [j for j in range(NT) if blkstate[t][j] != "M"]
        assert nz and min(nz) == 0, "chunk 0 must be active for every tile"
        extc.append(max(nz) + 1)

    consts = ctx.enter_context(tc.tile_pool(name="consts", bufs=1))

    # dt-critical consts first on the sync ring so the dt pass starts
    # immediately; all bulk loads go on the Activation DGE ring.
    wdtv = consts.tile([P, NT * HPC], F32R, name="c_wdtv")
    nc.sync.dma_start(wdtv[:], dram["wdtvPr"])
    acol_t = consts.tile([HPC, 1], F32, name="c_acol")
    nc.sync.dma_start(acol_t[:], dram["acol"])
    onescol_b = consts.tile([P, 1], BF16, name="onescol")
    nc.vector.memset(onescol_b[:], 1.0)
    kthc = consts.tile([HPC, 1], F32, name="kthc")
    nc.vector.memset(kthc[:], float(NUM_DYN) - 0.5)

    eye_r = consts.tile([P, P], F32R, name="cr_eye")
    perm_r = consts.tile([P, P], F32R, name="cr_perm")
    nvar = _num_varblocks(blkstate)
    varblkT = consts.tile([P, max(nvar, 1) * P], F32, name="c_varblkT")

    csp = ctx.enter_context(tc.tile_pool(name="csp", bufs=1))
    cos_t = csp.tile([D, S], F32, name="cos_t")
    sin_t = csp.tile([D, S], F32, name="sin_t")

    cstg = ctx.enter_context(tc.tile_pool(name="cstg", bufs=2))

    def side_consts():
        # side consts on the Activation ring behind the xs loads: keeps
        # their 2.5MB out of the critical first microseconds where the
        # dt x-stream needs every queue
        nc.scalar.dma_start(cos_t[:], dram["cosT"])
        nc.scalar.dma_start(sin_t[:], dram["sinT"])
        nc.scalar.dma_start(varblkT[:], dram["varblkT"])
        for nm, dst in [("eye", eye_r), ("perm", perm_r)]:
            t = cstg.tile([P, P], F32, name=f"s_{nm}", tag="s")
            nc.scalar.dma_start(t[:], dram[nm])
            nc.scalar.copy(dst[:], t[:])

    act = ctx.enter_context(tc.tile_pool(name="act", bufs=1))
    qkro = [act.tile([P, S], F32R, name=f"qro{h}") for h in range(HPC)]
    kro = [act.tile([P, S], F32R, name=f"kro{i}") for i in range(KVPC)]
    vnat = [act.tile([P, NT * P], BF16, name=f"vnat{i}") for i in range(KVPC)]
    dynT = act.tile([P, NT * HPC], F32, name="dynT")

    with ExitStack() as ctx1:
        xsp = ctx1.enter_context(tc.tile_pool(name="xsp", bufs=1))
        xs = [xsp.tile([P, NT * QW], BF16, name=f"xs{sg}")
              for sg in range(NQ)]
        vop = ctx1.enter_context(tc.tile_pool(name="vop", bufs=1))
        vT_own = [vop.tile([P, S], F32R, name=f"vTown{i}") for i in range(KVPC)]
        dt_sb = vop.tile([HPC, S], F32, name="dt_sb")

        # ---- dt pass (f32-accurate: decides the kth mask set) merged ----
        # with the projections; dt chains interleave with v-projections so
        # the PE stays fed while dt's x stream arrives. The dyn chain +
        # bisection is emitted right after the last dt chain so the scalar
        # and DVE queues reach it early (both are in-order engines).
        dyq = ctx1.enter_context(tc.tile_pool(name="dyq", bufs=1))
        kth_f = dyq.tile([HPC, 1], I32, name="kth_f")
        dynrow = dyq.tile([HPC, S], F32R, name="dynrow")
        dyn_t = dyq.tile([HPC, S], F32, name="dyn_t")
        work = dyq.tile([HPC, S], F32, name="work")
        # work is dead after the dyn chain; reuse its storage for the
        # bisection scratch (bf16 view) and later the penalty tile
        scr = work[:].bitcast(BF16)[:, 0:S]
        pen = work

        def emit_dyn_bisect():
            nc.scalar.activation(work[:], dt_sb[:], AF.Exp)
            nc.scalar.activation(work[:], work[:], AF.Ln, bias=1.0)
            nc.scalar.activation(dyn_t[:], work[:], AF.Exp, scale=acol_t[:])
            lo = dyq.tile([HPC, 1], I32, name="lo")
            hi = dyq.tile([HPC, 1], I32, name="hi")
            mid = dyq.tile([HPC, 1], I32, name="mid")
            dlt = dyq.tile([HPC, 1], I32, name="dlt")
            cges = dyq.tile([HPC, 1], I32, name="cges")
            cltv = dyq.tile([HPC, 1], I32, name="cltv")
            cnt = dyq.tile([HPC, 1], F32, name="cnt")
            nc.vector.memset(lo[:], 0)
            nc.vector.memset(hi[:], 0x7F800000)
            for _ in range(31):
                # mid = (lo + hi) >>> 1 (bit values < 2^31 so the unsigned
                # average is exact under logical shift)
                nc.vector.tensor_tensor(mid[:], hi[:], lo[:],
                                        op=AluOpType.add)
                nc.vector.tensor_scalar(mid[:], mid[:], 1, None,
                                        op0=AluOpType.logical_shift_right)
                nc.vector.tensor_scalar(scr, dyn_t[:],
                                        mid[:, 0:1].bitcast(F32), 0.0,
                                        op0=AluOpType.is_lt,
                                        op1=AluOpType.add,
                                        accum_out=cnt[:])
                nc.vector.tensor_scalar(cges[:], kthc[:], cnt[:, 0:1], None,
                                        op0=AluOpType.is_lt)
                nc.vector.tensor_scalar(cltv[:], kthc[:], cnt[:, 0:1], None,
                                        op0=AluOpType.is_ge)
                nc.vector.copy_predicated(hi[:], cges[:], mid[:])
                nc.vector.copy_predicated(lo[:], cltv[:], mid[:])
            nc.vector.tensor_copy(kth_f[:], lo[:])
            nc.vector.tensor_scalar(pen[:], dyn_t[:],
                                    kth_f[:, 0:1].bitcast(F32), -BIG,
                                    op0=AluOpType.is_lt, op1=AluOpType.mult)
            nc.vector.tensor_tensor(dynrow[:], dyn_t[:], pen[:],
                                    op=AluOpType.add)

        with tc.tile_pool(name="dps", bufs=2, space="PSUM") as dps, \
             tc.tile_pool(name="dtx", bufs=2) as dtx, \
             tc.tile_pool(name="wp", bufs=2) as wp, \
             tc.tile_pool(name="pjp", bufs=4) as pjp, \
             tc.tile_pool(name="pps", bufs=6, space="PSUM") as pps:

            def emit_dt(sg):
                dtp = dps.tile([HPC, QW], F32, name="dtp", tag="dtp")
                for cc in range(NT):
                    xf = dtx.tile([P, QW], F32R, name="xf", tag="xf")
                    # all xf chunks on the sync ring, AHEAD of the wfull
                    # loads: the scheduler's DMA model then completes the
                    # dt chains before the projections instead of smearing
                    # them across the whole phase (bisection started ~120us
                    # late otherwise)
                    nc.sync.dma_start(
                        xf[:], dram["xPf"][:, (sg * NT + cc) * QW:
                                           (sg * NT + cc + 1) * QW])
                    nc.tensor.matmul(dtp[:], wdtv[:, cc * HPC:(cc + 1) * HPC],
                                     xf[:],
                                     start=(cc == 0), stop=(cc == NT - 1))
                # DVE copy: keeps the in-order scalar queue free for DMA
                # issues and the dyn chain
                nc.vector.tensor_copy(dt_sb[:, sg * QW:(sg + 1) * QW], dtp[:])
                nc.scalar.dma_start(
                    xs[sg][:], dram["xP"][:, sg * NT * QW:(sg + 1) * NT * QW])

            wsrc = {"v": "wvP", "q": "wqP", "k": "wkP"}

            def emit_proj(kind, oi):
                wfull = wp.tile([P, NT * P], BF16, name="wfull", tag="wf")
                nc.sync.dma_start(
                    wfull[:],
                    dram[wsrc[kind]][:, oi * NT * P:(oi + 1) * NT * P])
                for sg in range(NQ):
                    ps = pps.tile([P, QW], F32, name="ps", tag="ps")
                    for cc in range(NT):
                        nc.tensor.matmul(ps[:], wfull[:, cc * P:(cc + 1) * P],
                                         xs[sg][:, cc * QW:(cc + 1) * QW],
                                         start=(cc == 0), stop=(cc == NT - 1))
                    if kind == "v":
                        # scalar engine: its queue reaches these after the
                        # dyn chain, so they never gate the bisection
                        nc.scalar.copy(
                            vT_own[oi][:, sg * QW:(sg + 1) * QW], ps[:])
                    else:
                        pj = pjp.tile([P, QW], F32R, name="pj", tag="pj")
                        nc.scalar.copy(pj[:], ps[:])
                        rh = pps.tile([P, QW], F32, name="rh", tag="ps")
                        nc.tensor.matmul(rh[:], perm_r[:], pj[:],
                                         start=True, stop=True)
                        # gpsimd cannot read PSUM: stage rh through SBUF on
                        # the scalar engine, then do all RoPE elementwise
                        # work on gpsimd (DVE is busy with the bisection and
                        # its in-order queue would pin pjp tiles for ~70us)
                        rhs = pjp.tile([P, QW], F32, name="rhs", tag="pj")
                        nc.scalar.copy(rhs[:], rh[:])
                        t1 = pjp.tile([P, QW], F32, name="t1", tag="pj")
                        nc.gpsimd.tensor_tensor(
                            t1[:], rhs[:], sin_t[:, sg * QW:(sg + 1) * QW],
                            op=AluOpType.mult)
                        t2 = pjp.tile([P, QW], F32, name="t2", tag="pj")
                        nc.gpsimd.tensor_tensor(
                            t2[:], pj[:], cos_t[:, sg * QW:(sg + 1) * QW],
                            op=AluOpType.mult)
                        dstro = (qkro[oi] if kind == "q" else kro[oi])
                        nc.gpsimd.tensor_tensor(
                            dstro[:, sg * QW:(sg + 1) * QW], t1[:], t2[:],
                            op=AluOpType.add)

            # dt chains first (DMA-paced), then the dyn chain + bisection so
            # its scalar/DVE ops sit ahead of all projection copies in the
            # in-order queues; projections follow and overlap the bisection.
            # (Do NOT wrap this in tc.high_priority(): duplicate priorities
            # desync the psum pool-allocation pass from the schedule and
            # produce garbage results.)
            for sg in range(NQ):
                emit_dt(sg)
            emit_dyn_bisect()
            side_consts()
            for kind, oi in [("v", 0), ("v", 1),
                             ("q", 0), ("q", 1), ("q", 2), ("q", 3),
                             ("k", 0), ("k", 1)]:
                emit_proj(kind, oi)

        # ---------------- natural-layout v tiles (SBUF resident) --------
        with tc.tile_pool(name="vps", bufs=4, space="PSUM") as vps:
            for i in range(KVPC):
                for cc in range(NT):
                    pt = vps.tile([P, P], F32, name="vt", tag="vt")
                    nc.tensor.transpose(pt[:].bitcast(F32R),
                                        vT_own[i][:, cc * P:(cc + 1) * P],
                                        eye_r[:])
                    nc.scalar.copy(vnat[i][:, cc * P:(cc + 1) * P], pt[:])

        # dynT transposes last in the PE queue before attention: they wait
        # on the DVE bisection, so anything emitted after them would stall
        # the in-order PE queue (cost a 122us bubble when emitted early).
        with tc.tile_pool(name="dtp2", bufs=1, space="PSUM") as dtp2:
            dyn_ps = dtp2.tile([P, NT * HPC], F32, name="dyn_ps")
            for cc in range(NT):
                nc.tensor.transpose(
                    dyn_ps[:, cc * HPC:(cc + 1) * HPC].bitcast(F32R),
                    dynrow[:, cc * P:(cc + 1) * P], eye_r[0:HPC, 0:HPC])
            nc.scalar.copy(dynT[:], dyn_ps[:])

    # ---------------- attention (transposed scores) + outproj ----------
    # wo resident: loaded once (not once per query-group), via the
    # Activation DGE ring while the first group's attention runs
    wop = ctx.enter_context(tc.tile_pool(name="wop", bufs=1))
    wos = []
    for ht in range(NT):
        wo = wop.tile([P, HPC * P], F32R, name=f"wo{ht}")
        nc.gpsimd.dma_start(
            wo[:], dram["woP"][:, ht * HPC * P:(ht + 1) * HPC * P])
        wos.append(wo)
    with tc.tile_pool(name="scp", bufs=3, space="PSUM") as scp, \
         tc.tile_pool(name="ovl", bufs=2, space="PSUM") as ovl, \
         tc.tile_pool(name="lpp", bufs=1, space="PSUM") as lpp, \
         tc.tile_pool(name="ptp", bufs=3) as ptp, \
         tc.tile_pool(name="atn", bufs=8) as atn, \
         tc.tile_pool(name="lnb", bufs=2) as lnb, \
         tc.tile_pool(name="lnv", bufs=2) as lnv, \
         tc.tile_pool(name="oub", bufs=4) as oub, \
         tc.tile_pool(name="ops", bufs=2, space="PSUM") as ops:
        for grp in range(NQ):
            base = grp * QW
            tiles = list(range(grp * 4, grp * 4 + 4))
            jmax = max(extc[t] for t in tiles)
            at_grp = {}
            for h in range(HPC):
                kv = h // GROUPS
                ovp = ovl.tile([P, QW], F32, name="ovp", tag="ovp")
                lps = lpp.tile([1, QW], F32, name="lps", tag="lps")

                qlos = []
                for j in range(jmax):
                    acts = [t for t in tiles if j < extc[t]]
                    assert acts == tiles[-len(acts):], \
                        "active tiles must be a suffix of the group"
                    qlos.append(acts[0] * P - base)

                def emit_score(j):
                    qlo = qlos[j]
                    sc = scp.tile([P, QW], F32, name="sc", tag="sc")
                    nc.tensor.matmul(
                        sc[:, qlo:QW], kro[kv][:, j * P:(j + 1) * P],
                        qkro[h][:, base + qlo:base + QW],
                        start=True, stop=True, skip_group_check=True)
                    for t in tiles:
                        if j >= extc[t]:
                            continue
                        st = blkstate[t][j]
                        if st.startswith("V"):
                            vi = int(st[2:])
                            off = t * P - base
                            nc.vector.tensor_tensor(
                                sc[:, off:off + P], sc[:, off:off + P],
                                varblkT[:, vi * P:(vi + 1) * P],
                                op=AluOpType.add)
                    pt = ptp.tile([P, QW], BF16, name="pt", tag="pt")
                    nc.scalar.activation(
                        pt[:, qlo:QW], sc[:, qlo:QW], AF.Exp,
                        bias=dynT[:, j * HPC + h:j * HPC + h + 1])
                    return pt

                # software-pipeline by two chunks: emit chunk j+1/j+2's
                # score matmuls before chunk j's l/av matmuls so the PE
                # works through the exp latency instead of waiting on it.
                ptq = [emit_score(0)]
                if jmax > 1:
                    ptq.append(emit_score(1))
                for j in range(jmax):
                    pt, qlo = ptq.pop(0), qlos[j]
                    if j + 2 < jmax:
                        ptq.append(emit_score(j + 2))
                    nc.tensor.matmul(
                        lps[:, qlo:QW], onescol_b[:], pt[:, qlo:QW],
                        start=(j == 0), stop=(j == jmax - 1),
                        skip_group_check=True)
                    nc.tensor.matmul(
                        ovp[:, qlo:QW], vnat[kv][:, j * P:(j + 1) * P],
                        pt[:, qlo:QW],
                        start=(j == 0), stop=(j == jmax - 1),
                        skip_group_check=True)
                lsb = lnv.tile([1, QW], F32, name="lsb", tag="lv")
                nc.scalar.copy(lsb[:], lps[:])
                nc.sync.dma_start(l_d[h:h + 1, base:base + QW], lsb[:])
                linv = lnv.tile([1, QW], F32, name="linv", tag="lv")
                nc.vector.reciprocal_approx_fast(linv[:], lps[:])
                lbc = lnb.tile([P, QW], F32, name="lbc", tag="lb")
                nc.gpsimd.partition_broadcast(lbc[:], linv[:])
                at = atn.tile([P, QW], F32R, name="at", tag="at")
                nc.vector.tensor_tensor(at[:], ovp[:], lbc[:],
                                        op=AluOpType.mult)
                at_grp[h] = at
            for ht in range(NT):
                op = ops.tile([P, QW], F32, name="op", tag="op")
                for h in range(HPC):
                    nc.tensor.matmul(op[:], wos[ht][:, h * P:(h + 1) * P],
                                     at_grp[h][:], start=(h == 0),
                                     stop=(h == HPC - 1))
                osb = oub.tile([P, QW], F32, name="osb", tag="ob")
                # alternate the psum drain between scalar and DVE so
                # neither in-order queue delays the next group's exps
                if ht % 2 == 0:
                    nc.scalar.copy(osb[:], op[:])
                else:
                    nc.vector.tensor_copy(osb[:], op[:])
                nc.sync.dma_start(
                    outT_d[ht * P:(ht + 1) * P, base:base + QW], osb[:])
    ctx.close()


def _host_prep(hidden_states, cos, sin, attention_mask, Wq, Wk, Wv, A, Wdt, Wo):
    eye = np.eye(P, dtype=np.float32)
    perm = np.zeros((P, P), dtype=np.float32)
    for j in range(64):
        perm[j + 64, j] = -1.0
        perm[j, j + 64] = 1.0

    def pack_w(wT, nblk):
        # wT [HID, nblk*P] f32 -> [P, nblk*NT*P] bf16:
        # [p, (oi*NT+c)*P+f] = wT[c*P+p, oi*P+f]
        w4 = wT.reshape(NT, P, nblk, P)            # [c, p, oi, f]
        return np.ascontiguousarray(
            w4.transpose(1, 2, 0, 3).reshape(P, nblk * NT * P)
        ).astype(np.float32)

    in_maps = []
    blkstates = []
    for c in range(NCORES):
        b, g = divmod(c, 4)
        heads = list(range(4 * g, 4 * g + 4))
        # x packed: [p, ((sg*NT)+c)*QW+f] = x[b][sg*QW+f, c*P+p]
        xb = np.asarray(hidden_states[b], dtype=np.float32)
        xP = np.ascontiguousarray(
            xb.reshape(NQ, QW, NT, P).transpose(3, 0, 2, 1)
            .reshape(P, NQ * NT * QW))
        wqT = (Wq[4 * g * D:(4 * g + 4) * D] * np.float32(SCALING)).T
        wkT = Wk[2 * g * D:(2 * g + 2) * D].T
        wvT = Wv[2 * g * D:(2 * g + 2) * D].T
        wqP = pack_w(np.ascontiguousarray(wqT), HPC)
        wkP = pack_w(np.ascontiguousarray(wkT), KVPC)
        wvP = pack_w(np.ascontiguousarray(wvT), KVPC)
        wdtvT = np.ascontiguousarray(
            (Wdt[heads].astype(np.float64) @ Wv.astype(np.float64))
            .T.astype(np.float32))                 # [HID, 4]
        wdtvP = np.ascontiguousarray(
            wdtvT.reshape(NT, P, HPC).transpose(1, 0, 2).reshape(P, NT * HPC))
        # woP: [p, (ht*HPC+h)*P+f] = WoT[h*P+p, ht*P+f]
        woT = np.ascontiguousarray(Wo[:, 4 * g * D:(4 * g + 4) * D].T)
        woP = np.ascontiguousarray(
            woT.reshape(HPC, P, NT, P).transpose(1, 2, 0, 3)
            .reshape(P, NT * HPC * P))
        acol = A[heads].astype(np.float32).reshape(HPC, 1)
        cosT = np.ascontiguousarray(cos[b].T)
        sinT = np.ascontiguousarray(sin[b].T)
        m = attention_mask[b, 0]
        mb = np.asarray(m).reshape(NT, P, NT, P)
        blkrows = []
        varlist = []
        varkeys = {}
        for t in range(NT):
            row = []
            for j in range(NT):
                blkv = mb[t, :, j, :]
                if np.all(blkv == 0):
                    row.append("Z")
                elif np.all(blkv <= -1e30):
                    row.append("M")
                else:
                    bT = np.ascontiguousarray(
                        np.maximum(blkv, -BIG).T)  # [key, query]
                    kk = bT.tobytes()
                    if kk not in varkeys:
                        varkeys[kk] = len(varlist)
                        varlist.append(bT)
                    row.append(f"V:{varkeys[kk]}")
            # interior M blocks (before a later non-M block) become varying
            nz = [j for j in range(NT) if row[j] != "M"]
            lim = (max(nz) + 1) if nz else 0
            for j in range(lim):
                if row[j] == "M":
                    bT = np.full((P, P), -BIG, np.float32)
                    kk = bT.tobytes()
                    if kk not in varkeys:
                        varkeys[kk] = len(varlist)
                        varlist.append(bT)
                    row[j] = f"V:{varkeys[kk]}"
            blkrows.append(tuple(row))
        if len(varlist) > 8:
            raise NotImplementedError("too many varying mask blocks")
        varblkT = np.zeros((P, max(len(varlist), 1) * P), dtype=np.float32)
        for vi, blkv in enumerate(varlist):
            varblkT[:, vi * P:(vi + 1) * P] = blkv
        blkstate = tuple(blkrows)
        in_maps.append({
            "xP": xP.astype(BF16NP), "xPf": xP,
            "wqP": wqP.astype(BF16NP),
            "wkP": wkP.astype(BF16NP), "wvP": wvP.astype(BF16NP),
            "wdtvPr": wdtvP, "woP": woP, "cosT": cosT,
            "sinT": sinT, "acol": acol, "eye": eye, "perm": perm,
            "varblkT": varblkT,
        })
        blkstates.append(blkstate)
    if len(set(blkstates)) != 1:
        raise NotImplementedError("mask structure differs across batches")
    return in_maps, blkstates[0]


def _softplus64(x):
    x = x.astype(np.float64)
    return np.log1p(np.exp(-np.abs(x))) + np.maximum(x, 0)


def _repair_rows(out, bad, inputs):
    """Recompute rows flagged bad [B, S] with faithful numpy reference math."""
    if not bad.any():
        return out
    hs = inputs["hidden_states"]; cos = inputs["cos"]; sin = inputs["sin"]
    am = inputs["attention_mask"]; Wq = inputs["Wq"]; Wk = inputs["Wk"]
    Wv = inputs["Wv"]; A = inputs["A"]; Wdt = inputs["Wdt"]; Wo = inputs["Wo"]

    def rope(x, c, s):
        x1, x2 = x[..., :D // 2], x[..., D // 2:]
        return x * c + np.concatenate([-x2, x1], axis=-1) * s

    for b in range(B):
        rows = np.where(bad[b])[0]
        if len(rows) == 0:
            continue
        x = hs[b].astype(np.float32)
        k = (x @ Wk.T).reshape(S, KV, D)
        v = (x @ Wv.T).reshape(S, KV, D)
        k = rope(k, cos[b][:, None, :], sin[b][:, None, :])
        v_flat = v.reshape(S, KV * D)
        dt = v_flat @ Wdt.T
        dyn = np.exp(A[None, :] * _softplus64(dt)).astype(np.float32).T
        kth = np.sort(dyn, axis=-1)[:, NUM_DYN - 1:NUM_DYN]
        dmask = np.where(dyn < kth, MIN, dyn).astype(np.float32)
        for s_i in rows:
            q_row = (x[s_i] @ Wq.T).reshape(H, D)
            q_row = rope(q_row, cos[b][s_i][None, :], sin[b][s_i][None, :])
            attn_row = np.zeros((H, D), dtype=np.float32)
            for h in range(H):
                kvh = h // GROUPS
                sc = ((q_row[h] @ k[:, kvh].T) * np.float32(SCALING)
                      + (dmask[h] + am[b, 0, s_i])).astype(np.float32)
                w = np.exp(sc - sc.max())
                w = (w / w.sum()).astype(np.float32)
                attn_row[h] = w @ v[:, kvh]
            out[b, s_i] = attn_row.reshape(H * D) @ Wo.T
    return out


def kernel(**inputs):
    inputs = {k: np.asarray(v) for k, v in inputs.items()}
    in_maps, blkstate = _host_prep(**inputs)
    nc = _build_program(blkstate)
    res = run_bass_kernel_spmd(nc, in_maps, list(range(NCORES)))
    out = np.zeros((B, S, HID), dtype=np.float32)
    bad = np.zeros((B, S), dtype=bool)
    for c in range(NCORES):
        b = c // 4
        out[b] += res.results[c]["outT"].T
        bad[b] |= (res.results[c]["l_out"] == 0).any(axis=0)
    bad |= ~np.isfinite(out).all(axis=2)
    out = _repair_rows(out, bad, inputs)
    return out
